# revision 1
# baseline (speedup 1.0000x reference)
"""Trainium2 Bass kernel for nn_DeformGCN (6-layer GCN + dense decoder).

Strategy (v2, fp8 DoubleRow):
  - Host precompute from `edges`: dense propagation matrix P (N x N) with
    P[dst,src] += 1/sqrt(deg_s*deg_d) and P[n,n] += 1/deg_n, then P2 = P @ P.
    GCN layer pairs fuse into 3 propagation stages (A, B, C):
      z = P2 @ (h @ (Wa@Wb)) + r (x) (ba@Wb) + 1 (x) bb,  r = P @ 1
    followed by LeakyReLU(0.01).
  - All heavy matmuls run as fp8e4m3 DoubleRow (2 x 128-deep products per
    instruction at 0.5 cycles/row = 4x bf16 MAC rate). Accuracy is restored
    with a hi/lo residual split of both operands; the lo*lo term is dropped
    (3-term scheme, 0.75x the bf16 row count). End-to-end rel err ~2e-3.
  - Activations are split on the fly during PSUM eviction:
      hi = ACT copy(psum, scale=k) -> fp8 ; lo = DVE (psum*k - hi) -> fp8
    LeakyReLU uses  v = 0.99k*relu(z) (ACT)  +  0.01k*z (DVE stt add).
  - Small stages (W45, decoder) run in fp16 (1.0 cycles/row, 10-bit mantissa).
  - Decoder is column-sharded (768 cols/core) and oriented [col_part x batch]
    so its cost is 48*6*16 rows. Features are AllGather'd per local batch (2
    collectives; the first fully overlaps with batch 1's GCN compute).
  - Biases ride the propagation matmuls as extra K=2 DoubleRow slots.
"""

import os
import numpy as np
import ml_dtypes

_STAGE_LIMIT = int(os.environ.get("KSTAGE", "99"))

import bass_rust
import concourse.bass as bass
import concourse.mybir as mybir
import concourse.tile as tile
from concourse.tile import ScopedClock
from concourse.bass_utils import run_bass_kernel_spmd

F8 = mybir.dt.float8e4
F16 = mybir.dt.float16
F32 = mybir.dt.float32
NPF8 = ml_dtypes.float8_e4m3
NPF16 = np.float16
DRM = mybir.MatmulPerfMode.DoubleRow
ALU = mybir.AluOpType
ACTF = mybir.ActivationFunctionType

N_CORES = 8
B = 16
N = 2048
C_IN = 1475
C_PAD = 1536           # 12 * 128
NT = N // 128          # 16 node tiles
CT = C_PAD // 128      # 12 channel tiles
BL = B // N_CORES      # 2 local batches
D_DEC = N * 3          # 6144
DEC_SH = D_DEC // N_CORES  # 768 decoder columns per core
KDEC = D_DEC // 128    # 48 decoder k tiles
ALPHA = 0.01

# scales (powers of two; products of the two operand scales give the PSUM
# scale, the ACT evict rescales to the next storage scale)
SX = 16.0
SW01 = 2048.0
SCT = 32.0
SP2 = 64.0
SCH1 = 128.0
SW23 = 512.0
SCT2 = 512.0
SCT3 = 2048.0
SEXD = 64.0
SBA = SCT * SP2 / SEXD     # 32
SBB = SCT2 * SP2 / SEXD    # 512
SBC = SCT3 * SP2 / SEXD    # 2048
KT = SCT / (SX * SW01)     # 2^-10
K1 = SCH1 / (SCT * SP2)    # 2^-4
K2 = SCT2 / (SCH1 * SW23)  # 2^-7
K3 = SCT3 / (SCT2 * SP2)   # 2^-4
KD = 1.0 / (SCT3 * SP2)    # 2^-17


# ---------------------------------------------------------------------------
# Workaround: this walrus build caps sync-waits per control instruction very
# low, so TileContext's tail drain (which waits on every proc's semaphore)
# fails codegen. Split the global-clock waits into one single-wait
# EventSemaphore each, then emit a bare Drain.
def _patched_drain_and_barrier(self, tick_clock, wait_clock):
    nc = self.nc
    num_to_handle = {h.num: h for h in self.sems.allocated().values()}
    probe = nc.sync.nop(nofuse=True)
    wait_clock.add_sem_waits(probe.ins, ScopedClock({None: tick_clock.global_clock}))
    waits = list(probe.ins.sync_info.on_wait)
    probe.ins.sync_info = bass_rust.SyncInfo(on_wait=[], on_update=[])
    engines = [nc.sync, nc.scalar, nc.vector, nc.tensor, nc.gpsimd]
    for i, w in enumerate(waits):
        h = num_to_handle.get(w.id)
        if h is None:
            raise RuntimeError(f"no sem handle for {w.id} ({w.ant_name})")
        engines[i % len(engines)].wait_ge(h, w.wait_value)
    nc.all_engine_barrier()
    nc.sync.drain()
    assert self.sems is not None
    popped = nc._tile_sem_poison_stack.pop()
    assert popped is self._sem_poison
    nc.clear_and_free_semaphores(list(self.sems.allocated().values()))
    nc.all_engine_barrier()


tile.TileContext._drain_and_barrier = _patched_drain_and_barrier


def _split_multi_waits(nc, max_waits=1):
    """This walrus build rejects instructions carrying more than one sync
    wait. Hoist extra waits into standalone EventSemaphore instructions
    placed immediately before the instruction on the same engine queue."""
    ctr = 0
    for fn in nc.m.functions:
        for bb in fn.blocks:
            insts = bb.instructions
            new = []
            changed = False
            for inst in insts:
                si = inst.sync_info
                waits = list(si.on_wait) if si is not None else []
                if len(waits) > max_waits:
                    changed = True
                    for w in waits[:-max_waits]:
                        ev = bass_rust.InstEventSemaphore(
                            name=f"splitw_{ctr}", ins=[], outs=[]
                        )
                        ctr += 1
                        ev.engine = inst.engine
                        ev.sync_info = bass_rust.SyncInfo(
                            on_wait=[w], on_update=[]
                        )
                        new.append(ev)
                    inst.sync_info = bass_rust.SyncInfo(
                        on_wait=waits[-max_waits:], on_update=list(si.on_update)
                    )
                new.append(inst)
            if changed:
                bb.instructions = new


# ---------------------------------------------------------------------------
def _build_program() -> bass.Bass:
    nc = bass.Bass()

    xhi = nc.declare_dram_parameter("xhi", [BL, NT, 128, CT, 128], F8, isOutput=False)
    xlo = nc.declare_dram_parameter("xlo", [BL, NT, 128, CT, 128], F8, isOutput=False)
    p2thi = nc.declare_dram_parameter("p2thi", [NT, 128, N], F8, isOutput=False)
    p2tlo = nc.declare_dram_parameter("p2tlo", [NT, 128, N], F8, isOutput=False)
    w01hi = nc.declare_dram_parameter("w01hi", [CT, 128, 512], F8, isOutput=False)
    w01lo = nc.declare_dram_parameter("w01lo", [CT, 128, 512], F8, isOutput=False)
    w23hi = nc.declare_dram_parameter("w23hi", [4, 128, 256], F8, isOutput=False)
    w23lo = nc.declare_dram_parameter("w23lo", [4, 128, 256], F8, isOutput=False)
    w45p = nc.declare_dram_parameter("w45p", [2, 128, 3], F16, isOutput=False)
    exdp = nc.declare_dram_parameter("exdp", [2, 2, N], F8, isOutput=False)
    biasA = nc.declare_dram_parameter("biasA", [2, 4, 512], F8, isOutput=False)
    biasB = nc.declare_dram_parameter("biasB", [2, 4, 256], F8, isOutput=False)
    biasC = nc.declare_dram_parameter("biasC", [2, 4, 3], F8, isOutput=False)
    wdp = nc.declare_dram_parameter("wdp", [KDEC, 128, DEC_SH], F16, isOutput=False)
    bdp = nc.declare_dram_parameter("bdp", [128, 6], F32, isOutput=False)
    y_out = nc.declare_dram_parameter("y", [128, 6, B], F32, isOutput=True)

    cc_in = [nc.dram_tensor(f"cc_in{b}", [3, 128, NT], F16) for b in range(BL)]
    cc_out = [
        nc.dram_tensor(f"cc_out{b}", [N_CORES, 3, 128, NT], F16, addr_space="Shared")
        for b in range(BL)
    ]

    with tile.TileContext(nc) as tc:
        with (
            tc.tile_pool(name="const", bufs=1) as constp,
            tc.tile_pool(name="xp", bufs=2) as xpool,
            tc.tile_pool(name="tp", bufs=1) as tpool,
            tc.tile_pool(name="h1p", bufs=1) as h1pool,
            tc.tile_pool(name="t2p", bufs=1) as t2pool,
            tc.tile_pool(name="h3p", bufs=1) as h3pool,
            tc.tile_pool(name="vp", bufs=2) as vpool,
            tc.tile_pool(name="wdpool", bufs=37) as wdpool,
            tc.tile_pool(name="ps", bufs=5, space="PSUM") as ps,
            tc.tile_pool(name="psd", bufs=2, space="PSUM") as psd,
        ):
            # ---- const tiles ----
            w01hi_sb = constp.tile([128, CT, 512], F8, tag="w01hi")
            w01lo_sb = constp.tile([128, CT, 512], F8, tag="w01lo")
            p2thi_sb = constp.tile([128, NT, N], F8, tag="p2thi")
            p2tlo_sb = constp.tile([128, NT, N], F8, tag="p2tlo")
            w23hi_sb = constp.tile([128, 4, 256], F8, tag="w23hi")
            w23lo_sb = constp.tile([128, 4, 256], F8, tag="w23lo")
            w45_sb = constp.tile([128, 2, 3], F16, tag="w45")
            exd_sb = constp.tile([2, 2, N], F8, tag="exd")
            biasA_sb = constp.tile([2, 4, 512], F8, tag="biasA")
            biasB_sb = constp.tile([2, 4, 256], F8, tag="biasB")
            biasC_sb = constp.tile([2, 4, 3], F8, tag="biasC")
            bdp_sb = constp.tile([128, 6], F32, tag="bdp")
            featT_sb = constp.tile([128, B, KDEC], F16, tag="featT")
            y_sb = constp.tile([128, 6, B], F32, tag="ysb")
            t3hi = constp.tile([128, NT * 3], F8, tag="t3hi")
            t3lo = constp.tile([128, NT * 3], F8, tag="t3lo")
            rl5 = constp.tile([128, NT * 3], F16, tag="rl5")
            h5_0 = constp.tile([128, 3, NT], F16, tag="h5_0")
            h5_1 = constp.tile([128, 3, NT], F16, tag="h5_1")
            h5_t = [h5_0, h5_1]

            wd_tiles: list = [None] * KDEC

            # w01 DMAs (start-latency critical)
            nc.sync.dma_start(w01hi_sb[:], w01hi[:].rearrange("c p f -> p c f"))
            nc.sync.dma_start(w01lo_sb[:], w01lo[:].rearrange("c p f -> p c f"))

            def emit_small_consts():
                nc.sync.dma_start(w23hi_sb[:], w23hi[:].rearrange("c p f -> p c f"))
                nc.sync.dma_start(w23lo_sb[:], w23lo[:].rearrange("c p f -> p c f"))
                nc.sync.dma_start(w45_sb[:], w45p[:].rearrange("c p f -> p c f"))
                nc.sync.dma_start(exd_sb[:], exdp[:])
                nc.sync.dma_start(biasA_sb[:], biasA[:])
                nc.sync.dma_start(biasB_sb[:], biasB[:])
                nc.sync.dma_start(biasC_sb[:], biasC[:])
                nc.sync.dma_start(bdp_sb[:], bdp[:])

            # p2t row DMAs: hi rows on the HWDGE (sync) path, lo rows via the
            # otherwise-idle Pool engine's SWDGE to halve HWDGE serialization.
            p2_rows = [(si, 0) for si in range(NT)] + [(si, 1) for si in range(NT)]
            p2_pos = [0]

            def emit_p2(n):
                for _ in range(n):
                    if p2_pos[0] >= len(p2_rows):
                        return
                    si, hl = p2_rows[p2_pos[0]]
                    p2_pos[0] += 1
                    if hl == 0:
                        nc.sync.dma_start(p2thi_sb[:, si, :], p2thi[si])
                    else:
                        nc.gpsimd.dma_start(p2tlo_sb[:, si, :], p2tlo[si])

            wd_pos = [0]

            def emit_wd(n, pool=None, tag="wd"):
                """Stream decoder-weight tiles via Pool SWDGE. Dedicated pool
                holds 38; the last 10 borrow slots of dead GCN tiles."""
                for _ in range(n):
                    kt = wd_pos[0]
                    if kt >= KDEC or (pool is None and kt >= 37):
                        return
                    wd_pos[0] += 1
                    p = pool if pool is not None else wdpool
                    wt = p.tile([128, DEC_SH], F16, tag=tag)
                    nc.gpsimd.dma_start(wt[:], wdp[kt])
                    wd_tiles[kt] = wt

            prefetched = {}

            def grp_dma_for(b, g):
                ghi = xpool.tile([128, 2, CT, 128], F8, tag="xhi")
                glo = xpool.tile([128, 2, CT, 128], F8, tag="xlo")
                nc.sync.dma_start(
                    ghi[:], xhi[b, 2 * g : 2 * g + 2].rearrange(
                        "n p c j -> p n c j"))
                nc.gpsimd.dma_start(
                    glo[:], xlo[b, 2 * g : 2 * g + 2].rearrange(
                        "n p c j -> p n c j"))
                return (ghi, glo)

            def gcn_chain(b):
                # ---- L0: t = x @ W01 (3-term fp8 DR) ----
                # x streams in 2-node-tile groups: hi via HWDGE, lo via SWDGE
                def grp_dma(g):
                    return grp_dma_for(b, g)

                if (b, 0) in prefetched:
                    nxt = [prefetched.pop((b, 0)), prefetched.pop((b, 1))]
                else:
                    nxt = [grp_dma(0), grp_dma(1)]
                if b == 0:
                    emit_small_consts()

                t_hi = tpool.tile([128, NT, 512], F8, tag="t_hi")
                t_lo = tpool.tile([128, NT, 512], F8, tag="t_lo")
                for g in range(NT // 2):
                    ghi, glo = nxt[g % 2]
                    for k in range(2):
                        nt = 2 * g + k
                        pt = ps.tile([128, 512], F32, tag="ps")
                        first = True
                        for p in range(CT // 2):
                            nc.tensor.matmul(
                                pt[:], ghi[:, k, 2 * p : 2 * p + 2, :],
                                w01hi_sb[:, 2 * p : 2 * p + 2, :],
                                start=first, stop=False, perf_mode=DRM,
                            )
                            first = False
                        for p in range(CT // 2):
                            nc.tensor.matmul(
                                pt[:], glo[:, k, 2 * p : 2 * p + 2, :],
                                w01hi_sb[:, 2 * p : 2 * p + 2, :],
                                start=False, stop=False, perf_mode=DRM,
                            )
                        for p in range(CT // 2):
                            nc.tensor.matmul(
                                pt[:], ghi[:, k, 2 * p : 2 * p + 2, :],
                                w01lo_sb[:, 2 * p : 2 * p + 2, :],
                                start=False, stop=(p == CT // 2 - 1),
                                perf_mode=DRM,
                            )
                        nc.scalar.activation(t_hi[:, nt, :], pt[:], ACTF.Copy,
                                             scale=KT)
                        nc.vector.scalar_tensor_tensor(
                            t_lo[:, nt, :], pt[:], KT, t_hi[:, nt, :],
                            ALU.mult, ALU.subtract,
                        )
                    if g + 2 < NT // 2:
                        nxt[g % 2] = grp_dma(g + 2)
                    if b == 0:
                        emit_p2(4)
                    else:
                        emit_wd(5)
                if b == 1:
                    emit_wd(2, xpool, "xhi")
                    emit_wd(2, xpool, "xlo")

                if _STAGE_LIMIT < 1:
                    return
                # ---- stage A: z1 = P2 @ t + bias ; h1 = leaky(z1) ----
                h1hi = h1pool.tile([128, 4, N], F8, tag="h1hi")
                h1lo = h1pool.tile([128, 4, N], F8, tag="h1lo")
                for dc in range(4):
                    dsl = slice(dc * 512, (dc + 1) * 512)
                    for fj in range(4):
                        fsl = slice(fj * 128, (fj + 1) * 128)
                        pt = ps.tile([128, 512], F32, tag="ps")
                        for sp in range(NT // 2):
                            ssl = slice(2 * sp, 2 * sp + 2)
                            nc.tensor.matmul(
                                pt[:], t_hi[:, ssl, fsl], p2thi_sb[:, ssl, dsl],
                                start=(sp == 0), stop=False, perf_mode=DRM,
                            )
                        for sp in range(NT // 2):
                            ssl = slice(2 * sp, 2 * sp + 2)
                            nc.tensor.matmul(
                                pt[:], t_lo[:, ssl, fsl], p2thi_sb[:, ssl, dsl],
                                start=False, stop=False, perf_mode=DRM,
                            )
                        for sp in range(NT // 2):
                            ssl = slice(2 * sp, 2 * sp + 2)
                            nc.tensor.matmul(
                                pt[:], t_hi[:, ssl, fsl], p2tlo_sb[:, ssl, dsl],
                                start=False, stop=False, perf_mode=DRM,
                            )
                        nc.tensor.matmul(
                            pt[:], biasA_sb[:, 0:2, fsl], exd_sb[:, :, dsl],
                            start=False, stop=False, perf_mode=DRM,
                        )
                        nc.tensor.matmul(
                            pt[:], biasA_sb[:, 2:4, fsl], exd_sb[:, :, dsl],
                            start=False, stop=True, perf_mode=DRM,
                        )
                        rl = vpool.tile([128, 512], F16, tag="rl")
                        nc.scalar.activation(rl[:], pt[:], ACTF.Relu,
                                             scale=0.99 * K1)
                        v = vpool.tile([128, 512], F16, tag="v")
                        nc.vector.scalar_tensor_tensor(
                            v[:], pt[:], ALPHA * K1, rl[:], ALU.mult, ALU.add)
                        nc.scalar.activation(h1hi[:, fj, dsl], v[:], ACTF.Copy)
                        nc.vector.tensor_tensor(
                            h1lo[:, fj, dsl], v[:], h1hi[:, fj, dsl],
                            ALU.subtract)
                        if b == 0:
                            emit_wd(3)
                if b == 1:
                    emit_wd(1, tpool, "t_hi")
                    emit_wd(1, tpool, "t_lo")

                if _STAGE_LIMIT < 2:
                    return
                # ---- W23: t2 = h1 @ W23 ----
                t2hi = t2pool.tile([128, NT, 256], F8, tag="t2hi")
                t2lo = t2pool.tile([128, NT, 256], F8, tag="t2lo")
                for nt in range(NT):
                    nsl = slice(nt * 128, (nt + 1) * 128)
                    pt = ps.tile([128, 512], F32, tag="ps")
                    for fp in range(2):
                        fsl = slice(2 * fp, 2 * fp + 2)
                        nc.tensor.matmul(
                            pt[:, 0:256], h1hi[:, fsl, nsl], w23hi_sb[:, fsl, :],
                            start=(fp == 0), stop=False, perf_mode=DRM,
                        )
                    for fp in range(2):
                        fsl = slice(2 * fp, 2 * fp + 2)
                        nc.tensor.matmul(
                            pt[:, 0:256], h1lo[:, fsl, nsl], w23hi_sb[:, fsl, :],
                            start=False, stop=False, perf_mode=DRM,
                        )
                    for fp in range(2):
                        fsl = slice(2 * fp, 2 * fp + 2)
                        nc.tensor.matmul(
                            pt[:, 0:256], h1hi[:, fsl, nsl], w23lo_sb[:, fsl, :],
                            start=False, stop=(fp == 1), perf_mode=DRM,
                        )
                    nc.scalar.activation(t2hi[:, nt, :], pt[:, 0:256], ACTF.Copy,
                                         scale=K2)
                    nc.vector.scalar_tensor_tensor(
                        t2lo[:, nt, :], pt[:, 0:256], K2, t2hi[:, nt, :],
                        ALU.mult, ALU.subtract,
                    )
                if b == 1:
                    emit_wd(1, h1pool, "h1hi")
                    emit_wd(1, h1pool, "h1lo")
                    # featT for batch-0 collective (done by now)
                    for c in range(3):
                        nc.sync.dma_start(
                            featT_sb[:, 0:B:BL, c * NT : (c + 1) * NT],
                            cc_out[0][:, c].rearrange("core p n -> p core n"),
                        )

                if _STAGE_LIMIT < 3:
                    return
                # ---- stage B: z3 = P2 @ t2 + bias ; h3 = leaky(z3) ----
                h3 = h3pool.tile([128, 2, N], F16, tag="h3")
                for dc in range(4):
                    dsl = slice(dc * 512, (dc + 1) * 512)
                    for fj in range(2):
                        fsl = slice(fj * 128, (fj + 1) * 128)
                        pt = ps.tile([128, 512], F32, tag="ps")
                        for sp in range(NT // 2):
                            ssl = slice(2 * sp, 2 * sp + 2)
                            nc.tensor.matmul(
                                pt[:], t2hi[:, ssl, fsl], p2thi_sb[:, ssl, dsl],
                                start=(sp == 0), stop=False, perf_mode=DRM,
                            )
                        for sp in range(NT // 2):
                            ssl = slice(2 * sp, 2 * sp + 2)
                            nc.tensor.matmul(
                                pt[:], t2lo[:, ssl, fsl], p2thi_sb[:, ssl, dsl],
                                start=False, stop=False, perf_mode=DRM,
                            )
                        for sp in range(NT // 2):
                            ssl = slice(2 * sp, 2 * sp + 2)
                            nc.tensor.matmul(
                                pt[:], t2hi[:, ssl, fsl], p2tlo_sb[:, ssl, dsl],
                                start=False, stop=False, perf_mode=DRM,
                            )
                        nc.tensor.matmul(
                            pt[:], biasB_sb[:, 0:2, fsl], exd_sb[:, :, dsl],
                            start=False, stop=False, perf_mode=DRM,
                        )
                        nc.tensor.matmul(
                            pt[:], biasB_sb[:, 2:4, fsl], exd_sb[:, :, dsl],
                            start=False, stop=True, perf_mode=DRM,
                        )
                        rl = vpool.tile([128, 512], F16, tag="rl")
                        nc.scalar.activation(rl[:], pt[:], ACTF.Relu, scale=0.99)
                        nc.vector.scalar_tensor_tensor(
                            h3[:, fj, dsl], pt[:], ALPHA, rl[:],
                            ALU.mult, ALU.add)
                if b == 0:
                    prefetched[(1, 0)] = grp_dma_for(1, 0)
                    prefetched[(1, 1)] = grp_dma_for(1, 1)
                else:
                    emit_wd(1, t2pool, "t2hi")
                    emit_wd(1, t2pool, "t2lo")

                if _STAGE_LIMIT < 4:
                    return
                # ---- W45: t3 = h3 @ W45 (fp16) ----
                pt45 = ps.tile([128, 512], F32, tag="ps")
                for nt in range(NT):
                    nsl = slice(nt * 128, (nt + 1) * 128)
                    for cj in range(2):
                        nc.tensor.matmul(
                            pt45[:, nt * 3 : nt * 3 + 3],
                            h3[:, cj, nsl], w45_sb[:, cj, :],
                            start=(cj == 0), stop=(cj == 1),
                        )
                nc.scalar.activation(t3hi[:], pt45[:, 0 : NT * 3], ACTF.Copy,
                                     scale=K3)
                nc.vector.scalar_tensor_tensor(
                    t3lo[:], pt45[:, 0 : NT * 3], K3, t3hi[:],
                    ALU.mult, ALU.subtract,
                )
                if b == 1:
                    emit_wd(1, h3pool, "h3")

                # ---- stage C: z5 = P2 @ t3 + bias ; h5 = leaky(z5) ----
                psC = ps.tile([128, 512], F32, tag="ps")
                for di in range(NT):
                    dsl = slice(di * 128, (di + 1) * 128)
                    osl = slice(di * 3, di * 3 + 3)
                    for sp in range(NT // 2):
                        t3sl = t3hi[:, 6 * sp : 6 * sp + 6].rearrange(
                            "p (s c) -> p s c", s=2, c=3)
                        nc.tensor.matmul(
                            psC[:, osl], p2thi_sb[:, 2 * sp : 2 * sp + 2, dsl],
                            t3sl, start=(sp == 0), stop=False, perf_mode=DRM,
                        )
                    for sp in range(NT // 2):
                        t3sl = t3lo[:, 6 * sp : 6 * sp + 6].rearrange(
                            "p (s c) -> p s c", s=2, c=3)
                        nc.tensor.matmul(
                            psC[:, osl], p2thi_sb[:, 2 * sp : 2 * sp + 2, dsl],
                            t3sl, start=False, stop=False, perf_mode=DRM,
                        )
                    for sp in range(NT // 2):
                        t3sl = t3hi[:, 6 * sp : 6 * sp + 6].rearrange(
                            "p (s c) -> p s c", s=2, c=3)
                        nc.tensor.matmul(
                            psC[:, osl], p2tlo_sb[:, 2 * sp : 2 * sp + 2, dsl],
                            t3sl, start=False, stop=False, perf_mode=DRM,
                        )
                    nc.tensor.matmul(
                        psC[:, osl], exd_sb[:, :, dsl], biasC_sb[:, 0:2, :],
                        start=False, stop=False, perf_mode=DRM,
                    )
                    nc.tensor.matmul(
                        psC[:, osl], exd_sb[:, :, dsl], biasC_sb[:, 2:4, :],
                        start=False, stop=True, perf_mode=DRM,
                    )
                nc.scalar.activation(rl5[:], psC[:, 0 : NT * 3], ACTF.Relu,
                                     scale=0.99)
                nc.vector.scalar_tensor_tensor(
                    h5_t[b][:].rearrange("p c d -> p d c"),
                    psC[:, 0 : NT * 3].rearrange("p (d c) -> p d c", d=NT, c=3),
                    ALPHA,
                    rl5[:].rearrange("p (d c) -> p d c", d=NT, c=3),
                    ALU.mult, ALU.add,
                )
                nc.gpsimd.dma_start(
                    cc_in[b][:].rearrange("c p n -> p c n"), h5_t[b][:])
                nc.gpsimd.collective_compute(
                    "AllGather",
                    ALU.bypass,
                    replica_groups=[list(range(N_CORES))],
                    ins=[cc_in[b][:]],
                    outs=[cc_out[b][:]],
                )

            def decoder_pass(half):
                pd = psd.tile([128, 512], F32, tag="psd")
                for kt in range(KDEC):
                    rhs = featT_sb[:, half : B : BL, kt : kt + 1]
                    for ct in range(6):
                        nc.tensor.matmul(
                            pd[:, ct * 8 : (ct + 1) * 8],
                            wd_tiles[kt][:, ct * 128 : (ct + 1) * 128],
                            rhs, start=(kt == 0), stop=(kt == KDEC - 1),
                        )
                for ct in range(6):
                    nc.scalar.activation(
                        y_sb[:, ct, half : B : BL], pd[:, ct * 8 : (ct + 1) * 8],
                        ACTF.Tanh, bias=bdp_sb[:, ct : ct + 1], scale=KD,
                    )

            gcn_chain(0)
            gcn_chain(1)
            if _STAGE_LIMIT < 5:
                nc.vector.memset(y_sb[:], 0)
                nc.sync.dma_start(y_out[:], y_sb[:])
                _split_multi_waits(nc)
                return nc
            decoder_pass(0)

            # featT for batch-1 collective
            for c in range(3):
                nc.sync.dma_start(
                    featT_sb[:, 1:B:BL, c * NT : (c + 1) * NT],
                    cc_out[1][:, c].rearrange("core p n -> p core n"),
                )
            decoder_pass(1)
            nc.sync.dma_start(y_out[:], y_sb[:])

    _split_multi_waits(nc)
    return nc


# ---------------------------------------------------------------------------
def _split8(a):
    hi = np.asarray(a, np.float32).astype(NPF8)
    lo = (np.asarray(a, np.float64) - hi.astype(np.float64)).astype(
        np.float32).astype(NPF8)
    return hi, lo


def _host_precompute(x, edges, Ws, bs, Wd, bd_np):
    edges = np.asarray(edges)
    src = edges[0].astype(np.int64)
    dst = edges[1].astype(np.int64)

    deg = np.bincount(dst, minlength=N).astype(np.float64) + 1.0
    isd = 1.0 / np.sqrt(deg)
    idg = 1.0 / deg

    P = np.zeros((N, N), np.float64)
    np.add.at(P, (dst, src), isd[src] * isd[dst])
    P[np.arange(N), np.arange(N)] += idg
    P2 = P @ P
    r = P.sum(axis=1)

    W0, W1, W2, W3, W4, W5 = [np.asarray(w, np.float64) for w in Ws]
    b0, b1, b2, b3, b4, b5 = [np.asarray(b, np.float64) for b in bs]
    W01 = W0 @ W1
    W23 = W2 @ W3
    W45 = W4 @ W5
    a1 = b0 @ W1
    a3 = b2 @ W3
    a5 = b4 @ W5

    # p2t[si, p, d] = P2[d, si*128+p] * SP2
    p2t_s = np.ascontiguousarray((P2.T * SP2).reshape(NT, 128, N))
    p2thi, p2tlo = _split8(p2t_s)

    w01_pad = np.zeros((C_PAD, 512), np.float64)
    w01_pad[:C_IN] = W01
    w01hi, w01lo = _split8((w01_pad * SW01).reshape(CT, 128, 512))
    w23hi, w23lo = _split8((W23 * SW23).reshape(4, 128, 256))
    w45_np = W45.reshape(2, 128, 3).astype(NPF16)

    exdh, exdl = _split8(np.stack([r, np.ones(N)]) * SEXD)
    exdp = np.ascontiguousarray(np.stack([exdh, exdl], axis=1))  # [2,2,N]

    def bias4(a, bvec, s):
        hi, lo = _split8(np.stack([a, bvec]) * s)
        return np.ascontiguousarray(
            np.stack([hi, hi, lo, lo], axis=1))  # [2,4,F]

    biasA_np = bias4(a1, b1, SBA)
    biasB_np = bias4(a3, b3, SBB)
    biasC_np = bias4(a5, b5, SBC)

    # x: pad channels, scale, split; layout [BL,NT,p=chan,CT,j=node]
    x_np = np.asarray(x, np.float32)
    x_pad = np.zeros((B, N, C_PAD), np.float32)
    x_pad[:, :, :C_IN] = x_np * SX
    xt_all = np.ascontiguousarray(
        x_pad.reshape(B, NT, 128, CT, 128).transpose(0, 1, 4, 3, 2))
    xhi_all, xlo_all = _split8(xt_all)

    # decoder: permuted rows j' = c*2048 + node
    Wd_np = np.asarray(Wd, np.float64)
    node = np.arange(N)
    rows = np.concatenate([node * 3 + c for c in range(3)])  # j' -> orig j
    Wd_perm = Wd_np[rows]  # [6144, 6144] in j' order
    bd_full = np.asarray(bd_np, np.float64)

    shared = {
        "p2thi": p2thi, "p2tlo": p2tlo,
        "w01hi": w01hi, "w01lo": w01lo,
        "w23hi": w23hi, "w23lo": w23lo,
        "w45p": w45_np,
        "exdp": exdp,
        "biasA": biasA_np, "biasB": biasB_np, "biasC": biasC_np,
    }
    in_maps = []
    for c in range(N_CORES):
        wd_c = np.ascontiguousarray(
            Wd_perm[:, c * DEC_SH : (c + 1) * DEC_SH]
            .reshape(KDEC, 128, DEC_SH).astype(NPF16))
        bd_c = np.ascontiguousarray(
            bd_full[c * DEC_SH : (c + 1) * DEC_SH]
            .reshape(6, 128).T.astype(np.float32))  # [128, 6]
        m = dict(shared)
        m["xhi"] = xhi_all[c * BL : (c + 1) * BL]
        m["xlo"] = xlo_all[c * BL : (c + 1) * BL]
        m["wdp"] = wd_c
        m["bdp"] = bd_c
        in_maps.append(m)
    return in_maps


_NC_CACHE = {}


def kernel(**inputs) -> np.ndarray:
    x = inputs["x"]
    edges = inputs["edges"]
    Ws = [inputs[f"W{i}"] for i in range(6)]
    bs = [inputs[f"b{i}"] for i in range(6)]
    Wd = inputs["Wd"]
    bd_np = inputs["bd"]

    in_maps = _host_precompute(x, edges, Ws, bs, Wd, bd_np)

    if "nc" not in _NC_CACHE:
        _NC_CACHE["nc"] = _build_program()
    nc = _NC_CACHE["nc"]

    res = run_bass_kernel_spmd(nc, in_maps, list(range(N_CORES)))

    out = np.zeros((B, D_DEC), np.float32)
    for c in range(N_CORES):
        y = res.results[c]["y"]  # [128, 6, 16]
        out[:, c * DEC_SH : (c + 1) * DEC_SH] = (
            0.1 * y.transpose(2, 1, 0).reshape(B, DEC_SH))
    return out.reshape(B, N, 3)



# revision 29
# speedup vs baseline: 1.0431x; 1.0431x over previous
"""Trainium2 Bass kernel for nn_DeformGCN (6-layer GCN + dense decoder).

Strategy (v2, fp8 DoubleRow):
  - Host precompute from `edges`: dense propagation matrix P (N x N) with
    P[dst,src] += 1/sqrt(deg_s*deg_d) and P[n,n] += 1/deg_n, then P2 = P @ P.
    GCN layer pairs fuse into 3 propagation stages (A, B, C):
      z = P2 @ (h @ (Wa@Wb)) + r (x) (ba@Wb) + 1 (x) bb,  r = P @ 1
    followed by LeakyReLU(0.01).
  - All heavy matmuls run as fp8e4m3 DoubleRow (2 x 128-deep products per
    instruction at 0.5 cycles/row = 4x bf16 MAC rate). Accuracy is restored
    with a hi/lo residual split of both operands; the lo*lo term is dropped
    (3-term scheme, 0.75x the bf16 row count). End-to-end rel err ~2e-3.
  - Activations are split on the fly during PSUM eviction:
      hi = ACT copy(psum, scale=k) -> fp8 ; lo = DVE (psum*k - hi) -> fp8
    LeakyReLU uses  v = 0.99k*relu(z) (ACT)  +  0.01k*z (DVE stt add).
  - Small stages (W45, decoder) run in fp16 (1.0 cycles/row, 10-bit mantissa).
  - Decoder is column-sharded (768 cols/core) and oriented [col_part x batch]
    so its cost is 48*6*16 rows. Features are AllGather'd per local batch (2
    collectives; the first fully overlaps with batch 1's GCN compute).
  - Biases ride the propagation matmuls as extra K=2 DoubleRow slots.
"""

import os
import numpy as np
import ml_dtypes

_STAGE_LIMIT = int(os.environ.get("KSTAGE", "99"))

import bass_rust
import concourse.bass as bass
import concourse.mybir as mybir
import concourse.tile as tile
from concourse.tile import ScopedClock
from concourse.bass_utils import run_bass_kernel_spmd

F8 = mybir.dt.float8e4
F16 = mybir.dt.float16
F32 = mybir.dt.float32
NPF8 = ml_dtypes.float8_e4m3
NPF16 = np.float16
DRM = mybir.MatmulPerfMode.DoubleRow
ALU = mybir.AluOpType
ACTF = mybir.ActivationFunctionType

N_CORES = 8
B = 16
N = 2048
C_IN = 1475
C_PAD = 1536           # 12 * 128
NT = N // 128          # 16 node tiles
CT = C_PAD // 128      # 12 channel tiles
BL = B // N_CORES      # 2 local batches
D_DEC = N * 3          # 6144
DEC_SH = D_DEC // N_CORES  # 768 decoder columns per core
KDEC = D_DEC // 128    # 48 decoder k tiles
ALPHA = 0.01

# scales (powers of two; products of the two operand scales give the PSUM
# scale, the ACT evict rescales to the next storage scale)
SX = 16.0
SW01 = 2048.0
SCT = 32.0
SP2 = 64.0
SCH1 = 128.0
SW23 = 512.0
SCT2 = 512.0
SCT3 = 2048.0
SEXD = 64.0
SBA = SCT * SP2 / SEXD     # 32
SBB = SCT2 * SP2 / SEXD    # 512
SBC = SCT3 * SP2 / SEXD    # 2048
KT = SCT / (SX * SW01)     # 2^-10
K1 = SCH1 / (SCT * SP2)    # 2^-4
K2 = SCT2 / (SCH1 * SW23)  # 2^-7
K3 = SCT3 / (SCT2 * SP2)   # 2^-4
KD = 1.0 / (SCT3 * SP2)    # 2^-17


# ---------------------------------------------------------------------------
# Workaround: this walrus build caps sync-waits per control instruction very
# low, so TileContext's tail drain (which waits on every proc's semaphore)
# fails codegen. Split the global-clock waits into one single-wait
# EventSemaphore each, then emit a bare Drain.
def _patched_drain_and_barrier(self, tick_clock, wait_clock):
    nc = self.nc
    num_to_handle = {h.num: h for h in self.sems.allocated().values()}
    probe = nc.sync.nop(nofuse=True)
    wait_clock.add_sem_waits(probe.ins, ScopedClock({None: tick_clock.global_clock}))
    waits = list(probe.ins.sync_info.on_wait)
    probe.ins.sync_info = bass_rust.SyncInfo(on_wait=[], on_update=[])
    engines = [nc.sync, nc.scalar, nc.vector, nc.tensor, nc.gpsimd]
    for i, w in enumerate(waits):
        h = num_to_handle.get(w.id)
        if h is None:
            raise RuntimeError(f"no sem handle for {w.id} ({w.ant_name})")
        engines[i % len(engines)].wait_ge(h, w.wait_value)
    nc.all_engine_barrier()
    nc.sync.drain()
    assert self.sems is not None
    popped = nc._tile_sem_poison_stack.pop()
    assert popped is self._sem_poison
    nc.clear_and_free_semaphores(list(self.sems.allocated().values()))
    nc.all_engine_barrier()


tile.TileContext._drain_and_barrier = _patched_drain_and_barrier


def _split_multi_waits(nc, max_waits=1):
    """This walrus build rejects instructions carrying more than one sync
    wait. Hoist extra waits into standalone EventSemaphore instructions
    placed immediately before the instruction on the same engine queue."""
    ctr = 0
    for fn in nc.m.functions:
        for bb in fn.blocks:
            insts = bb.instructions
            new = []
            changed = False
            for inst in insts:
                si = inst.sync_info
                waits = list(si.on_wait) if si is not None else []
                if len(waits) > max_waits:
                    changed = True
                    for w in waits[:-max_waits]:
                        ev = bass_rust.InstEventSemaphore(
                            name=f"splitw_{ctr}", ins=[], outs=[]
                        )
                        ctr += 1
                        ev.engine = inst.engine
                        ev.sync_info = bass_rust.SyncInfo(
                            on_wait=[w], on_update=[]
                        )
                        new.append(ev)
                    inst.sync_info = bass_rust.SyncInfo(
                        on_wait=waits[-max_waits:], on_update=list(si.on_update)
                    )
                new.append(inst)
            if changed:
                bb.instructions = new


# ---------------------------------------------------------------------------
# v3 schedule: L0(b0), L0(b1) run back-to-back up front (the front window is
# DMA-bandwidth-bound: w01 + x + p2t = 22.6 MB must land before stage A can
# start; batch 1's L0 gives the PE ~31 us of work that was otherwise idle
# wait). Then chain(b0) stages A..C + collective 0 overlap chain(b1); the
# decoder passes slot in where their inputs are ready. Decoder weights stream
# into 20 dedicated slots plus group-tiles borrowed from x/t tiles as they
# die. Bias rides a single hi-precision DR matmul (lo bias term dropped).
def _build_program() -> bass.Bass:
    nc = bass.Bass()

    xhi = nc.declare_dram_parameter("xhi", [BL, NT, 128, CT, 128], F8, isOutput=False)
    xlo = nc.declare_dram_parameter("xlo", [BL, NT, 128, CT, 128], F8, isOutput=False)
    p2thi = nc.declare_dram_parameter("p2thi", [NT, 128, N], F8, isOutput=False)
    p2tlo = nc.declare_dram_parameter("p2tlo", [NT, 128, N], F8, isOutput=False)
    w01hi = nc.declare_dram_parameter("w01hi", [CT, 128, 512], F8, isOutput=False)
    w01lo = nc.declare_dram_parameter("w01lo", [CT, 128, 512], F8, isOutput=False)
    w23hi = nc.declare_dram_parameter("w23hi", [4, 128, 256], F8, isOutput=False)
    w23lo = nc.declare_dram_parameter("w23lo", [4, 128, 256], F8, isOutput=False)
    w45p = nc.declare_dram_parameter("w45p", [2, 128, 3], F16, isOutput=False)
    exdp = nc.declare_dram_parameter("exdp", [2, 2, N], F8, isOutput=False)
    biasA = nc.declare_dram_parameter("biasA", [2, 4, 512], F8, isOutput=False)
    biasB = nc.declare_dram_parameter("biasB", [2, 4, 256], F8, isOutput=False)
    biasC = nc.declare_dram_parameter("biasC", [2, 4, 3], F8, isOutput=False)
    wdp = nc.declare_dram_parameter("wdp", [KDEC, 128, DEC_SH], F16, isOutput=False)
    bdp = nc.declare_dram_parameter("bdp", [128, 6], F32, isOutput=False)
    y_out = nc.declare_dram_parameter("y", [128, 6, B], F32, isOutput=True)

    cc_in = [nc.dram_tensor(f"cc_in{b}", [3, 128, NT], F16) for b in range(BL)]
    cc_out = [
        nc.dram_tensor(f"cc_out{b}", [N_CORES, 3, 128, NT], F16, addr_space="Shared")
        for b in range(BL)
    ]

    with tile.TileContext(nc) as tc:
        with (
            tc.tile_pool(name="const", bufs=1) as constp,
            tc.tile_pool(name="xp", bufs=2) as xpool,
            tc.tile_pool(name="tp", bufs=1) as tpool,
            tc.tile_pool(name="h1p", bufs=1) as h1pool,
            tc.tile_pool(name="t2p", bufs=1) as t2pool,
            tc.tile_pool(name="h3p", bufs=1) as h3pool,
            tc.tile_pool(name="vp", bufs=2) as vpool,
            tc.tile_pool(name="wdpool", bufs=1) as wdpool,
            tc.tile_pool(name="ps", bufs=5, space="PSUM") as ps,
            tc.tile_pool(name="psd", bufs=2, space="PSUM") as psd,
        ):
            # ---- const tiles ----
            w01hi_sb = constp.tile([128, CT, 512], F8, tag="w01hi")
            w01lo_sb = constp.tile([128, CT, 512], F8, tag="w01lo")
            p2thi_sb = constp.tile([128, NT, N], F8, tag="p2thi")
            p2tlo_sb = constp.tile([128, NT, N], F8, tag="p2tlo")
            w23hi_sb = constp.tile([128, 4, 256], F8, tag="w23hi")
            w23lo_sb = constp.tile([128, 4, 256], F8, tag="w23lo")
            w45_sb = constp.tile([128, 2, 3], F16, tag="w45")
            exd_sb = constp.tile([2, 2, N], F8, tag="exd")
            biasA_sb = constp.tile([2, 4, 512], F8, tag="biasA")
            biasB_sb = constp.tile([2, 4, 256], F8, tag="biasB")
            biasC_sb = constp.tile([2, 4, 3], F8, tag="biasC")
            bdp_sb = constp.tile([128, 6], F32, tag="bdp")
            featT_sb = constp.tile([128, B, KDEC], F16, tag="featT")
            y_sb = constp.tile([128, 6, B], F32, tag="ysb")
            t3hi = constp.tile([128, NT * 3], F8, tag="t3hi")
            t3lo = constp.tile([128, NT * 3], F8, tag="t3lo")
            rl5 = constp.tile([128, NT * 3], F16, tag="rl5")
            h5_0 = constp.tile([128, 3, NT], F16, tag="h5_0")
            h5_1 = constp.tile([128, 3, NT], F16, tag="h5_1")
            h5_t = [h5_0, h5_1]

            wd_tiles: list = [None] * KDEC

            prefetched = {}

            def post_x(b, g0, ng, tag_hi, tag_lo):
                """DMA x node-tile groups [2*g0, 2*(g0+ng)) as ONE transfer
                per hi/lo half (DGE fixed costs dominate small DMAs).
                Singles and pairs use distinct tags so slot reuse (WAR) never
                blocks the stream head-of-line."""
                ghi = xpool.tile([128, 2 * ng, CT, 128], F8, tag=tag_hi)
                glo = xpool.tile([128, 2 * ng, CT, 128], F8, tag=tag_lo)
                nc.sync.dma_start(
                    ghi[:], xhi[b, 2 * g0 : 2 * g0 + 2 * ng].rearrange(
                        "n p c j -> p n c j"))
                nc.gpsimd.dma_start(
                    glo[:], xlo[b, 2 * g0 : 2 * g0 + 2 * ng].rearrange(
                        "n p c j -> p n c j"))
                for i in range(ng):
                    prefetched[(b, g0 + i)] = ((ghi, glo), i)

            # Startup: group 0 of L0 needs ALL of w01hi within ~0.6us of its
            # first matmul, so the full block goes first; then the first two
            # x groups as singles.
            nc.sync.dma_start(
                w01hi_sb[:], w01hi[:].rearrange("c p f -> p c f"))
            post_x(0, 0, 1, "xs_hi", "xs_lo")
            nc.gpsimd.dma_start(
                w01lo_sb[:], w01lo[:].rearrange("c p f -> p c f"))
            post_x(0, 1, 1, "xs_hi", "xs_lo")

            def emit_small_consts():
                nc.sync.dma_start(w23hi_sb[:], w23hi[:].rearrange("c p f -> p c f"))
                nc.sync.dma_start(w23lo_sb[:], w23lo[:].rearrange("c p f -> p c f"))
                nc.sync.dma_start(w45_sb[:], w45p[:].rearrange("c p f -> p c f"))
                nc.sync.dma_start(exd_sb[:], exdp[:])
                nc.sync.dma_start(biasA_sb[:], biasA[:])
                nc.sync.dma_start(biasB_sb[:], biasB[:])
                nc.sync.dma_start(biasC_sb[:], biasC[:])
                nc.sync.dma_start(bdp_sb[:], bdp[:])

            # p2t quad-row DMAs: hi on the HWDGE (sync) path, lo via the
            # otherwise-idle Pool engine's SWDGE; 4 src tiles per transfer to
            # amortize the ~1us per-DMA DGE cost.
            p2_rows = [(4 * i, 0) for i in range(4)] + [(4 * i, 1) for i in range(4)]
            p2_pos = [0]

            def emit_p2(n):
                for _ in range(n):
                    if p2_pos[0] >= len(p2_rows):
                        return
                    si, hl = p2_rows[p2_pos[0]]
                    p2_pos[0] += 1
                    if hl == 0:
                        nc.sync.dma_start(
                            p2thi_sb[:, si : si + 4, :],
                            p2thi[si : si + 4].rearrange("s p n -> p s n"))
                    else:
                        nc.gpsimd.dma_start(
                            p2tlo_sb[:, si : si + 4, :],
                            p2tlo[si : si + 4].rearrange("s p n -> p s n"))

            wd_pos = [0]
            wd_direct = [0]
            N_WD_DIRECT_GROUPS = 1

            def borrow_wd(pool, tag, n):
                """Land n decoder k-tiles in one group DMA into a dead tile's
                slot (tag must never be allocated again afterwards)."""
                k0 = wd_pos[0]
                assert k0 + n <= KDEC
                wd_pos[0] += n
                gt = pool.tile([128, n, DEC_SH], F16, tag=tag)
                nc.gpsimd.dma_start(
                    gt[:], wdp[k0 : k0 + n].rearrange("k p f -> p k f"))
                for i in range(n):
                    wd_tiles[k0 + i] = gt[:, i, :]

            def emit_wd(n):
                """Stream decoder-weight k-tiles into the dedicated pool,
                4 per transfer."""
                for _ in range(n):
                    if wd_direct[0] >= N_WD_DIRECT_GROUPS or wd_pos[0] + 4 > KDEC:
                        return
                    wd_direct[0] += 1
                    borrow_wd(wdpool, "wd", 4)

            def l0_stage(b, t_hi, t_lo):
                # ---- L0: t = x @ W01 (3-term fp8 DR) ----
                # x streams in node-tile groups: hi via HWDGE, lo via SWDGE.
                # Groups 0/1 land as singles (startup latency), the rest as
                # pair transfers; batch 1's stream is fully posted during
                # batch 0's loop.
                for g in range(NT // 2):
                    (ghi, glo), gi = prefetched.pop((b, g))
                    for k in range(2):
                        nt = 2 * g + k
                        pt = ps.tile([128, 512], F32, tag="ps")
                        first = True
                        for p in range(CT // 2):
                            nc.tensor.matmul(
                                pt[:], ghi[:, 2 * gi + k, 2 * p : 2 * p + 2, :],
                                w01hi_sb[:, 2 * p : 2 * p + 2, :],
                                start=first, stop=False, perf_mode=DRM,
                            )
                            first = False
                        for p in range(CT // 2):
                            nc.tensor.matmul(
                                pt[:], glo[:, 2 * gi + k, 2 * p : 2 * p + 2, :],
                                w01hi_sb[:, 2 * p : 2 * p + 2, :],
                                start=False, stop=False, perf_mode=DRM,
                            )
                        for p in range(CT // 2):
                            nc.tensor.matmul(
                                pt[:], ghi[:, 2 * gi + k, 2 * p : 2 * p + 2, :],
                                w01lo_sb[:, 2 * p : 2 * p + 2, :],
                                start=False, stop=(p == CT // 2 - 1),
                                perf_mode=DRM,
                            )
                        nc.scalar.activation(t_hi[:, nt, :], pt[:], ACTF.Copy,
                                             scale=KT)
                        nc.vector.scalar_tensor_tensor(
                            t_lo[:, nt, :], pt[:], KT, t_hi[:, nt, :],
                            ALU.mult, ALU.subtract,
                        )
                    if b == 0:
                        if g == 0:
                            post_x(0, 2, 2, "xhi", "xlo")
                        elif g == 1:
                            post_x(0, 4, 2, "xhi", "xlo")
                        elif g == 2:
                            post_x(0, 6, 2, "xhi", "xlo")
                        elif g == 3:
                            emit_small_consts()
                        elif g == 5:
                            post_x(1, 0, 1, "xs_hi", "xs_lo")
                            post_x(1, 2, 2, "xhi", "xlo")
                        elif g == 6:
                            post_x(1, 1, 1, "xs_hi", "xs_lo")
                            post_x(1, 4, 2, "xhi", "xlo")
                        elif g == 7:
                            post_x(1, 6, 2, "xhi", "xlo")

            def prop_stage(b, src_hi, src_lo, bias_sb, nf, out_cb, wd_budget):
                # ---- z = P2 @ src + bias (single hi-bias DR matmul) ----
                # nf: number of 128-wide feature tiles in src (4 for stage A,
                # 2 for stage B). out_cb(dc, fj, pt) evicts the PSUM tile.
                for dc in range(4):
                    dsl = slice(dc * 512, (dc + 1) * 512)
                    for fj in range(nf):
                        fsl = slice(fj * 128, (fj + 1) * 128)
                        pt = ps.tile([128, 512], F32, tag="ps")
                        for sp in range(NT // 2):
                            ssl = slice(2 * sp, 2 * sp + 2)
                            nc.tensor.matmul(
                                pt[:], src_hi[:, ssl, fsl],
                                p2thi_sb[:, ssl, dsl],
                                start=(sp == 0), stop=False, perf_mode=DRM,
                            )
                        for sp in range(NT // 2):
                            ssl = slice(2 * sp, 2 * sp + 2)
                            nc.tensor.matmul(
                                pt[:], src_lo[:, ssl, fsl],
                                p2thi_sb[:, ssl, dsl],
                                start=False, stop=False, perf_mode=DRM,
                            )
                        for sp in range(NT // 2):
                            ssl = slice(2 * sp, 2 * sp + 2)
                            nc.tensor.matmul(
                                pt[:], src_hi[:, ssl, fsl],
                                p2tlo_sb[:, ssl, dsl],
                                start=False, stop=False, perf_mode=DRM,
                            )
                        nc.tensor.matmul(
                            pt[:], bias_sb[:, 0:2, fsl], exd_sb[:, :, dsl],
                            start=False, stop=False, perf_mode=DRM,
                        )
                        nc.tensor.matmul(
                            pt[:], bias_sb[:, 2:4, fsl], exd_sb[:, :, dsl],
                            start=False, stop=True, perf_mode=DRM,
                        )
                        out_cb(dc, fj, pt)
                        emit_wd(wd_budget)

            def stageA(b, t_hi, t_lo, h1hi, h1lo):
                def evict(dc, fj, pt):
                    dsl = slice(dc * 512, (dc + 1) * 512)
                    rl = vpool.tile([128, 512], F16, tag="rl")
                    nc.scalar.activation(rl[:], pt[:], ACTF.Relu,
                                         scale=0.99 * K1)
                    v = vpool.tile([128, 512], F16, tag="v")
                    nc.vector.scalar_tensor_tensor(
                        v[:], pt[:], ALPHA * K1, rl[:], ALU.mult, ALU.add)
                    nc.scalar.activation(h1hi[:, fj, dsl], v[:], ACTF.Copy)
                    nc.vector.tensor_tensor(
                        h1lo[:, fj, dsl], v[:], h1hi[:, fj, dsl],
                        ALU.subtract)

                prop_stage(b, t_hi, t_lo, biasA_sb, 4, evict,
                           1 if b == 0 else 0)

            def w23_stage(h1hi, h1lo, t2hi, t2lo):
                # ---- W23: t2 = h1 @ W23 ----
                for nt in range(NT):
                    nsl = slice(nt * 128, (nt + 1) * 128)
                    pt = ps.tile([128, 512], F32, tag="ps")
                    for fp in range(2):
                        fsl = slice(2 * fp, 2 * fp + 2)
                        nc.tensor.matmul(
                            pt[:, 0:256], h1hi[:, fsl, nsl], w23hi_sb[:, fsl, :],
                            start=(fp == 0), stop=False, perf_mode=DRM,
                        )
                    for fp in range(2):
                        fsl = slice(2 * fp, 2 * fp + 2)
                        nc.tensor.matmul(
                            pt[:, 0:256], h1lo[:, fsl, nsl], w23hi_sb[:, fsl, :],
                            start=False, stop=False, perf_mode=DRM,
                        )
                    for fp in range(2):
                        fsl = slice(2 * fp, 2 * fp + 2)
                        nc.tensor.matmul(
                            pt[:, 0:256], h1hi[:, fsl, nsl], w23lo_sb[:, fsl, :],
                            start=False, stop=(fp == 1), perf_mode=DRM,
                        )
                    nc.scalar.activation(t2hi[:, nt, :], pt[:, 0:256], ACTF.Copy,
                                         scale=K2)
                    nc.vector.scalar_tensor_tensor(
                        t2lo[:, nt, :], pt[:, 0:256], K2, t2hi[:, nt, :],
                        ALU.mult, ALU.subtract,
                    )

            def stageB(b, t2hi, t2lo, h3):
                def evict(dc, fj, pt):
                    dsl = slice(dc * 512, (dc + 1) * 512)
                    rl = vpool.tile([128, 512], F16, tag="rl")
                    nc.scalar.activation(rl[:], pt[:], ACTF.Relu, scale=0.99)
                    nc.vector.scalar_tensor_tensor(
                        h3[:, fj, dsl], pt[:], ALPHA, rl[:],
                        ALU.mult, ALU.add)

                prop_stage(b, t2hi, t2lo, biasB_sb, 2, evict, 0)

            def tail_stages(b, h3):
                # ---- W45: t3 = h3 @ W45 (fp16) ----
                pt45 = ps.tile([128, 512], F32, tag="ps")
                for nt in range(NT):
                    nsl = slice(nt * 128, (nt + 1) * 128)
                    for cj in range(2):
                        nc.tensor.matmul(
                            pt45[:, nt * 3 : nt * 3 + 3],
                            h3[:, cj, nsl], w45_sb[:, cj, :],
                            start=(cj == 0), stop=(cj == 1),
                        )
                nc.scalar.activation(t3hi[:], pt45[:, 0 : NT * 3], ACTF.Copy,
                                     scale=K3)
                nc.vector.scalar_tensor_tensor(
                    t3lo[:], pt45[:, 0 : NT * 3], K3, t3hi[:],
                    ALU.mult, ALU.subtract,
                )

                # ---- stage C: z5 = P2 @ t3 + bias ; h5 = leaky(z5) ----
                psC = ps.tile([128, 512], F32, tag="ps")
                for di in range(NT):
                    dsl = slice(di * 128, (di + 1) * 128)
                    osl = slice(di * 3, di * 3 + 3)
                    for sp in range(NT // 2):
                        t3sl = t3hi[:, 6 * sp : 6 * sp + 6].rearrange(
                            "p (s c) -> p s c", s=2, c=3)
                        nc.tensor.matmul(
                            psC[:, osl], p2thi_sb[:, 2 * sp : 2 * sp + 2, dsl],
                            t3sl, start=(sp == 0), stop=False, perf_mode=DRM,
                        )
                    for sp in range(NT // 2):
                        t3sl = t3lo[:, 6 * sp : 6 * sp + 6].rearrange(
                            "p (s c) -> p s c", s=2, c=3)
                        nc.tensor.matmul(
                            psC[:, osl], p2thi_sb[:, 2 * sp : 2 * sp + 2, dsl],
                            t3sl, start=False, stop=False, perf_mode=DRM,
                        )
                    for sp in range(NT // 2):
                        t3sl = t3hi[:, 6 * sp : 6 * sp + 6].rearrange(
                            "p (s c) -> p s c", s=2, c=3)
                        nc.tensor.matmul(
                            psC[:, osl], p2tlo_sb[:, 2 * sp : 2 * sp + 2, dsl],
                            t3sl, start=False, stop=False, perf_mode=DRM,
                        )
                    nc.tensor.matmul(
                        psC[:, osl], exd_sb[:, :, dsl], biasC_sb[:, 0:2, :],
                        start=False, stop=False, perf_mode=DRM,
                    )
                    nc.tensor.matmul(
                        psC[:, osl], exd_sb[:, :, dsl], biasC_sb[:, 2:4, :],
                        start=False, stop=True, perf_mode=DRM,
                    )
                nc.scalar.activation(rl5[:], psC[:, 0 : NT * 3], ACTF.Relu,
                                     scale=0.99)
                nc.vector.scalar_tensor_tensor(
                    h5_t[b][:].rearrange("p c d -> p d c"),
                    psC[:, 0 : NT * 3].rearrange("p (d c) -> p d c", d=NT, c=3),
                    ALPHA,
                    rl5[:].rearrange("p (d c) -> p d c", d=NT, c=3),
                    ALU.mult, ALU.add,
                )
                nc.sync.dma_start(
                    cc_in[b][:].rearrange("c p n -> p c n"), h5_t[b][:])
                nc.gpsimd.collective_compute(
                    "AllGather",
                    ALU.bypass,
                    replica_groups=[list(range(N_CORES))],
                    ins=[cc_in[b][:]],
                    outs=[cc_out[b][:]],
                )

            def featT_dma(half):
                for c in range(3):
                    nc.sync.dma_start(
                        featT_sb[:, half : B : BL, c * NT : (c + 1) * NT],
                        cc_out[half][:, c].rearrange("core p n -> p core n"),
                    )

            def decoder_pass(half):
                pd = psd.tile([128, 512], F32, tag="psd")
                for kt in range(KDEC):
                    rhs = featT_sb[:, half : B : BL, kt : kt + 1]
                    for ct in range(6):
                        nc.tensor.matmul(
                            pd[:, ct * 8 : (ct + 1) * 8],
                            wd_tiles[kt][:, ct * 128 : (ct + 1) * 128],
                            rhs, start=(kt == 0), stop=(kt == KDEC - 1),
                        )
                for ct in range(6):
                    nc.scalar.activation(
                        y_sb[:, ct, half : B : BL], pd[:, ct * 8 : (ct + 1) * 8],
                        ACTF.Tanh, bias=bdp_sb[:, ct : ct + 1], scale=KD,
                    )

            # ---- emission schedule ----
            t_hi0 = tpool.tile([128, NT, 512], F8, tag="t_hi0")
            t_lo0 = tpool.tile([128, NT, 512], F8, tag="t_lo0")
            t_hi1 = tpool.tile([128, NT, 512], F8, tag="t_hi1")
            t_lo1 = tpool.tile([128, NT, 512], F8, tag="t_lo1")

            l0_stage(0, t_hi0, t_lo0)
            # All x for both batches is now posted on the sync/pool queues;
            # p2t goes behind it in the same queues, so the shared DMA bus
            # serves x strictly first (queues drain in program order and run
            # far ahead of the PE — emission points alone don't pace DMA).
            emit_p2(8)
            l0_stage(1, t_hi1, t_lo1)
            # x tiles are dead from here: 24 decoder k-tiles into their slots
            for tag in ("xs_hi", "xs_lo"):
                borrow_wd(xpool, tag, 2)
                borrow_wd(xpool, tag, 2)
            for tag in ("xhi", "xlo"):
                borrow_wd(xpool, tag, 4)
                borrow_wd(xpool, tag, 4)

            h1hi = h1pool.tile([128, 4, N], F8, tag="h1hi")
            h1lo = h1pool.tile([128, 4, N], F8, tag="h1lo")
            stageA(0, t_hi0, t_lo0, h1hi, h1lo)
            emit_wd(N_WD_DIRECT_GROUPS)  # any remainder
            borrow_wd(tpool, "t_hi0", 5)
            borrow_wd(tpool, "t_lo0", 5)

            t2hi = t2pool.tile([128, NT, 256], F8, tag="t2hi")
            t2lo = t2pool.tile([128, NT, 256], F8, tag="t2lo")
            w23_stage(h1hi, h1lo, t2hi, t2lo)
            h3 = h3pool.tile([128, 2, N], F16, tag="h3")
            stageB(0, t2hi, t2lo, h3)
            tail_stages(0, h3)
            featT_dma(0)

            h1hi = h1pool.tile([128, 4, N], F8, tag="h1hi")
            h1lo = h1pool.tile([128, 4, N], F8, tag="h1lo")
            stageA(1, t_hi1, t_lo1, h1hi, h1lo)
            borrow_wd(tpool, "t_hi1", 5)
            borrow_wd(tpool, "t_lo1", 5)
            assert wd_pos[0] == KDEC, wd_pos[0]

            t2hi = t2pool.tile([128, NT, 256], F8, tag="t2hi")
            t2lo = t2pool.tile([128, NT, 256], F8, tag="t2lo")
            w23_stage(h1hi, h1lo, t2hi, t2lo)
            h3 = h3pool.tile([128, 2, N], F16, tag="h3")
            stageB(1, t2hi, t2lo, h3)
            tail_stages(1, h3)
            decoder_pass(0)  # fills part of the cc1 wait
            featT_dma(1)
            decoder_pass(1)
            nc.sync.dma_start(y_out[:], y_sb[:])

    _split_multi_waits(nc)
    return nc


# ---------------------------------------------------------------------------
def _split8(a):
    hi = np.asarray(a, np.float32).astype(NPF8)
    lo = (np.asarray(a, np.float64) - hi.astype(np.float64)).astype(
        np.float32).astype(NPF8)
    return hi, lo


def _host_precompute(x, edges, Ws, bs, Wd, bd_np):
    edges = np.asarray(edges)
    src = edges[0].astype(np.int64)
    dst = edges[1].astype(np.int64)

    deg = np.bincount(dst, minlength=N).astype(np.float64) + 1.0
    isd = 1.0 / np.sqrt(deg)
    idg = 1.0 / deg

    P = np.zeros((N, N), np.float64)
    np.add.at(P, (dst, src), isd[src] * isd[dst])
    P[np.arange(N), np.arange(N)] += idg
    P2 = P @ P
    r = P.sum(axis=1)

    W0, W1, W2, W3, W4, W5 = [np.asarray(w, np.float64) for w in Ws]
    b0, b1, b2, b3, b4, b5 = [np.asarray(b, np.float64) for b in bs]
    W01 = W0 @ W1
    W23 = W2 @ W3
    W45 = W4 @ W5
    a1 = b0 @ W1
    a3 = b2 @ W3
    a5 = b4 @ W5

    # p2t[si, p, d] = P2[d, si*128+p] * SP2
    p2t_s = np.ascontiguousarray((P2.T * SP2).reshape(NT, 128, N))
    p2thi, p2tlo = _split8(p2t_s)

    w01_pad = np.zeros((C_PAD, 512), np.float64)
    w01_pad[:C_IN] = W01
    w01hi, w01lo = _split8((w01_pad * SW01).reshape(CT, 128, 512))
    w23hi, w23lo = _split8((W23 * SW23).reshape(4, 128, 256))
    w45_np = W45.reshape(2, 128, 3).astype(NPF16)

    exdh, exdl = _split8(np.stack([r, np.ones(N)]) * SEXD)
    exdp = np.ascontiguousarray(np.stack([exdh, exdl], axis=1))  # [2,2,N]

    def bias4(a, bvec, s):
        hi, lo = _split8(np.stack([a, bvec]) * s)
        return np.ascontiguousarray(
            np.stack([hi, hi, lo, lo], axis=1))  # [2,4,F]

    biasA_np = bias4(a1, b1, SBA)
    biasB_np = bias4(a3, b3, SBB)
    biasC_np = bias4(a5, b5, SBC)

    # x: pad channels, scale, split; layout [BL,NT,p=chan,CT,j=node]
    x_np = np.asarray(x, np.float32)
    x_pad = np.zeros((B, N, C_PAD), np.float32)
    x_pad[:, :, :C_IN] = x_np * SX
    xt_all = np.ascontiguousarray(
        x_pad.reshape(B, NT, 128, CT, 128).transpose(0, 1, 4, 3, 2))
    xhi_all, xlo_all = _split8(xt_all)

    # decoder: permuted rows j' = c*2048 + node
    Wd_np = np.asarray(Wd, np.float64)
    node = np.arange(N)
    rows = np.concatenate([node * 3 + c for c in range(3)])  # j' -> orig j
    Wd_perm = Wd_np[rows]  # [6144, 6144] in j' order
    bd_full = np.asarray(bd_np, np.float64)

    shared = {
        "p2thi": p2thi, "p2tlo": p2tlo,
        "w01hi": w01hi, "w01lo": w01lo,
        "w23hi": w23hi, "w23lo": w23lo,
        "w45p": w45_np,
        "exdp": exdp,
        "biasA": biasA_np, "biasB": biasB_np, "biasC": biasC_np,
    }
    in_maps = []
    for c in range(N_CORES):
        wd_c = np.ascontiguousarray(
            Wd_perm[:, c * DEC_SH : (c + 1) * DEC_SH]
            .reshape(KDEC, 128, DEC_SH).astype(NPF16))
        bd_c = np.ascontiguousarray(
            bd_full[c * DEC_SH : (c + 1) * DEC_SH]
            .reshape(6, 128).T.astype(np.float32))  # [128, 6]
        m = dict(shared)
        m["xhi"] = xhi_all[c * BL : (c + 1) * BL]
        m["xlo"] = xlo_all[c * BL : (c + 1) * BL]
        m["wdp"] = wd_c
        m["bdp"] = bd_c
        in_maps.append(m)
    return in_maps


_NC_CACHE = {}


def kernel(**inputs) -> np.ndarray:
    x = inputs["x"]
    edges = inputs["edges"]
    Ws = [inputs[f"W{i}"] for i in range(6)]
    bs = [inputs[f"b{i}"] for i in range(6)]
    Wd = inputs["Wd"]
    bd_np = inputs["bd"]

    in_maps = _host_precompute(x, edges, Ws, bs, Wd, bd_np)

    if "nc" not in _NC_CACHE:
        _NC_CACHE["nc"] = _build_program()
    nc = _NC_CACHE["nc"]

    res = run_bass_kernel_spmd(nc, in_maps, list(range(N_CORES)))

    out = np.zeros((B, D_DEC), np.float32)
    for c in range(N_CORES):
        y = res.results[c]["y"]  # [128, 6, 16]
        out[:, c * DEC_SH : (c + 1) * DEC_SH] = (
            0.1 * y.transpose(2, 1, 0).reshape(B, DEC_SH))
    return out.reshape(B, N, 3)



# revision 33
# speedup vs baseline: 1.0725x; 1.0282x over previous
"""Trainium2 Bass kernel for nn_DeformGCN (6-layer GCN + dense decoder).

Strategy (v2, fp8 DoubleRow):
  - Host precompute from `edges`: dense propagation matrix P (N x N) with
    P[dst,src] += 1/sqrt(deg_s*deg_d) and P[n,n] += 1/deg_n, then P2 = P @ P.
    GCN layer pairs fuse into 3 propagation stages (A, B, C):
      z = P2 @ (h @ (Wa@Wb)) + r (x) (ba@Wb) + 1 (x) bb,  r = P @ 1
    followed by LeakyReLU(0.01).
  - All heavy matmuls run as fp8e4m3 DoubleRow (2 x 128-deep products per
    instruction at 0.5 cycles/row = 4x bf16 MAC rate). Accuracy is restored
    with a hi/lo residual split of both operands; the lo*lo term is dropped
    (3-term scheme, 0.75x the bf16 row count). End-to-end rel err ~2e-3.
  - Activations are split on the fly during PSUM eviction:
      hi = ACT copy(psum, scale=k) -> fp8 ; lo = DVE (psum*k - hi) -> fp8
    LeakyReLU uses  v = 0.99k*relu(z) (ACT)  +  0.01k*z (DVE stt add).
  - Small stages (W45, decoder) run in fp16 (1.0 cycles/row, 10-bit mantissa).
  - Decoder is column-sharded (768 cols/core) and oriented [col_part x batch]
    so its cost is 48*6*16 rows. Features are AllGather'd per local batch (2
    collectives; the first fully overlaps with batch 1's GCN compute).
  - Biases ride the propagation matmuls as extra K=2 DoubleRow slots.
"""

import os
import numpy as np
import ml_dtypes

_STAGE_LIMIT = int(os.environ.get("KSTAGE", "99"))

import bass_rust
import concourse.bass as bass
import concourse.mybir as mybir
import concourse.tile as tile
from concourse.tile import ScopedClock
from concourse.bass_utils import run_bass_kernel_spmd

F8 = mybir.dt.float8e4
F16 = mybir.dt.float16
F32 = mybir.dt.float32
NPF8 = ml_dtypes.float8_e4m3
NPF16 = np.float16
DRM = mybir.MatmulPerfMode.DoubleRow
ALU = mybir.AluOpType
ACTF = mybir.ActivationFunctionType

N_CORES = 8
B = 16
N = 2048
C_IN = 1475
C_PAD = 1536           # 12 * 128
NT = N // 128          # 16 node tiles
CT = C_PAD // 128      # 12 channel tiles
BL = B // N_CORES      # 2 local batches
D_DEC = N * 3          # 6144
DEC_SH = D_DEC // N_CORES  # 768 decoder columns per core
KDEC = D_DEC // 128    # 48 decoder k tiles
ALPHA = 0.01

# scales (powers of two; products of the two operand scales give the PSUM
# scale, the ACT evict rescales to the next storage scale)
SX = 16.0
SW01 = 2048.0
SCT = 32.0
SP2 = 64.0
SCH1 = 128.0
SW23 = 512.0
SCT2 = 512.0
SCT3 = 2048.0
SEXD = 64.0
SBA = SCT * SP2 / SEXD     # 32
SBB = SCT2 * SP2 / SEXD    # 512
SBC = SCT3 * SP2 / SEXD    # 2048
KT = SCT / (SX * SW01)     # 2^-10
K1 = SCH1 / (SCT * SP2)    # 2^-4
K2 = SCT2 / (SCH1 * SW23)  # 2^-7
K3 = SCT3 / (SCT2 * SP2)   # 2^-4
KD = 1.0 / (SCT3 * SP2)    # 2^-17


# ---------------------------------------------------------------------------
# Workaround: this walrus build caps sync-waits per control instruction very
# low, so TileContext's tail drain (which waits on every proc's semaphore)
# fails codegen. Split the global-clock waits into one single-wait
# EventSemaphore each, then emit a bare Drain.
def _patched_drain_and_barrier(self, tick_clock, wait_clock):
    nc = self.nc
    num_to_handle = {h.num: h for h in self.sems.allocated().values()}
    probe = nc.sync.nop(nofuse=True)
    wait_clock.add_sem_waits(probe.ins, ScopedClock({None: tick_clock.global_clock}))
    waits = list(probe.ins.sync_info.on_wait)
    probe.ins.sync_info = bass_rust.SyncInfo(on_wait=[], on_update=[])
    engines = [nc.sync, nc.scalar, nc.vector, nc.tensor, nc.gpsimd]
    for i, w in enumerate(waits):
        h = num_to_handle.get(w.id)
        if h is None:
            raise RuntimeError(f"no sem handle for {w.id} ({w.ant_name})")
        engines[i % len(engines)].wait_ge(h, w.wait_value)
    nc.all_engine_barrier()
    nc.sync.drain()
    assert self.sems is not None
    popped = nc._tile_sem_poison_stack.pop()
    assert popped is self._sem_poison
    nc.clear_and_free_semaphores(list(self.sems.allocated().values()))
    nc.all_engine_barrier()


tile.TileContext._drain_and_barrier = _patched_drain_and_barrier


def _split_multi_waits(nc, max_waits=1):
    """This walrus build rejects instructions carrying more than one sync
    wait. Hoist extra waits into standalone EventSemaphore instructions
    placed immediately before the instruction on the same engine queue."""
    ctr = 0
    for fn in nc.m.functions:
        for bb in fn.blocks:
            insts = bb.instructions
            new = []
            changed = False
            for inst in insts:
                si = inst.sync_info
                waits = list(si.on_wait) if si is not None else []
                if len(waits) > max_waits:
                    changed = True
                    for w in waits[:-max_waits]:
                        ev = bass_rust.InstEventSemaphore(
                            name=f"splitw_{ctr}", ins=[], outs=[]
                        )
                        ctr += 1
                        ev.engine = inst.engine
                        ev.sync_info = bass_rust.SyncInfo(
                            on_wait=[w], on_update=[]
                        )
                        new.append(ev)
                    inst.sync_info = bass_rust.SyncInfo(
                        on_wait=waits[-max_waits:], on_update=list(si.on_update)
                    )
                new.append(inst)
            if changed:
                bb.instructions = new


# ---------------------------------------------------------------------------
# v3 schedule: L0(b0), L0(b1) run back-to-back up front (the front window is
# DMA-bandwidth-bound: w01 + x + p2t = 22.6 MB must land before stage A can
# start; batch 1's L0 gives the PE ~31 us of work that was otherwise idle
# wait). Then chain(b0) stages A..C + collective 0 overlap chain(b1); the
# decoder passes slot in where their inputs are ready (pass 0 fills part of
# the collective-1 wait). DMAs are batched into multi-tile transfers (DGE
# fixed costs ~0.6-1 us/DMA dominate small ones) and ordered by need within
# each queue — queues drain in program order far ahead of the PE, so program
# order per queue IS the bus priority. Decoder weights stream into one
# dedicated 4-tile slot plus group-tiles borrowed from x/t tiles as they die.
# NOTE: both bias DR matmuls (hi and lo) are required — layer-5's bias scale
# (1/sqrt(64)) makes the fp8-hi-only bias error ~5e-2 end-to-end.
def _build_program() -> bass.Bass:
    nc = bass.Bass()

    xhi = nc.declare_dram_parameter("xhi", [BL, NT, 128, CT, 128], F8, isOutput=False)
    xlo = nc.declare_dram_parameter("xlo", [BL, NT, 128, CT, 128], F8, isOutput=False)
    p2thi = nc.declare_dram_parameter("p2thi", [NT, 128, N], F8, isOutput=False)
    p2tlo = nc.declare_dram_parameter("p2tlo", [NT, 128, N], F8, isOutput=False)
    w01hi = nc.declare_dram_parameter("w01hi", [CT, 128, 512], F8, isOutput=False)
    w01lo = nc.declare_dram_parameter("w01lo", [CT, 128, 512], F8, isOutput=False)
    w23hi = nc.declare_dram_parameter("w23hi", [4, 128, 256], F8, isOutput=False)
    w23lo = nc.declare_dram_parameter("w23lo", [4, 128, 256], F8, isOutput=False)
    w45p = nc.declare_dram_parameter("w45p", [2, 128, 3], F16, isOutput=False)
    exdp = nc.declare_dram_parameter("exdp", [4, 2, N], F8, isOutput=False)
    biasA = nc.declare_dram_parameter("biasA", [4, 2, 512], F8, isOutput=False)
    biasB = nc.declare_dram_parameter("biasB", [4, 2, 256], F8, isOutput=False)
    biasC = nc.declare_dram_parameter("biasC", [4, 2, 3], F8, isOutput=False)
    wdp = nc.declare_dram_parameter("wdp", [KDEC, 128, DEC_SH], F16, isOutput=False)
    bdp = nc.declare_dram_parameter("bdp", [128, 6], F32, isOutput=False)
    y_out = nc.declare_dram_parameter("y", [128, 6, B], F32, isOutput=True)

    cc_in = [nc.dram_tensor(f"cc_in{b}", [3, 128, NT], F16) for b in range(BL)]
    cc_out = [
        nc.dram_tensor(f"cc_out{b}", [N_CORES, 3, 128, NT], F16, addr_space="Shared")
        for b in range(BL)
    ]

    with tile.TileContext(nc) as tc:
        with (
            tc.tile_pool(name="const", bufs=1) as constp,
            tc.tile_pool(name="xp", bufs=2) as xpool,
            tc.tile_pool(name="tp", bufs=1) as tpool,
            tc.tile_pool(name="h1p", bufs=1) as h1pool,
            tc.tile_pool(name="t2p", bufs=1) as t2pool,
            tc.tile_pool(name="h3p", bufs=1) as h3pool,
            tc.tile_pool(name="vp", bufs=2) as vpool,
            tc.tile_pool(name="wdpool", bufs=1) as wdpool,
            tc.tile_pool(name="ps", bufs=5, space="PSUM") as ps,
            tc.tile_pool(name="psd", bufs=2, space="PSUM") as psd,
        ):
            # ---- const tiles ----
            w01hi_sb = constp.tile([128, CT, 512], F8, tag="w01hi")
            w01lo_sb = constp.tile([128, CT, 512], F8, tag="w01lo")
            p2thi_sb = constp.tile([128, NT, N], F8, tag="p2thi")
            p2tlo_sb = constp.tile([128, NT, N], F8, tag="p2tlo")
            w23hi_sb = constp.tile([128, 4, 256], F8, tag="w23hi")
            w23lo_sb = constp.tile([128, 4, 256], F8, tag="w23lo")
            w45_sb = constp.tile([128, 2, 3], F16, tag="w45")
            exd_sb = constp.tile([4, 2, N], F8, tag="exd")
            biasA_sb = constp.tile([4, 2, 512], F8, tag="biasA")
            biasB_sb = constp.tile([4, 2, 256], F8, tag="biasB")
            biasC_sb = constp.tile([4, 2, 3], F8, tag="biasC")
            bdp_sb = constp.tile([128, 6], F32, tag="bdp")
            featT_sb = constp.tile([128, B, KDEC], F16, tag="featT")
            y_sb = constp.tile([128, 6, B], F32, tag="ysb")
            t3hi = constp.tile([128, NT * 3], F8, tag="t3hi")
            t3lo = constp.tile([128, NT * 3], F8, tag="t3lo")
            rl5 = constp.tile([128, NT * 3], F16, tag="rl5")
            h5_0 = constp.tile([128, 3, NT], F16, tag="h5_0")
            h5_1 = constp.tile([128, 3, NT], F16, tag="h5_1")
            h5_t = [h5_0, h5_1]

            wd_tiles: list = [None] * KDEC

            prefetched = {}

            def post_x(b, g0, ng, tag_hi, tag_lo):
                """DMA x node-tile groups [2*g0, 2*(g0+ng)) as ONE transfer
                per hi/lo half (DGE fixed costs dominate small DMAs).
                Singles and pairs use distinct tags so slot reuse (WAR) never
                blocks the stream head-of-line."""
                ghi = xpool.tile([128, 2 * ng, CT, 128], F8, tag=tag_hi)
                glo = xpool.tile([128, 2 * ng, CT, 128], F8, tag=tag_lo)
                nc.sync.dma_start(
                    ghi[:], xhi[b, 2 * g0 : 2 * g0 + 2 * ng].rearrange(
                        "n p c j -> p n c j"))
                nc.gpsimd.dma_start(
                    glo[:], xlo[b, 2 * g0 : 2 * g0 + 2 * ng].rearrange(
                        "n p c j -> p n c j"))
                for i in range(ng):
                    prefetched[(b, g0 + i)] = ((ghi, glo), i)

            # Startup: group 0 of L0 needs ALL of w01hi within ~0.6us of its
            # first matmul, so the full block goes first; then x in pair
            # transfers. Batch 1 gets its own tags (x1*) so its DMAs carry no
            # WAR on batch-0 slots — a WAR head-of-line-blocks the queue and
            # lets p2t cut ahead on the shared bus.
            nc.sync.dma_start(
                w01hi_sb[:], w01hi[:].rearrange("c p f -> p c f"))
            post_x(0, 0, 2, "xhi", "xlo")
            nc.gpsimd.dma_start(
                w01lo_sb[:], w01lo[:].rearrange("c p f -> p c f"))

            def emit_small_consts():
                nc.sync.dma_start(w23hi_sb[:], w23hi[:].rearrange("c p f -> p c f"))
                nc.sync.dma_start(w23lo_sb[:], w23lo[:].rearrange("c p f -> p c f"))
                nc.sync.dma_start(w45_sb[:], w45p[:].rearrange("c p f -> p c f"))
                nc.sync.dma_start(exd_sb[:], exdp[:])
                nc.sync.dma_start(biasA_sb[:], biasA[:])
                nc.sync.dma_start(biasB_sb[:], biasB[:])
                nc.sync.dma_start(biasC_sb[:], biasC[:])
                nc.sync.dma_start(bdp_sb[:], bdp[:])

            # p2t quad-row DMAs: hi on the HWDGE (sync) path, lo via the
            # otherwise-idle Pool engine's SWDGE; 4 src tiles per transfer to
            # amortize the ~1us per-DMA DGE cost.
            p2_rows = [(4 * i, 0) for i in range(4)] + [(4 * i, 1) for i in range(4)]
            p2_pos = [0]

            def emit_p2(n):
                for _ in range(n):
                    if p2_pos[0] >= len(p2_rows):
                        return
                    si, hl = p2_rows[p2_pos[0]]
                    p2_pos[0] += 1
                    if hl == 0:
                        nc.sync.dma_start(
                            p2thi_sb[:, si : si + 4, :],
                            p2thi[si : si + 4].rearrange("s p n -> p s n"))
                    else:
                        nc.gpsimd.dma_start(
                            p2tlo_sb[:, si : si + 4, :],
                            p2tlo[si : si + 4].rearrange("s p n -> p s n"))

            wd_pos = [0]
            wd_direct = [0]
            N_WD_DIRECT_GROUPS = 1

            def borrow_wd(pool, tag, n):
                """Land n decoder k-tiles in one group DMA into a dead tile's
                slot (tag must never be allocated again afterwards)."""
                k0 = wd_pos[0]
                assert k0 + n <= KDEC
                wd_pos[0] += n
                gt = pool.tile([128, n, DEC_SH], F16, tag=tag)
                nc.gpsimd.dma_start(
                    gt[:], wdp[k0 : k0 + n].rearrange("k p f -> p k f"))
                for i in range(n):
                    wd_tiles[k0 + i] = gt[:, i, :]

            def emit_wd(n):
                """Stream decoder-weight k-tiles into the dedicated pool,
                4 per transfer."""
                for _ in range(n):
                    if wd_direct[0] >= N_WD_DIRECT_GROUPS or wd_pos[0] + 4 > KDEC:
                        return
                    wd_direct[0] += 1
                    borrow_wd(wdpool, "wd", 4)

            def l0_stage(b, t_hi, t_lo):
                # ---- L0: t = x @ W01 (3-term fp8 DR) ----
                # x streams in node-tile groups: hi via HWDGE, lo via SWDGE.
                # Groups 0/1 land as singles (startup latency), the rest as
                # pair transfers; batch 1's stream is fully posted during
                # batch 0's loop.
                for g in range(NT // 2):
                    (ghi, glo), gi = prefetched.pop((b, g))
                    for k in range(2):
                        nt = 2 * g + k
                        pt = ps.tile([128, 512], F32, tag="ps")
                        first = True
                        for p in range(CT // 2):
                            nc.tensor.matmul(
                                pt[:], ghi[:, 2 * gi + k, 2 * p : 2 * p + 2, :],
                                w01hi_sb[:, 2 * p : 2 * p + 2, :],
                                start=first, stop=False, perf_mode=DRM,
                            )
                            first = False
                        for p in range(CT // 2):
                            nc.tensor.matmul(
                                pt[:], glo[:, 2 * gi + k, 2 * p : 2 * p + 2, :],
                                w01hi_sb[:, 2 * p : 2 * p + 2, :],
                                start=False, stop=False, perf_mode=DRM,
                            )
                        for p in range(CT // 2):
                            nc.tensor.matmul(
                                pt[:], ghi[:, 2 * gi + k, 2 * p : 2 * p + 2, :],
                                w01lo_sb[:, 2 * p : 2 * p + 2, :],
                                start=False, stop=(p == CT // 2 - 1),
                                perf_mode=DRM,
                            )
                        nc.scalar.activation(t_hi[:, nt, :], pt[:], ACTF.Copy,
                                             scale=KT)
                        nc.vector.scalar_tensor_tensor(
                            t_lo[:, nt, :], pt[:], KT, t_hi[:, nt, :],
                            ALU.mult, ALU.subtract,
                        )
                    if b == 0:
                        if g == 0:
                            post_x(0, 2, 2, "xhi", "xlo")
                        elif g == 1:
                            post_x(0, 4, 2, "xhi", "xlo")
                        elif g == 2:
                            post_x(0, 6, 2, "xhi", "xlo")
                        elif g == 3:
                            emit_small_consts()
                        elif g == 5:
                            post_x(1, 0, 2, "x1hi", "x1lo")
                        elif g == 6:
                            post_x(1, 2, 2, "x1hi", "x1lo")
                        elif g == 7:
                            post_x(1, 4, 2, "x1hi", "x1lo")
                            post_x(1, 6, 2, "x1hi", "x1lo")

            def prop_stage(b, src_hi, src_lo, bias_sb, nf, out_cb, wd_budget):
                # ---- z = P2 @ src + bias (single hi-bias DR matmul) ----
                # nf: number of 128-wide feature tiles in src (4 for stage A,
                # 2 for stage B). out_cb(dc, fj, pt) evicts the PSUM tile.
                for dc in range(4):
                    dsl = slice(dc * 512, (dc + 1) * 512)
                    for fj in range(nf):
                        fsl = slice(fj * 128, (fj + 1) * 128)
                        pt = ps.tile([128, 512], F32, tag="ps")
                        for sp in range(NT // 2):
                            ssl = slice(2 * sp, 2 * sp + 2)
                            nc.tensor.matmul(
                                pt[:], src_hi[:, ssl, fsl],
                                p2thi_sb[:, ssl, dsl],
                                start=(sp == 0), stop=False, perf_mode=DRM,
                            )
                        for sp in range(NT // 2):
                            ssl = slice(2 * sp, 2 * sp + 2)
                            nc.tensor.matmul(
                                pt[:], src_lo[:, ssl, fsl],
                                p2thi_sb[:, ssl, dsl],
                                start=False, stop=False, perf_mode=DRM,
                            )
                        for sp in range(NT // 2):
                            ssl = slice(2 * sp, 2 * sp + 2)
                            nc.tensor.matmul(
                                pt[:], src_hi[:, ssl, fsl],
                                p2tlo_sb[:, ssl, dsl],
                                start=False, stop=False, perf_mode=DRM,
                            )
                        # bias: all four hi/lo row-pairs packed into one
                        # K=4x2 DoubleRow matmul (exact same sum as the old
                        # two-instruction form)
                        nc.tensor.matmul(
                            pt[:], bias_sb[:, :, fsl], exd_sb[:, :, dsl],
                            start=False, stop=True, perf_mode=DRM,
                        )
                        out_cb(dc, fj, pt)
                        emit_wd(wd_budget)

            def stageA(b, t_hi, t_lo, h1hi, h1lo):
                def evict(dc, fj, pt):
                    dsl = slice(dc * 512, (dc + 1) * 512)
                    rl = vpool.tile([128, 512], F16, tag="rl")
                    nc.scalar.activation(rl[:], pt[:], ACTF.Relu,
                                         scale=0.99 * K1)
                    v = vpool.tile([128, 512], F16, tag="v")
                    nc.vector.scalar_tensor_tensor(
                        v[:], pt[:], ALPHA * K1, rl[:], ALU.mult, ALU.add)
                    nc.scalar.activation(h1hi[:, fj, dsl], v[:], ACTF.Copy)
                    nc.vector.tensor_tensor(
                        h1lo[:, fj, dsl], v[:], h1hi[:, fj, dsl],
                        ALU.subtract)

                prop_stage(b, t_hi, t_lo, biasA_sb, 4, evict,
                           1 if b == 0 else 0)

            def w23_stage(h1hi, h1lo, t2hi, t2lo):
                # ---- W23: t2 = h1 @ W23 ----
                for nt in range(NT):
                    nsl = slice(nt * 128, (nt + 1) * 128)
                    pt = ps.tile([128, 512], F32, tag="ps")
                    for fp in range(2):
                        fsl = slice(2 * fp, 2 * fp + 2)
                        nc.tensor.matmul(
                            pt[:, 0:256], h1hi[:, fsl, nsl], w23hi_sb[:, fsl, :],
                            start=(fp == 0), stop=False, perf_mode=DRM,
                        )
                    for fp in range(2):
                        fsl = slice(2 * fp, 2 * fp + 2)
                        nc.tensor.matmul(
                            pt[:, 0:256], h1lo[:, fsl, nsl], w23hi_sb[:, fsl, :],
                            start=False, stop=False, perf_mode=DRM,
                        )
                    for fp in range(2):
                        fsl = slice(2 * fp, 2 * fp + 2)
                        nc.tensor.matmul(
                            pt[:, 0:256], h1hi[:, fsl, nsl], w23lo_sb[:, fsl, :],
                            start=False, stop=(fp == 1), perf_mode=DRM,
                        )
                    nc.scalar.activation(t2hi[:, nt, :], pt[:, 0:256], ACTF.Copy,
                                         scale=K2)
                    nc.vector.scalar_tensor_tensor(
                        t2lo[:, nt, :], pt[:, 0:256], K2, t2hi[:, nt, :],
                        ALU.mult, ALU.subtract,
                    )

            def stageB(b, t2hi, t2lo, h3):
                def evict(dc, fj, pt):
                    dsl = slice(dc * 512, (dc + 1) * 512)
                    rl = vpool.tile([128, 512], F16, tag="rl")
                    nc.scalar.activation(rl[:], pt[:], ACTF.Relu, scale=0.99)
                    nc.vector.scalar_tensor_tensor(
                        h3[:, fj, dsl], pt[:], ALPHA, rl[:],
                        ALU.mult, ALU.add)

                prop_stage(b, t2hi, t2lo, biasB_sb, 2, evict, 0)

            def tail_stages(b, h3):
                # ---- W45: t3 = h3 @ W45 (fp16) ----
                # psd bank: no wait on stage-B eviction chains draining ps
                pt45 = psd.tile([128, 512], F32, tag="psd")
                for nt in range(NT):
                    nsl = slice(nt * 128, (nt + 1) * 128)
                    for cj in range(2):
                        nc.tensor.matmul(
                            pt45[:, nt * 3 : nt * 3 + 3],
                            h3[:, cj, nsl], w45_sb[:, cj, :],
                            start=(cj == 0), stop=(cj == 1),
                        )
                nc.scalar.activation(t3hi[:], pt45[:, 0 : NT * 3], ACTF.Copy,
                                     scale=K3)
                nc.vector.scalar_tensor_tensor(
                    t3lo[:], pt45[:, 0 : NT * 3], K3, t3hi[:],
                    ALU.mult, ALU.subtract,
                )

                # ---- stage C: z5 = P2 @ t3 + bias ; h5 = leaky(z5) ----
                psC = psd.tile([128, 512], F32, tag="psd")
                for di in range(NT):
                    dsl = slice(di * 128, (di + 1) * 128)
                    osl = slice(di * 3, di * 3 + 3)
                    for sp in range(NT // 2):
                        t3sl = t3hi[:, 6 * sp : 6 * sp + 6].rearrange(
                            "p (s c) -> p s c", s=2, c=3)
                        nc.tensor.matmul(
                            psC[:, osl], p2thi_sb[:, 2 * sp : 2 * sp + 2, dsl],
                            t3sl, start=(sp == 0), stop=False, perf_mode=DRM,
                        )
                    for sp in range(NT // 2):
                        t3sl = t3lo[:, 6 * sp : 6 * sp + 6].rearrange(
                            "p (s c) -> p s c", s=2, c=3)
                        nc.tensor.matmul(
                            psC[:, osl], p2thi_sb[:, 2 * sp : 2 * sp + 2, dsl],
                            t3sl, start=False, stop=False, perf_mode=DRM,
                        )
                    for sp in range(NT // 2):
                        t3sl = t3hi[:, 6 * sp : 6 * sp + 6].rearrange(
                            "p (s c) -> p s c", s=2, c=3)
                        nc.tensor.matmul(
                            psC[:, osl], p2tlo_sb[:, 2 * sp : 2 * sp + 2, dsl],
                            t3sl, start=False, stop=False, perf_mode=DRM,
                        )
                    nc.tensor.matmul(
                        psC[:, osl], exd_sb[:, :, dsl], biasC_sb[:],
                        start=False, stop=True, perf_mode=DRM,
                    )
                nc.scalar.activation(rl5[:], psC[:, 0 : NT * 3], ACTF.Relu,
                                     scale=0.99)
                nc.vector.scalar_tensor_tensor(
                    h5_t[b][:].rearrange("p c d -> p d c"),
                    psC[:, 0 : NT * 3].rearrange("p (d c) -> p d c", d=NT, c=3),
                    ALPHA,
                    rl5[:].rearrange("p (d c) -> p d c", d=NT, c=3),
                    ALU.mult, ALU.add,
                )
                nc.sync.dma_start(
                    cc_in[b][:].rearrange("c p n -> p c n"), h5_t[b][:])
                nc.gpsimd.collective_compute(
                    "AllGather",
                    ALU.bypass,
                    replica_groups=[list(range(N_CORES))],
                    ins=[cc_in[b][:]],
                    outs=[cc_out[b][:]],
                )

            def featT_dma(half):
                for c in range(3):
                    nc.sync.dma_start(
                        featT_sb[:, half : B : BL, c * NT : (c + 1) * NT],
                        cc_out[half][:, c].rearrange("core p n -> p core n"),
                    )

            def decoder_pass(half):
                pd = psd.tile([128, 512], F32, tag="psd")
                for kt in range(KDEC):
                    rhs = featT_sb[:, half : B : BL, kt : kt + 1]
                    for ct in range(6):
                        nc.tensor.matmul(
                            pd[:, ct * 8 : (ct + 1) * 8],
                            wd_tiles[kt][:, ct * 128 : (ct + 1) * 128],
                            rhs, start=(kt == 0), stop=(kt == KDEC - 1),
                        )
                for ct in range(6):
                    nc.scalar.activation(
                        y_sb[:, ct, half : B : BL], pd[:, ct * 8 : (ct + 1) * 8],
                        ACTF.Tanh, bias=bdp_sb[:, ct : ct + 1], scale=KD,
                    )

            # ---- emission schedule ----
            t_hi0 = tpool.tile([128, NT, 512], F8, tag="t_hi0")
            t_lo0 = tpool.tile([128, NT, 512], F8, tag="t_lo0")
            t_hi1 = tpool.tile([128, NT, 512], F8, tag="t_hi1")
            t_lo1 = tpool.tile([128, NT, 512], F8, tag="t_lo1")

            l0_stage(0, t_hi0, t_lo0)
            # All x for both batches is now posted on the sync/pool queues;
            # p2t goes behind it in the same queues, so the shared DMA bus
            # serves x strictly first (queues drain in program order and run
            # far ahead of the PE — emission points alone don't pace DMA).
            emit_p2(8)
            l0_stage(1, t_hi1, t_lo1)
            # x tiles are dead from here: 24 decoder k-tiles into their slots
            for tag in ("xhi", "xlo", "x1hi"):
                borrow_wd(xpool, tag, 4)
                borrow_wd(xpool, tag, 4)

            h1hi = h1pool.tile([128, 4, N], F8, tag="h1hi")
            h1lo = h1pool.tile([128, 4, N], F8, tag="h1lo")
            stageA(0, t_hi0, t_lo0, h1hi, h1lo)
            emit_wd(N_WD_DIRECT_GROUPS)  # any remainder
            borrow_wd(tpool, "t_hi0", 5)
            borrow_wd(tpool, "t_lo0", 5)

            t2hi = t2pool.tile([128, NT, 256], F8, tag="t2hi")
            t2lo = t2pool.tile([128, NT, 256], F8, tag="t2lo")
            w23_stage(h1hi, h1lo, t2hi, t2lo)
            h3 = h3pool.tile([128, 2, N], F16, tag="h3")
            stageB(0, t2hi, t2lo, h3)
            tail_stages(0, h3)
            featT_dma(0)

            h1hi = h1pool.tile([128, 4, N], F8, tag="h1hi")
            h1lo = h1pool.tile([128, 4, N], F8, tag="h1lo")
            stageA(1, t_hi1, t_lo1, h1hi, h1lo)
            borrow_wd(tpool, "t_hi1", 5)
            borrow_wd(tpool, "t_lo1", 5)
            assert wd_pos[0] == KDEC, wd_pos[0]

            t2hi = t2pool.tile([128, NT, 256], F8, tag="t2hi")
            t2lo = t2pool.tile([128, NT, 256], F8, tag="t2lo")
            w23_stage(h1hi, h1lo, t2hi, t2lo)
            h3 = h3pool.tile([128, 2, N], F16, tag="h3")
            stageB(1, t2hi, t2lo, h3)
            tail_stages(1, h3)
            decoder_pass(0)  # fills part of the cc1 wait
            featT_dma(1)
            decoder_pass(1)
            nc.sync.dma_start(y_out[:], y_sb[:])

    _split_multi_waits(nc)
    return nc


# ---------------------------------------------------------------------------
def _split8(a):
    hi = np.asarray(a, np.float32).astype(NPF8)
    lo = (np.asarray(a, np.float64) - hi.astype(np.float64)).astype(
        np.float32).astype(NPF8)
    return hi, lo


def _host_precompute(x, edges, Ws, bs, Wd, bd_np):
    edges = np.asarray(edges)
    src = edges[0].astype(np.int64)
    dst = edges[1].astype(np.int64)

    deg = np.bincount(dst, minlength=N).astype(np.float64) + 1.0
    isd = 1.0 / np.sqrt(deg)
    idg = 1.0 / deg

    P = np.zeros((N, N), np.float64)
    np.add.at(P, (dst, src), isd[src] * isd[dst])
    P[np.arange(N), np.arange(N)] += idg
    P2 = P @ P
    r = P.sum(axis=1)

    W0, W1, W2, W3, W4, W5 = [np.asarray(w, np.float64) for w in Ws]
    b0, b1, b2, b3, b4, b5 = [np.asarray(b, np.float64) for b in bs]
    W01 = W0 @ W1
    W23 = W2 @ W3
    W45 = W4 @ W5
    a1 = b0 @ W1
    a3 = b2 @ W3
    a5 = b4 @ W5

    # p2t[si, p, d] = P2[d, si*128+p] * SP2
    p2t_s = np.ascontiguousarray((P2.T * SP2).reshape(NT, 128, N))
    p2thi, p2tlo = _split8(p2t_s)

    w01_pad = np.zeros((C_PAD, 512), np.float64)
    w01_pad[:C_IN] = W01
    w01hi, w01lo = _split8((w01_pad * SW01).reshape(CT, 128, 512))
    w23hi, w23lo = _split8((W23 * SW23).reshape(4, 128, 256))
    w45_np = W45.reshape(2, 128, 3).astype(NPF16)

    # Packed bias operands: one K=4x2 DoubleRow matmul computes
    #   (a_hi+a_lo)(r_hi+r_lo) + (b_hi+b_lo)*SEXD
    # exactly as the old two-matmul form.
    rhi, rlo = _split8(r * SEXD)
    one8 = np.full(N, SEXD, np.float32).astype(NPF8)
    exdp_np = np.zeros((4, 2, N), NPF8)
    exdp_np[0] = np.stack([rhi, rlo])
    exdp_np[1] = np.stack([rhi, rlo])
    exdp_np[2] = np.stack([one8, one8])
    exdp = np.ascontiguousarray(exdp_np)

    def bias4(a, bvec, s):
        ahi, alo = _split8(np.asarray(a) * s)
        bhi, blo = _split8(np.asarray(bvec) * s)
        out = np.zeros((4, 2, len(ahi)), NPF8)
        out[0] = np.stack([ahi, ahi])
        out[1] = np.stack([alo, alo])
        out[2] = np.stack([bhi, blo])
        return np.ascontiguousarray(out)

    biasA_np = bias4(a1, b1, SBA)
    biasB_np = bias4(a3, b3, SBB)
    biasC_np = bias4(a5, b5, SBC)

    # x: pad channels, scale, split; layout [BL,NT,p=chan,CT,j=node]
    x_np = np.asarray(x, np.float32)
    x_pad = np.zeros((B, N, C_PAD), np.float32)
    x_pad[:, :, :C_IN] = x_np * SX
    xt_all = np.ascontiguousarray(
        x_pad.reshape(B, NT, 128, CT, 128).transpose(0, 1, 4, 3, 2))
    xhi_all, xlo_all = _split8(xt_all)

    # decoder: permuted rows j' = c*2048 + node
    Wd_np = np.asarray(Wd, np.float64)
    node = np.arange(N)
    rows = np.concatenate([node * 3 + c for c in range(3)])  # j' -> orig j
    Wd_perm = Wd_np[rows]  # [6144, 6144] in j' order
    bd_full = np.asarray(bd_np, np.float64)

    shared = {
        "p2thi": p2thi, "p2tlo": p2tlo,
        "w01hi": w01hi, "w01lo": w01lo,
        "w23hi": w23hi, "w23lo": w23lo,
        "w45p": w45_np,
        "exdp": exdp,
        "biasA": biasA_np, "biasB": biasB_np, "biasC": biasC_np,
    }
    in_maps = []
    for c in range(N_CORES):
        wd_c = np.ascontiguousarray(
            Wd_perm[:, c * DEC_SH : (c + 1) * DEC_SH]
            .reshape(KDEC, 128, DEC_SH).astype(NPF16))
        bd_c = np.ascontiguousarray(
            bd_full[c * DEC_SH : (c + 1) * DEC_SH]
            .reshape(6, 128).T.astype(np.float32))  # [128, 6]
        m = dict(shared)
        m["xhi"] = xhi_all[c * BL : (c + 1) * BL]
        m["xlo"] = xlo_all[c * BL : (c + 1) * BL]
        m["wdp"] = wd_c
        m["bdp"] = bd_c
        in_maps.append(m)
    return in_maps


_NC_CACHE = {}


def kernel(**inputs) -> np.ndarray:
    x = inputs["x"]
    edges = inputs["edges"]
    Ws = [inputs[f"W{i}"] for i in range(6)]
    bs = [inputs[f"b{i}"] for i in range(6)]
    Wd = inputs["Wd"]
    bd_np = inputs["bd"]

    in_maps = _host_precompute(x, edges, Ws, bs, Wd, bd_np)

    if "nc" not in _NC_CACHE:
        _NC_CACHE["nc"] = _build_program()
    nc = _NC_CACHE["nc"]

    res = run_bass_kernel_spmd(nc, in_maps, list(range(N_CORES)))

    out = np.zeros((B, D_DEC), np.float32)
    for c in range(N_CORES):
        y = res.results[c]["y"]  # [128, 6, 16]
        out[:, c * DEC_SH : (c + 1) * DEC_SH] = (
            0.1 * y.transpose(2, 1, 0).reshape(B, DEC_SH))
    return out.reshape(B, N, 3)



# revision 40
# speedup vs baseline: 1.0879x; 1.0143x over previous
"""Trainium2 Bass kernel for nn_DeformGCN (6-layer GCN + dense decoder).

Strategy (v3, fp8 DoubleRow):
  - Host precompute from `edges`: dense propagation matrix P (N x N) with
    P[dst,src] += 1/sqrt(deg_s*deg_d) and P[n,n] += 1/deg_n, then P2 = P @ P.
    GCN layer pairs fuse into 3 propagation stages (A, B, C):
      z = P2 @ (h @ (Wa@Wb)) + r (x) (ba@Wb) + 1 (x) bb,  r = P @ 1
    followed by LeakyReLU(0.01).
  - All heavy matmuls run as fp8e4m3 DoubleRow (2 x 128-deep products per
    instruction at 0.5 cycles/row = 4x bf16 MAC rate). Accuracy is restored
    with a hi/lo residual split of both operands; the lo*lo term is dropped
    (3-term scheme, 0.75x the bf16 row count). End-to-end rel err ~2e-3.
  - Activations are split on the fly during PSUM eviction:
      hi = ACT copy(psum, scale=k) -> fp8 ; lo = DVE (psum*k - hi) -> fp8
    LeakyReLU uses  v = 0.99k*relu(z) (ACT)  +  0.01k*z (DVE stt add).
  - Small stages (W45, decoder) run in fp16 (1.0 cycles/row, 10-bit mantissa).
  - Decoder is column-sharded (768 cols/core) and oriented [col_part x batch]
    so its cost is 48*6*16 rows. Features are AllGather'd per local batch (2
    collectives; the first fully overlaps with batch 1's GCN compute, and
    decoder pass 0 fills part of the second collective's wait).
  - Biases ride the propagation matmuls as ONE K=4x2 DoubleRow matmul per
    PSUM group: rows (a*r)hi/lo and (b*1)hi/lo packed on 4 partitions --
    exactly the old two-instruction sum at half the PE cost. Both hi and lo
    bias halves are required: layer-5's bias scale (1/sqrt(64)) makes an
    fp8-hi-only bias error ~5e-2 end-to-end.
  - v3 schedule: both batches' L0 run back-to-back up front (the front is
    DMA-bus-bound: w01 + x + p2t = 22.6 MB must land before stage A starts;
    batch 1's L0 fills the otherwise-idle PE). DMAs are batched into
    multi-tile transfers (DGE fixed costs ~0.6-1 us each dominate small
    ones); each queue's program order IS the bus priority since queues run
    far ahead of the PE. Batch 1's x stream uses its own SBUF tags so slot
    WARs never head-of-line-block the queue behind them. W45/stage-C PSUM
    comes from the decoder's pool so it never waits on stage-B evictions.
"""

import os
import numpy as np
import ml_dtypes

_STAGE_LIMIT = int(os.environ.get("KSTAGE", "99"))

import bass_rust
import concourse.bass as bass
import concourse.mybir as mybir
import concourse.tile as tile
from concourse.tile import ScopedClock
from concourse.bass_utils import run_bass_kernel_spmd

F8 = mybir.dt.float8e4
F16 = mybir.dt.float16
F32 = mybir.dt.float32
NPF8 = ml_dtypes.float8_e4m3
NPF16 = np.float16
DRM = mybir.MatmulPerfMode.DoubleRow
ALU = mybir.AluOpType
ACTF = mybir.ActivationFunctionType

N_CORES = 8
B = 16
N = 2048
C_IN = 1475
C_PAD = 1536           # 12 * 128
NT = N // 128          # 16 node tiles
CT = C_PAD // 128      # 12 channel tiles
BL = B // N_CORES      # 2 local batches
D_DEC = N * 3          # 6144
DEC_SH = D_DEC // N_CORES  # 768 decoder columns per core
KDEC = D_DEC // 128    # 48 decoder k tiles
ALPHA = 0.01

# scales (powers of two; products of the two operand scales give the PSUM
# scale, the ACT evict rescales to the next storage scale)
SX = 16.0
SW01 = 2048.0
SCT = 32.0
SP2 = 64.0
SCH1 = 128.0
SW23 = 512.0
SCT2 = 512.0
SCT3 = 2048.0
SEXD = 64.0
SBA = SCT * SP2 / SEXD     # 32
SBB = SCT2 * SP2 / SEXD    # 512
SBC = SCT3 * SP2 / SEXD    # 2048
KT = SCT / (SX * SW01)     # 2^-10
K1 = SCH1 / (SCT * SP2)    # 2^-4
K2 = SCT2 / (SCH1 * SW23)  # 2^-7
K3 = SCT3 / (SCT2 * SP2)   # 2^-4
KD = 1.0 / (SCT3 * SP2)    # 2^-17


# ---------------------------------------------------------------------------
# Workaround: this walrus build caps sync-waits per control instruction very
# low, so TileContext's tail drain (which waits on every proc's semaphore)
# fails codegen. Split the global-clock waits into one single-wait
# EventSemaphore each, then emit a bare Drain.
def _patched_drain_and_barrier(self, tick_clock, wait_clock):
    nc = self.nc
    num_to_handle = {h.num: h for h in self.sems.allocated().values()}
    probe = nc.sync.nop(nofuse=True)
    wait_clock.add_sem_waits(probe.ins, ScopedClock({None: tick_clock.global_clock}))
    waits = list(probe.ins.sync_info.on_wait)
    probe.ins.sync_info = bass_rust.SyncInfo(on_wait=[], on_update=[])
    engines = [nc.sync, nc.scalar, nc.vector, nc.tensor, nc.gpsimd]
    for i, w in enumerate(waits):
        h = num_to_handle.get(w.id)
        if h is None:
            raise RuntimeError(f"no sem handle for {w.id} ({w.ant_name})")
        engines[i % len(engines)].wait_ge(h, w.wait_value)
    nc.all_engine_barrier()
    nc.sync.drain()
    assert self.sems is not None
    popped = nc._tile_sem_poison_stack.pop()
    assert popped is self._sem_poison
    nc.clear_and_free_semaphores(list(self.sems.allocated().values()))
    nc.all_engine_barrier()


tile.TileContext._drain_and_barrier = _patched_drain_and_barrier


def _split_multi_waits(nc, max_waits=1):
    """This walrus build rejects instructions carrying more than one sync
    wait. Hoist extra waits into standalone EventSemaphore instructions
    placed immediately before the instruction on the same engine queue."""
    ctr = 0
    for fn in nc.m.functions:
        for bb in fn.blocks:
            insts = bb.instructions
            new = []
            changed = False
            for inst in insts:
                si = inst.sync_info
                waits = list(si.on_wait) if si is not None else []
                if len(waits) > max_waits:
                    changed = True
                    for w in waits[:-max_waits]:
                        ev = bass_rust.InstEventSemaphore(
                            name=f"splitw_{ctr}", ins=[], outs=[]
                        )
                        ctr += 1
                        ev.engine = inst.engine
                        ev.sync_info = bass_rust.SyncInfo(
                            on_wait=[w], on_update=[]
                        )
                        new.append(ev)
                    inst.sync_info = bass_rust.SyncInfo(
                        on_wait=waits[-max_waits:], on_update=list(si.on_update)
                    )
                new.append(inst)
            if changed:
                bb.instructions = new


# ---------------------------------------------------------------------------
# v3 schedule: L0(b0), L0(b1) run back-to-back up front (the front window is
# DMA-bandwidth-bound: w01 + x + p2t = 22.6 MB must land before stage A can
# start; batch 1's L0 gives the PE ~31 us of work that was otherwise idle
# wait). Then chain(b0) stages A..C + collective 0 overlap chain(b1); the
# decoder passes slot in where their inputs are ready (pass 0 fills part of
# the collective-1 wait). DMAs are batched into multi-tile transfers (DGE
# fixed costs ~0.6-1 us/DMA dominate small ones) and ordered by need within
# each queue — queues drain in program order far ahead of the PE, so program
# order per queue IS the bus priority. Decoder weights stream into one
# dedicated 4-tile slot plus group-tiles borrowed from x/t tiles as they die.
# NOTE: both bias DR matmuls (hi and lo) are required — layer-5's bias scale
# (1/sqrt(64)) makes the fp8-hi-only bias error ~5e-2 end-to-end.
def _build_program() -> bass.Bass:
    nc = bass.Bass()

    xhi = nc.declare_dram_parameter("xhi", [BL, NT, 128, CT, 128], F8, isOutput=False)
    xlo = nc.declare_dram_parameter("xlo", [BL, NT, 128, CT, 128], F8, isOutput=False)
    p2thi = nc.declare_dram_parameter("p2thi", [NT, 128, N], F8, isOutput=False)
    p2tlo = nc.declare_dram_parameter("p2tlo", [NT, 128, N], F8, isOutput=False)
    w01hi = nc.declare_dram_parameter("w01hi", [CT, 128, 512], F8, isOutput=False)
    w01lo = nc.declare_dram_parameter("w01lo", [CT, 128, 512], F8, isOutput=False)
    w23hi = nc.declare_dram_parameter("w23hi", [4, 128, 256], F8, isOutput=False)
    w23lo = nc.declare_dram_parameter("w23lo", [4, 128, 256], F8, isOutput=False)
    w45p = nc.declare_dram_parameter("w45p", [2, 128, 3], F16, isOutput=False)
    exdp = nc.declare_dram_parameter("exdp", [4, 2, N], F8, isOutput=False)
    biasA = nc.declare_dram_parameter("biasA", [4, 2, 512], F8, isOutput=False)
    biasB = nc.declare_dram_parameter("biasB", [4, 2, 256], F8, isOutput=False)
    biasC = nc.declare_dram_parameter("biasC", [4, 2, 3], F8, isOutput=False)
    wdp = nc.declare_dram_parameter("wdp", [KDEC, 128, DEC_SH], F16, isOutput=False)
    bdp = nc.declare_dram_parameter("bdp", [128, 6], F32, isOutput=False)
    y_out = nc.declare_dram_parameter("y", [128, 6, B], F32, isOutput=True)

    cc_in = [nc.dram_tensor(f"cc_in{b}", [3, 128, NT], F16) for b in range(BL)]
    cc_out = [
        nc.dram_tensor(f"cc_out{b}", [N_CORES, 3, 128, NT], F16, addr_space="Shared")
        for b in range(BL)
    ]

    with tile.TileContext(nc) as tc:
        with (
            tc.tile_pool(name="const", bufs=1) as constp,
            tc.tile_pool(name="xp", bufs=2) as xpool,
            tc.tile_pool(name="tp", bufs=1) as tpool,
            tc.tile_pool(name="h1p", bufs=1) as h1pool,
            tc.tile_pool(name="t2p", bufs=1) as t2pool,
            tc.tile_pool(name="h3p", bufs=1) as h3pool,
            tc.tile_pool(name="vp", bufs=2) as vpool,
            tc.tile_pool(name="wdpool", bufs=1) as wdpool,
            tc.tile_pool(name="ps", bufs=5, space="PSUM") as ps,
            tc.tile_pool(name="psd", bufs=2, space="PSUM") as psd,
        ):
            # ---- const tiles ----
            w01hi_sb = constp.tile([128, CT, 512], F8, tag="w01hi")
            w01lo_sb = constp.tile([128, CT, 512], F8, tag="w01lo")
            p2thi_sb = constp.tile([128, NT, N], F8, tag="p2thi")
            p2tlo_sb = constp.tile([128, NT, N], F8, tag="p2tlo")
            w23hi_sb = constp.tile([128, 4, 256], F8, tag="w23hi")
            w23lo_sb = constp.tile([128, 4, 256], F8, tag="w23lo")
            w45_sb = constp.tile([128, 2, 3], F16, tag="w45")
            exd_sb = constp.tile([4, 2, N], F8, tag="exd")
            biasA_sb = constp.tile([4, 2, 512], F8, tag="biasA")
            biasB_sb = constp.tile([4, 2, 256], F8, tag="biasB")
            biasC_sb = constp.tile([4, 2, 3], F8, tag="biasC")
            bdp_sb = constp.tile([128, 6], F32, tag="bdp")
            featT_sb = constp.tile([128, B, KDEC], F16, tag="featT")
            y_sb = constp.tile([128, 6, B], F32, tag="ysb")
            t3hi = constp.tile([128, NT * 3], F8, tag="t3hi")
            t3lo = constp.tile([128, NT * 3], F8, tag="t3lo")
            rl5 = constp.tile([128, NT * 3], F16, tag="rl5")
            h5_0 = constp.tile([128, 3, NT], F16, tag="h5_0")
            h5_1 = constp.tile([128, 3, NT], F16, tag="h5_1")
            h5_t = [h5_0, h5_1]

            wd_tiles: list = [None] * KDEC

            prefetched = {}

            def post_x(b, g0, ng, tag_hi, tag_lo):
                """DMA x node-tile groups [2*g0, 2*(g0+ng)) as ONE transfer
                per hi/lo half (DGE fixed costs dominate small DMAs). Batch 1
                rides the scalar/vector HWDGE queues with its own tags, so a
                WAR slot-reuse wait in one stream never head-of-line-blocks
                another stream's DMAs."""
                ghi = xpool.tile([128, 2 * ng, CT, 128], F8, tag=tag_hi)
                glo = xpool.tile([128, 2 * ng, CT, 128], F8, tag=tag_lo)
                nc.sync.dma_start(
                    ghi[:], xhi[b, 2 * g0 : 2 * g0 + 2 * ng].rearrange(
                        "n p c j -> p n c j"))
                nc.gpsimd.dma_start(
                    glo[:], xlo[b, 2 * g0 : 2 * g0 + 2 * ng].rearrange(
                        "n p c j -> p n c j"))
                for i in range(ng):
                    prefetched[(b, g0 + i)] = ((ghi, glo), i)

            # Startup: group 0 of L0 needs ALL of w01hi within ~0.6us of its
            # first matmul, so the full block goes first; then x in pair
            # transfers. Batch 1 gets its own tags (x1*) so its DMAs carry no
            # WAR on batch-0 slots — a WAR head-of-line-blocks the queue and
            # lets p2t cut ahead on the shared bus.
            nc.sync.dma_start(
                w01hi_sb[:], w01hi[:].rearrange("c p f -> p c f"))
            post_x(0, 0, 2, "xhi", "xlo")
            nc.gpsimd.dma_start(
                w01lo_sb[:], w01lo[:].rearrange("c p f -> p c f"))

            def emit_small_consts():
                nc.sync.dma_start(w23hi_sb[:], w23hi[:].rearrange("c p f -> p c f"))
                nc.sync.dma_start(w23lo_sb[:], w23lo[:].rearrange("c p f -> p c f"))
                nc.sync.dma_start(w45_sb[:], w45p[:].rearrange("c p f -> p c f"))
                nc.sync.dma_start(exd_sb[:], exdp[:])
                nc.sync.dma_start(biasA_sb[:], biasA[:])
                nc.sync.dma_start(biasB_sb[:], biasB[:])
                nc.sync.dma_start(biasC_sb[:], biasC[:])
                nc.sync.dma_start(bdp_sb[:], bdp[:])

            # p2t quad-row DMAs: hi on the HWDGE (sync) path, lo via the
            # otherwise-idle Pool engine's SWDGE; 4 src tiles per transfer to
            # amortize the ~1us per-DMA DGE cost.
            p2_rows = [(4 * i, 0) for i in range(4)] + [(4 * i, 1) for i in range(4)]
            p2_pos = [0]

            def emit_p2(n):
                for _ in range(n):
                    if p2_pos[0] >= len(p2_rows):
                        return
                    si, hl = p2_rows[p2_pos[0]]
                    p2_pos[0] += 1
                    if hl == 0:
                        nc.sync.dma_start(
                            p2thi_sb[:, si : si + 4, :],
                            p2thi[si : si + 4].rearrange("s p n -> p s n"))
                    else:
                        nc.gpsimd.dma_start(
                            p2tlo_sb[:, si : si + 4, :],
                            p2tlo[si : si + 4].rearrange("s p n -> p s n"))

            wd_pos = [0]
            wd_direct = [0]
            N_WD_DIRECT_GROUPS = 1

            def borrow_wd(pool, tag, n):
                """Land n decoder k-tiles in one group DMA into a dead tile's
                slot (tag must never be allocated again afterwards)."""
                k0 = wd_pos[0]
                assert k0 + n <= KDEC
                wd_pos[0] += n
                gt = pool.tile([128, n, DEC_SH], F16, tag=tag)
                nc.gpsimd.dma_start(
                    gt[:], wdp[k0 : k0 + n].rearrange("k p f -> p k f"))
                for i in range(n):
                    wd_tiles[k0 + i] = gt[:, i, :]

            def emit_wd(n):
                """Stream decoder-weight k-tiles into the dedicated pool,
                2 per transfer."""
                for _ in range(n):
                    if wd_direct[0] >= N_WD_DIRECT_GROUPS or wd_pos[0] + 2 > KDEC:
                        return
                    wd_direct[0] += 1
                    borrow_wd(wdpool, "wd", 2)

            def l0_stage(b, t_hi, t_lo):
                # ---- L0: t = x @ W01 (3-term fp8 DR) ----
                # x streams in node-tile groups: hi via HWDGE, lo via SWDGE.
                # Groups 0/1 land as singles (startup latency), the rest as
                # pair transfers; batch 1's stream is fully posted during
                # batch 0's loop.
                for g in range(NT // 2):
                    (ghi, glo), gi = prefetched.pop((b, g))
                    for k in range(2):
                        nt = 2 * g + k
                        pt = ps.tile([128, 512], F32, tag="ps")
                        first = True
                        for p in range(CT // 2):
                            nc.tensor.matmul(
                                pt[:], ghi[:, 2 * gi + k, 2 * p : 2 * p + 2, :],
                                w01hi_sb[:, 2 * p : 2 * p + 2, :],
                                start=first, stop=False, perf_mode=DRM,
                            )
                            first = False
                        for p in range(CT // 2):
                            nc.tensor.matmul(
                                pt[:], glo[:, 2 * gi + k, 2 * p : 2 * p + 2, :],
                                w01hi_sb[:, 2 * p : 2 * p + 2, :],
                                start=False, stop=False, perf_mode=DRM,
                            )
                        for p in range(CT // 2):
                            nc.tensor.matmul(
                                pt[:], ghi[:, 2 * gi + k, 2 * p : 2 * p + 2, :],
                                w01lo_sb[:, 2 * p : 2 * p + 2, :],
                                start=False, stop=(p == CT // 2 - 1),
                                perf_mode=DRM,
                            )
                        nc.scalar.activation(t_hi[:, nt, :], pt[:], ACTF.Copy,
                                             scale=KT)
                        nc.vector.scalar_tensor_tensor(
                            t_lo[:, nt, :], pt[:], KT, t_hi[:, nt, :],
                            ALU.mult, ALU.subtract,
                        )
                    if b == 0:
                        if g == 0:
                            post_x(0, 2, 2, "xhi", "xlo")
                        elif g == 1:
                            post_x(0, 4, 2, "xhi", "xlo")
                        elif g == 2:
                            post_x(0, 6, 2, "xhi", "xlo")
                        elif g == 3:
                            emit_small_consts()
                        elif g == 5:
                            post_x(1, 0, 2, "x1hi", "x1lo")
                        elif g == 6:
                            post_x(1, 2, 2, "x1hi", "x1lo")
                        elif g == 7:
                            post_x(1, 4, 2, "x1hi", "x1lo")
                            post_x(1, 6, 2, "x1hi", "x1lo")

            def prop_stage(b, src_hi, src_lo, bias_sb, nf, out_cb, wd_budget):
                # ---- z = P2 @ src + bias (single hi-bias DR matmul) ----
                # nf: number of 128-wide feature tiles in src (4 for stage A,
                # 2 for stage B). out_cb(dc, fj, pt) evicts the PSUM tile.
                for dc in range(4):
                    dsl = slice(dc * 512, (dc + 1) * 512)
                    for fj in range(nf):
                        fsl = slice(fj * 128, (fj + 1) * 128)
                        pt = ps.tile([128, 512], F32, tag="ps")
                        for sp in range(NT // 2):
                            ssl = slice(2 * sp, 2 * sp + 2)
                            nc.tensor.matmul(
                                pt[:], src_hi[:, ssl, fsl],
                                p2thi_sb[:, ssl, dsl],
                                start=(sp == 0), stop=False, perf_mode=DRM,
                            )
                        for sp in range(NT // 2):
                            ssl = slice(2 * sp, 2 * sp + 2)
                            nc.tensor.matmul(
                                pt[:], src_lo[:, ssl, fsl],
                                p2thi_sb[:, ssl, dsl],
                                start=False, stop=False, perf_mode=DRM,
                            )
                        for sp in range(NT // 2):
                            ssl = slice(2 * sp, 2 * sp + 2)
                            nc.tensor.matmul(
                                pt[:], src_hi[:, ssl, fsl],
                                p2tlo_sb[:, ssl, dsl],
                                start=False, stop=False, perf_mode=DRM,
                            )
                        # bias: all four hi/lo row-pairs packed into one
                        # K=4x2 DoubleRow matmul (exact same sum as the old
                        # two-instruction form)
                        nc.tensor.matmul(
                            pt[:], bias_sb[:, :, fsl], exd_sb[:, :, dsl],
                            start=False, stop=True, perf_mode=DRM,
                        )
                        out_cb(dc, fj, pt)
                        emit_wd(wd_budget)

            def stageA(b, t_hi, t_lo, h1hi, h1lo):
                def evict(dc, fj, pt):
                    dsl = slice(dc * 512, (dc + 1) * 512)
                    rl = vpool.tile([128, 512], F16, tag="rl")
                    nc.scalar.activation(rl[:], pt[:], ACTF.Relu,
                                         scale=0.99 * K1)
                    v = vpool.tile([128, 512], F16, tag="v")
                    nc.vector.scalar_tensor_tensor(
                        v[:], pt[:], ALPHA * K1, rl[:], ALU.mult, ALU.add)
                    nc.scalar.activation(h1hi[:, fj, dsl], v[:], ACTF.Copy)
                    nc.vector.tensor_tensor(
                        h1lo[:, fj, dsl], v[:], h1hi[:, fj, dsl],
                        ALU.subtract)

                prop_stage(b, t_hi, t_lo, biasA_sb, 4, evict,
                           1 if b == 0 else 0)

            def w23_stage(h1hi, h1lo, t2hi, t2lo):
                # ---- W23: t2 = h1 @ W23 ----
                for nt in range(NT):
                    nsl = slice(nt * 128, (nt + 1) * 128)
                    pt = ps.tile([128, 512], F32, tag="ps")
                    for fp in range(2):
                        fsl = slice(2 * fp, 2 * fp + 2)
                        nc.tensor.matmul(
                            pt[:, 0:256], h1hi[:, fsl, nsl], w23hi_sb[:, fsl, :],
                            start=(fp == 0), stop=False, perf_mode=DRM,
                        )
                    for fp in range(2):
                        fsl = slice(2 * fp, 2 * fp + 2)
                        nc.tensor.matmul(
                            pt[:, 0:256], h1lo[:, fsl, nsl], w23hi_sb[:, fsl, :],
                            start=False, stop=False, perf_mode=DRM,
                        )
                    for fp in range(2):
                        fsl = slice(2 * fp, 2 * fp + 2)
                        nc.tensor.matmul(
                            pt[:, 0:256], h1hi[:, fsl, nsl], w23lo_sb[:, fsl, :],
                            start=False, stop=(fp == 1), perf_mode=DRM,
                        )
                    nc.scalar.activation(t2hi[:, nt, :], pt[:, 0:256], ACTF.Copy,
                                         scale=K2)
                    nc.vector.scalar_tensor_tensor(
                        t2lo[:, nt, :], pt[:, 0:256], K2, t2hi[:, nt, :],
                        ALU.mult, ALU.subtract,
                    )

            def stageB(b, t2hi, t2lo, h3):
                def evict(dc, fj, pt):
                    dsl = slice(dc * 512, (dc + 1) * 512)
                    rl = vpool.tile([128, 512], F16, tag="rl")
                    nc.scalar.activation(rl[:], pt[:], ACTF.Relu, scale=0.99)
                    nc.vector.scalar_tensor_tensor(
                        h3[:, fj, dsl], pt[:], ALPHA, rl[:],
                        ALU.mult, ALU.add)

                prop_stage(b, t2hi, t2lo, biasB_sb, 2, evict, 0)

            def tail_stages(b, h3):
                # ---- W45: t3 = h3 @ W45 (fp16) ----
                # psd bank: no wait on stage-B eviction chains draining ps
                pt45 = psd.tile([128, 512], F32, tag="psd")
                for nt in range(NT):
                    nsl = slice(nt * 128, (nt + 1) * 128)
                    for cj in range(2):
                        nc.tensor.matmul(
                            pt45[:, nt * 3 : nt * 3 + 3],
                            h3[:, cj, nsl], w45_sb[:, cj, :],
                            start=(cj == 0), stop=(cj == 1),
                        )
                nc.scalar.activation(t3hi[:], pt45[:, 0 : NT * 3], ACTF.Copy,
                                     scale=K3)
                nc.vector.scalar_tensor_tensor(
                    t3lo[:], pt45[:, 0 : NT * 3], K3, t3hi[:],
                    ALU.mult, ALU.subtract,
                )

                # ---- stage C: z5 = P2 @ t3 + bias ; h5 = leaky(z5) ----
                psC = psd.tile([128, 512], F32, tag="psd")
                for di in range(NT):
                    dsl = slice(di * 128, (di + 1) * 128)
                    osl = slice(di * 3, di * 3 + 3)
                    for sp in range(NT // 2):
                        t3sl = t3hi[:, 6 * sp : 6 * sp + 6].rearrange(
                            "p (s c) -> p s c", s=2, c=3)
                        nc.tensor.matmul(
                            psC[:, osl], p2thi_sb[:, 2 * sp : 2 * sp + 2, dsl],
                            t3sl, start=(sp == 0), stop=False, perf_mode=DRM,
                        )
                    for sp in range(NT // 2):
                        t3sl = t3lo[:, 6 * sp : 6 * sp + 6].rearrange(
                            "p (s c) -> p s c", s=2, c=3)
                        nc.tensor.matmul(
                            psC[:, osl], p2thi_sb[:, 2 * sp : 2 * sp + 2, dsl],
                            t3sl, start=False, stop=False, perf_mode=DRM,
                        )
                    for sp in range(NT // 2):
                        t3sl = t3hi[:, 6 * sp : 6 * sp + 6].rearrange(
                            "p (s c) -> p s c", s=2, c=3)
                        nc.tensor.matmul(
                            psC[:, osl], p2tlo_sb[:, 2 * sp : 2 * sp + 2, dsl],
                            t3sl, start=False, stop=False, perf_mode=DRM,
                        )
                    nc.tensor.matmul(
                        psC[:, osl], exd_sb[:, :, dsl], biasC_sb[:],
                        start=False, stop=True, perf_mode=DRM,
                    )
                nc.scalar.activation(rl5[:], psC[:, 0 : NT * 3], ACTF.Relu,
                                     scale=0.99)
                nc.vector.scalar_tensor_tensor(
                    h5_t[b][:].rearrange("p c d -> p d c"),
                    psC[:, 0 : NT * 3].rearrange("p (d c) -> p d c", d=NT, c=3),
                    ALPHA,
                    rl5[:].rearrange("p (d c) -> p d c", d=NT, c=3),
                    ALU.mult, ALU.add,
                )
                nc.sync.dma_start(
                    cc_in[b][:].rearrange("c p n -> p c n"), h5_t[b][:])
                nc.gpsimd.collective_compute(
                    "AllGather",
                    ALU.bypass,
                    replica_groups=[list(range(N_CORES))],
                    ins=[cc_in[b][:]],
                    outs=[cc_out[b][:]],
                )

            def featT_dma(half):
                for c in range(3):
                    nc.sync.dma_start(
                        featT_sb[:, half : B : BL, c * NT : (c + 1) * NT],
                        cc_out[half][:, c].rearrange("core p n -> p core n"),
                    )

            def decoder_pass(half):
                pd = psd.tile([128, 512], F32, tag="psd")
                for kt in range(KDEC):
                    rhs = featT_sb[:, half : B : BL, kt : kt + 1]
                    for ct in range(6):
                        nc.tensor.matmul(
                            pd[:, ct * 8 : (ct + 1) * 8],
                            wd_tiles[kt][:, ct * 128 : (ct + 1) * 128],
                            rhs, start=(kt == 0), stop=(kt == KDEC - 1),
                        )
                for ct in range(6):
                    nc.scalar.activation(
                        y_sb[:, ct, half : B : BL], pd[:, ct * 8 : (ct + 1) * 8],
                        ACTF.Tanh, bias=bdp_sb[:, ct : ct + 1], scale=KD,
                    )

            # ---- emission schedule ----
            t_hi0 = tpool.tile([128, NT, 512], F8, tag="t_hi0")
            t_lo0 = tpool.tile([128, NT, 512], F8, tag="t_lo0")
            t_hi1 = tpool.tile([128, NT, 512], F8, tag="t_hi1")
            t_lo1 = tpool.tile([128, NT, 512], F8, tag="t_lo1")

            l0_stage(0, t_hi0, t_lo0)
            # All x for both batches is now posted on the sync/pool queues;
            # p2t goes behind it in the same queues, so the shared DMA bus
            # serves x strictly first (queues drain in program order and run
            # far ahead of the PE — emission points alone don't pace DMA).
            emit_p2(8)
            l0_stage(1, t_hi1, t_lo1)
            # x tiles are dead from here: 24 decoder k-tiles into their slots
            for tag in ("xhi", "xlo", "x1hi"):
                borrow_wd(xpool, tag, 4)
                borrow_wd(xpool, tag, 4)
            borrow_wd(xpool, "x1lo", 2)

            h1hi = h1pool.tile([128, 4, N], F8, tag="h1hi")
            h1lo = h1pool.tile([128, 4, N], F8, tag="h1lo")
            stageA(0, t_hi0, t_lo0, h1hi, h1lo)
            emit_wd(N_WD_DIRECT_GROUPS)  # any remainder
            borrow_wd(tpool, "t_hi0", 5)
            borrow_wd(tpool, "t_lo0", 5)

            t2hi = t2pool.tile([128, NT, 256], F8, tag="t2hi")
            t2lo = t2pool.tile([128, NT, 256], F8, tag="t2lo")
            w23_stage(h1hi, h1lo, t2hi, t2lo)
            h3 = h3pool.tile([128, 2, N], F16, tag="h3")
            stageB(0, t2hi, t2lo, h3)
            tail_stages(0, h3)
            featT_dma(0)

            h1hi = h1pool.tile([128, 4, N], F8, tag="h1hi")
            h1lo = h1pool.tile([128, 4, N], F8, tag="h1lo")
            stageA(1, t_hi1, t_lo1, h1hi, h1lo)
            borrow_wd(tpool, "t_hi1", 5)
            borrow_wd(tpool, "t_lo1", 5)
            assert wd_pos[0] == KDEC, wd_pos[0]

            t2hi = t2pool.tile([128, NT, 256], F8, tag="t2hi")
            t2lo = t2pool.tile([128, NT, 256], F8, tag="t2lo")
            w23_stage(h1hi, h1lo, t2hi, t2lo)
            h3 = h3pool.tile([128, 2, N], F16, tag="h3")
            stageB(1, t2hi, t2lo, h3)
            tail_stages(1, h3)
            decoder_pass(0)  # fills part of the cc1 wait
            featT_dma(1)
            decoder_pass(1)
            nc.sync.dma_start(y_out[:], y_sb[:])

    _split_multi_waits(nc)
    return nc


# ---------------------------------------------------------------------------
def _split8(a):
    hi = np.asarray(a, np.float32).astype(NPF8)
    lo = (np.asarray(a, np.float64) - hi.astype(np.float64)).astype(
        np.float32).astype(NPF8)
    return hi, lo


def _host_precompute(x, edges, Ws, bs, Wd, bd_np):
    edges = np.asarray(edges)
    src = edges[0].astype(np.int64)
    dst = edges[1].astype(np.int64)

    deg = np.bincount(dst, minlength=N).astype(np.float64) + 1.0
    isd = 1.0 / np.sqrt(deg)
    idg = 1.0 / deg

    P = np.zeros((N, N), np.float64)
    np.add.at(P, (dst, src), isd[src] * isd[dst])
    P[np.arange(N), np.arange(N)] += idg
    P2 = P @ P
    r = P.sum(axis=1)

    W0, W1, W2, W3, W4, W5 = [np.asarray(w, np.float64) for w in Ws]
    b0, b1, b2, b3, b4, b5 = [np.asarray(b, np.float64) for b in bs]
    W01 = W0 @ W1
    W23 = W2 @ W3
    W45 = W4 @ W5
    a1 = b0 @ W1
    a3 = b2 @ W3
    a5 = b4 @ W5

    # p2t[si, p, d] = P2[d, si*128+p] * SP2
    p2t_s = np.ascontiguousarray((P2.T * SP2).reshape(NT, 128, N))
    p2thi, p2tlo = _split8(p2t_s)

    w01_pad = np.zeros((C_PAD, 512), np.float64)
    w01_pad[:C_IN] = W01
    w01hi, w01lo = _split8((w01_pad * SW01).reshape(CT, 128, 512))
    w23hi, w23lo = _split8((W23 * SW23).reshape(4, 128, 256))
    w45_np = W45.reshape(2, 128, 3).astype(NPF16)

    # Packed bias operands: one K=4x2 DoubleRow matmul computes
    #   (a_hi+a_lo)(r_hi+r_lo) + (b_hi+b_lo)*SEXD
    # exactly as the old two-matmul form.
    rhi, rlo = _split8(r * SEXD)
    one8 = np.full(N, SEXD, np.float32).astype(NPF8)
    exdp_np = np.zeros((4, 2, N), NPF8)
    exdp_np[0] = np.stack([rhi, rlo])
    exdp_np[1] = np.stack([rhi, rlo])
    exdp_np[2] = np.stack([one8, one8])
    exdp = np.ascontiguousarray(exdp_np)

    def bias4(a, bvec, s):
        ahi, alo = _split8(np.asarray(a) * s)
        bhi, blo = _split8(np.asarray(bvec) * s)
        out = np.zeros((4, 2, len(ahi)), NPF8)
        out[0] = np.stack([ahi, ahi])
        out[1] = np.stack([alo, alo])
        out[2] = np.stack([bhi, blo])
        return np.ascontiguousarray(out)

    biasA_np = bias4(a1, b1, SBA)
    biasB_np = bias4(a3, b3, SBB)
    biasC_np = bias4(a5, b5, SBC)

    # x: pad channels, scale, split; layout [BL,NT,p=chan,CT,j=node]
    x_np = np.asarray(x, np.float32)
    x_pad = np.zeros((B, N, C_PAD), np.float32)
    x_pad[:, :, :C_IN] = x_np * SX
    xt_all = np.ascontiguousarray(
        x_pad.reshape(B, NT, 128, CT, 128).transpose(0, 1, 4, 3, 2))
    xhi_all, xlo_all = _split8(xt_all)

    # decoder: permuted rows j' = c*2048 + node
    Wd_np = np.asarray(Wd, np.float64)
    node = np.arange(N)
    rows = np.concatenate([node * 3 + c for c in range(3)])  # j' -> orig j
    Wd_perm = Wd_np[rows]  # [6144, 6144] in j' order
    bd_full = np.asarray(bd_np, np.float64)

    shared = {
        "p2thi": p2thi, "p2tlo": p2tlo,
        "w01hi": w01hi, "w01lo": w01lo,
        "w23hi": w23hi, "w23lo": w23lo,
        "w45p": w45_np,
        "exdp": exdp,
        "biasA": biasA_np, "biasB": biasB_np, "biasC": biasC_np,
    }
    in_maps = []
    for c in range(N_CORES):
        wd_c = np.ascontiguousarray(
            Wd_perm[:, c * DEC_SH : (c + 1) * DEC_SH]
            .reshape(KDEC, 128, DEC_SH).astype(NPF16))
        bd_c = np.ascontiguousarray(
            bd_full[c * DEC_SH : (c + 1) * DEC_SH]
            .reshape(6, 128).T.astype(np.float32))  # [128, 6]
        m = dict(shared)
        m["xhi"] = xhi_all[c * BL : (c + 1) * BL]
        m["xlo"] = xlo_all[c * BL : (c + 1) * BL]
        m["wdp"] = wd_c
        m["bdp"] = bd_c
        in_maps.append(m)
    return in_maps


_NC_CACHE = {}


def kernel(**inputs) -> np.ndarray:
    x = inputs["x"]
    edges = inputs["edges"]
    Ws = [inputs[f"W{i}"] for i in range(6)]
    bs = [inputs[f"b{i}"] for i in range(6)]
    Wd = inputs["Wd"]
    bd_np = inputs["bd"]

    in_maps = _host_precompute(x, edges, Ws, bs, Wd, bd_np)

    if "nc" not in _NC_CACHE:
        _NC_CACHE["nc"] = _build_program()
    nc = _NC_CACHE["nc"]

    res = run_bass_kernel_spmd(nc, in_maps, list(range(N_CORES)))

    out = np.zeros((B, D_DEC), np.float32)
    for c in range(N_CORES):
        y = res.results[c]["y"]  # [128, 6, 16]
        out[:, c * DEC_SH : (c + 1) * DEC_SH] = (
            0.1 * y.transpose(2, 1, 0).reshape(B, DEC_SH))
    return out.reshape(B, N, 3)



# revision 45
# speedup vs baseline: 1.0960x; 1.0075x over previous
"""Trainium2 Bass kernel for nn_DeformGCN (6-layer GCN + dense decoder).

Strategy (v3, fp8 DoubleRow):
  - Host precompute from `edges`: dense propagation matrix P (N x N) with
    P[dst,src] += 1/sqrt(deg_s*deg_d) and P[n,n] += 1/deg_n, then P2 = P @ P.
    GCN layer pairs fuse into 3 propagation stages (A, B, C):
      z = P2 @ (h @ (Wa@Wb)) + r (x) (ba@Wb) + 1 (x) bb,  r = P @ 1
    followed by LeakyReLU(0.01).
  - All heavy matmuls run as fp8e4m3 DoubleRow (2 x 128-deep products per
    instruction at 0.5 cycles/row = 4x bf16 MAC rate). Accuracy is restored
    with a hi/lo residual split of both operands; the lo*lo term is dropped
    (3-term scheme, 0.75x the bf16 row count). End-to-end rel err ~2e-3.
  - Activations are split on the fly during PSUM eviction:
      hi = ACT copy(psum, scale=k) -> fp8 ; lo = DVE (psum*k - hi) -> fp8
    LeakyReLU uses  v = 0.99k*relu(z) (ACT)  +  0.01k*z (DVE stt add).
  - Small stages (W45, decoder) run in fp16 (1.0 cycles/row, 10-bit mantissa).
  - Decoder is column-sharded (768 cols/core) and oriented [col_part x batch]
    so its cost is 48*6*16 rows. Features are AllGather'd per local batch (2
    collectives; the first fully overlaps with batch 1's GCN compute, and
    decoder pass 0 fills part of the second collective's wait).
  - Biases ride the propagation matmuls as ONE K=4x2 DoubleRow matmul per
    PSUM group: rows (a*r)hi/lo and (b*1)hi/lo packed on 4 partitions --
    exactly the old two-instruction sum at half the PE cost. Both hi and lo
    bias halves are required: layer-5's bias scale (1/sqrt(64)) makes an
    fp8-hi-only bias error ~5e-2 end-to-end.
  - v3 schedule: both batches' L0 run back-to-back up front (the front is
    DMA-bus-bound: w01 + x + p2t = 22.6 MB must land before stage A starts;
    batch 1's L0 fills the otherwise-idle PE). DMAs are batched into
    multi-tile transfers (DGE fixed costs ~0.6-1 us each dominate small
    ones); each queue's program order IS the bus priority since queues run
    far ahead of the PE. Batch 1's x stream uses its own SBUF tags so slot
    WARs never head-of-line-block the queue behind them. W45/stage-C PSUM
    comes from the decoder's pool so it never waits on stage-B evictions.
"""

import os
import numpy as np
import ml_dtypes

_STAGE_LIMIT = int(os.environ.get("KSTAGE", "99"))

import bass_rust
import concourse.bass as bass
import concourse.mybir as mybir
import concourse.tile as tile
from concourse.tile import ScopedClock
from concourse.bass_utils import run_bass_kernel_spmd

F8 = mybir.dt.float8e4
F16 = mybir.dt.float16
F32 = mybir.dt.float32
NPF8 = ml_dtypes.float8_e4m3
NPF16 = np.float16
DRM = mybir.MatmulPerfMode.DoubleRow
ALU = mybir.AluOpType
ACTF = mybir.ActivationFunctionType

N_CORES = 8
B = 16
N = 2048
C_IN = 1475
C_PAD = 1536           # 12 * 128
NT = N // 128          # 16 node tiles
CT = C_PAD // 128      # 12 channel tiles
BL = B // N_CORES      # 2 local batches
D_DEC = N * 3          # 6144
DEC_SH = D_DEC // N_CORES  # 768 decoder columns per core
KDEC = D_DEC // 128    # 48 decoder k tiles
ALPHA = 0.01

# scales (powers of two; products of the two operand scales give the PSUM
# scale, the ACT evict rescales to the next storage scale)
SX = 16.0
SW01 = 2048.0
SCT = 32.0
SP2 = 64.0
SCH1 = 128.0
SW23 = 512.0
SCT2 = 512.0
SCT3 = 2048.0
SEXD = 64.0
SBA = SCT * SP2 / SEXD     # 32
SBB = SCT2 * SP2 / SEXD    # 512
SBC = SCT3 * SP2 / SEXD    # 2048
KT = SCT / (SX * SW01)     # 2^-10
K1 = SCH1 / (SCT * SP2)    # 2^-4
K2 = SCT2 / (SCH1 * SW23)  # 2^-7
K3 = SCT3 / (SCT2 * SP2)   # 2^-4
KD = 1.0 / (SCT3 * SP2)    # 2^-17


# ---------------------------------------------------------------------------
# Workaround: this walrus build caps sync-waits per control instruction very
# low, so TileContext's tail drain (which waits on every proc's semaphore)
# fails codegen. Split the global-clock waits into one single-wait
# EventSemaphore each, then emit a bare Drain.
def _patched_drain_and_barrier(self, tick_clock, wait_clock):
    nc = self.nc
    num_to_handle = {h.num: h for h in self.sems.allocated().values()}
    probe = nc.sync.nop(nofuse=True)
    wait_clock.add_sem_waits(probe.ins, ScopedClock({None: tick_clock.global_clock}))
    waits = list(probe.ins.sync_info.on_wait)
    probe.ins.sync_info = bass_rust.SyncInfo(on_wait=[], on_update=[])
    engines = [nc.sync, nc.scalar, nc.vector, nc.tensor, nc.gpsimd]
    for i, w in enumerate(waits):
        h = num_to_handle.get(w.id)
        if h is None:
            raise RuntimeError(f"no sem handle for {w.id} ({w.ant_name})")
        engines[i % len(engines)].wait_ge(h, w.wait_value)
    nc.all_engine_barrier()
    nc.sync.drain()
    assert self.sems is not None
    popped = nc._tile_sem_poison_stack.pop()
    assert popped is self._sem_poison
    nc.clear_and_free_semaphores(list(self.sems.allocated().values()))
    nc.all_engine_barrier()


tile.TileContext._drain_and_barrier = _patched_drain_and_barrier


def _split_multi_waits(nc, max_waits=1):
    """This walrus build rejects instructions carrying more than one sync
    wait. Hoist extra waits into standalone EventSemaphore instructions
    placed immediately before the instruction on the same engine queue."""
    ctr = 0
    for fn in nc.m.functions:
        for bb in fn.blocks:
            insts = bb.instructions
            new = []
            changed = False
            for inst in insts:
                si = inst.sync_info
                waits = list(si.on_wait) if si is not None else []
                if len(waits) > max_waits:
                    changed = True
                    for w in waits[:-max_waits]:
                        ev = bass_rust.InstEventSemaphore(
                            name=f"splitw_{ctr}", ins=[], outs=[]
                        )
                        ctr += 1
                        ev.engine = inst.engine
                        ev.sync_info = bass_rust.SyncInfo(
                            on_wait=[w], on_update=[]
                        )
                        new.append(ev)
                    inst.sync_info = bass_rust.SyncInfo(
                        on_wait=waits[-max_waits:], on_update=list(si.on_update)
                    )
                new.append(inst)
            if changed:
                bb.instructions = new


# ---------------------------------------------------------------------------
# v3 schedule: L0(b0), L0(b1) run back-to-back up front (the front window is
# DMA-bandwidth-bound: w01 + x + p2t = 22.6 MB must land before stage A can
# start; batch 1's L0 gives the PE ~31 us of work that was otherwise idle
# wait). Then chain(b0) stages A..C + collective 0 overlap chain(b1); the
# decoder passes slot in where their inputs are ready (pass 0 fills part of
# the collective-1 wait). DMAs are batched into multi-tile transfers (DGE
# fixed costs ~0.6-1 us/DMA dominate small ones) and ordered by need within
# each queue — queues drain in program order far ahead of the PE, so program
# order per queue IS the bus priority. Decoder weights stream into one
# dedicated 4-tile slot plus group-tiles borrowed from x/t tiles as they die.
# NOTE: both bias DR matmuls (hi and lo) are required — layer-5's bias scale
# (1/sqrt(64)) makes the fp8-hi-only bias error ~5e-2 end-to-end.
def _build_program() -> bass.Bass:
    nc = bass.Bass()

    xhi = nc.declare_dram_parameter("xhi", [BL, NT, 128, CT, 128], F8, isOutput=False)
    xlo = nc.declare_dram_parameter("xlo", [BL, NT, 128, CT, 128], F8, isOutput=False)
    p2thi = nc.declare_dram_parameter("p2thi", [NT, 128, N], F8, isOutput=False)
    p2tlo = nc.declare_dram_parameter("p2tlo", [NT, 128, N], F8, isOutput=False)
    w01hi = nc.declare_dram_parameter("w01hi", [CT, 128, 512], F8, isOutput=False)
    w01lo = nc.declare_dram_parameter("w01lo", [CT, 128, 512], F8, isOutput=False)
    w23hi = nc.declare_dram_parameter("w23hi", [4, 128, 256], F8, isOutput=False)
    w23lo = nc.declare_dram_parameter("w23lo", [4, 128, 256], F8, isOutput=False)
    w45p = nc.declare_dram_parameter("w45p", [2, 128, 3], F16, isOutput=False)
    exdp = nc.declare_dram_parameter("exdp", [4, 2, N], F8, isOutput=False)
    biasA = nc.declare_dram_parameter("biasA", [4, 2, 512], F8, isOutput=False)
    biasB = nc.declare_dram_parameter("biasB", [4, 2, 256], F8, isOutput=False)
    biasC = nc.declare_dram_parameter("biasC", [4, 2, 3], F8, isOutput=False)
    wdp = nc.declare_dram_parameter("wdp", [KDEC, 128, DEC_SH], F16, isOutput=False)
    bdp = nc.declare_dram_parameter("bdp", [128, 6], F32, isOutput=False)
    y_out = nc.declare_dram_parameter("y", [128, 6, B], F32, isOutput=True)

    cc_in = [nc.dram_tensor(f"cc_in{b}", [3, 128, NT], F16) for b in range(BL)]
    cc_out = [
        nc.dram_tensor(f"cc_out{b}", [N_CORES, 3, 128, NT], F16, addr_space="Shared")
        for b in range(BL)
    ]

    with tile.TileContext(nc) as tc:
        with (
            tc.tile_pool(name="const", bufs=1) as constp,
            tc.tile_pool(name="xp", bufs=2) as xpool,
            tc.tile_pool(name="tp", bufs=1) as tpool,
            tc.tile_pool(name="h1p", bufs=1) as h1pool,
            tc.tile_pool(name="t2p", bufs=1) as t2pool,
            tc.tile_pool(name="h3p", bufs=1) as h3pool,
            tc.tile_pool(name="vp", bufs=2) as vpool,
            tc.tile_pool(name="wdpool", bufs=1) as wdpool,
            tc.tile_pool(name="ps", bufs=5, space="PSUM") as ps,
            tc.tile_pool(name="psd", bufs=2, space="PSUM") as psd,
        ):
            # ---- const tiles ----
            w01hi_sb = constp.tile([128, CT, 512], F8, tag="w01hi")
            w01lo_sb = constp.tile([128, CT, 512], F8, tag="w01lo")
            p2thi_sb = constp.tile([128, NT, N], F8, tag="p2thi")
            p2tlo_sb = constp.tile([128, NT, N], F8, tag="p2tlo")
            w23hi_sb = constp.tile([128, 4, 256], F8, tag="w23hi")
            w23lo_sb = constp.tile([128, 4, 256], F8, tag="w23lo")
            w45_sb = constp.tile([128, 2, 3], F16, tag="w45")
            exd_sb = constp.tile([4, 2, N], F8, tag="exd")
            biasA_sb = constp.tile([4, 2, 512], F8, tag="biasA")
            biasB_sb = constp.tile([4, 2, 256], F8, tag="biasB")
            biasC_sb = constp.tile([4, 2, 3], F8, tag="biasC")
            bdp_sb = constp.tile([128, 6], F32, tag="bdp")
            featT_sb = constp.tile([128, B, KDEC], F16, tag="featT")
            y_sb = constp.tile([128, 6, B], F32, tag="ysb")
            t3hi = constp.tile([128, NT * 3], F8, tag="t3hi")
            t3lo = constp.tile([128, NT * 3], F8, tag="t3lo")
            rl5 = constp.tile([128, NT * 3], F16, tag="rl5")
            h5_0 = constp.tile([128, 3, NT], F16, tag="h5_0")
            h5_1 = constp.tile([128, 3, NT], F16, tag="h5_1")
            h5_t = [h5_0, h5_1]

            wd_tiles: list = [None] * KDEC

            prefetched = {}

            def post_x(b, g0, ng, tag_hi, tag_lo):
                """DMA x node-tile groups [2*g0, 2*(g0+ng)) as ONE transfer
                per hi/lo half (DGE fixed costs dominate small DMAs). Batch 1
                rides the scalar/vector HWDGE queues with its own tags, so a
                WAR slot-reuse wait in one stream never head-of-line-blocks
                another stream's DMAs."""
                ghi = xpool.tile([128, 2 * ng, CT, 128], F8, tag=tag_hi)
                glo = xpool.tile([128, 2 * ng, CT, 128], F8, tag=tag_lo)
                nc.sync.dma_start(
                    ghi[:], xhi[b, 2 * g0 : 2 * g0 + 2 * ng].rearrange(
                        "n p c j -> p n c j"))
                nc.gpsimd.dma_start(
                    glo[:], xlo[b, 2 * g0 : 2 * g0 + 2 * ng].rearrange(
                        "n p c j -> p n c j"))
                for i in range(ng):
                    prefetched[(b, g0 + i)] = ((ghi, glo), i)

            # Startup: a 2-tile w01hi starter chunk and a single first x
            # group gate the PE's first matmul (~3.5us of transfers); the
            # rest of w01hi follows immediately. Batch 1 gets its own tags
            # (x1*) so its DMAs carry no WAR on batch-0 slots — a WAR
            # head-of-line-blocks the queue and lets p2t cut ahead on the
            # shared bus.
            nc.sync.dma_start(
                w01hi_sb[:, 0:2, :], w01hi[0:2].rearrange("c p f -> p c f"))
            post_x(0, 0, 1, "xhi", "xlo")
            nc.sync.dma_start(
                w01hi_sb[:, 2:CT, :], w01hi[2:CT].rearrange("c p f -> p c f"))
            nc.gpsimd.dma_start(
                w01lo_sb[:], w01lo[:].rearrange("c p f -> p c f"))
            post_x(0, 1, 1, "xhi", "xlo")

            def emit_small_consts():
                nc.sync.dma_start(w23hi_sb[:], w23hi[:].rearrange("c p f -> p c f"))
                nc.sync.dma_start(w23lo_sb[:], w23lo[:].rearrange("c p f -> p c f"))
                nc.sync.dma_start(w45_sb[:], w45p[:].rearrange("c p f -> p c f"))
                nc.sync.dma_start(exd_sb[:], exdp[:])
                nc.sync.dma_start(biasA_sb[:], biasA[:])
                nc.sync.dma_start(biasB_sb[:], biasB[:])
                nc.sync.dma_start(biasC_sb[:], biasC[:])
                nc.sync.dma_start(bdp_sb[:], bdp[:])

            # p2t quad-row DMAs: hi on the HWDGE (sync) path, lo via the
            # otherwise-idle Pool engine's SWDGE; 4 src tiles per transfer to
            # amortize the ~1us per-DMA DGE cost.
            p2_rows = [(4 * i, 0) for i in range(4)] + [(4 * i, 1) for i in range(4)]
            p2_pos = [0]

            def emit_p2(n):
                for _ in range(n):
                    if p2_pos[0] >= len(p2_rows):
                        return
                    si, hl = p2_rows[p2_pos[0]]
                    p2_pos[0] += 1
                    if hl == 0:
                        nc.sync.dma_start(
                            p2thi_sb[:, si : si + 4, :],
                            p2thi[si : si + 4].rearrange("s p n -> p s n"))
                    else:
                        nc.gpsimd.dma_start(
                            p2tlo_sb[:, si : si + 4, :],
                            p2tlo[si : si + 4].rearrange("s p n -> p s n"))

            wd_pos = [0]
            wd_direct = [0]
            N_WD_DIRECT_GROUPS = 1

            def borrow_wd(pool, tag, n):
                """Land n decoder k-tiles in one group DMA into a dead tile's
                slot (tag must never be allocated again afterwards)."""
                k0 = wd_pos[0]
                assert k0 + n <= KDEC
                wd_pos[0] += n
                gt = pool.tile([128, n, DEC_SH], F16, tag=tag)
                nc.gpsimd.dma_start(
                    gt[:], wdp[k0 : k0 + n].rearrange("k p f -> p k f"))
                for i in range(n):
                    wd_tiles[k0 + i] = gt[:, i, :]

            def emit_wd(n):
                """Stream decoder-weight k-tiles into the dedicated pool,
                2 per transfer."""
                for _ in range(n):
                    if wd_direct[0] >= N_WD_DIRECT_GROUPS or wd_pos[0] + 2 > KDEC:
                        return
                    wd_direct[0] += 1
                    borrow_wd(wdpool, "wd", 2)

            def l0_stage(b, t_hi, t_lo):
                # ---- L0: t = x @ W01 (3-term fp8 DR) ----
                # x streams in node-tile groups: hi via HWDGE, lo via SWDGE.
                # Groups 0/1 land as singles (startup latency), the rest as
                # pair transfers; batch 1's stream is fully posted during
                # batch 0's loop.
                for g in range(NT // 2):
                    (ghi, glo), gi = prefetched.pop((b, g))
                    for k in range(2):
                        nt = 2 * g + k
                        pt = ps.tile([128, 512], F32, tag="ps")
                        first = True
                        for p in range(CT // 2):
                            nc.tensor.matmul(
                                pt[:], ghi[:, 2 * gi + k, 2 * p : 2 * p + 2, :],
                                w01hi_sb[:, 2 * p : 2 * p + 2, :],
                                start=first, stop=False, perf_mode=DRM,
                            )
                            first = False
                        for p in range(CT // 2):
                            nc.tensor.matmul(
                                pt[:], glo[:, 2 * gi + k, 2 * p : 2 * p + 2, :],
                                w01hi_sb[:, 2 * p : 2 * p + 2, :],
                                start=False, stop=False, perf_mode=DRM,
                            )
                        for p in range(CT // 2):
                            nc.tensor.matmul(
                                pt[:], ghi[:, 2 * gi + k, 2 * p : 2 * p + 2, :],
                                w01lo_sb[:, 2 * p : 2 * p + 2, :],
                                start=False, stop=(p == CT // 2 - 1),
                                perf_mode=DRM,
                            )
                        nc.scalar.activation(t_hi[:, nt, :], pt[:], ACTF.Copy,
                                             scale=KT)
                        nc.vector.scalar_tensor_tensor(
                            t_lo[:, nt, :], pt[:], KT, t_hi[:, nt, :],
                            ALU.mult, ALU.subtract,
                        )
                    if b == 0:
                        if g == 0:
                            post_x(0, 2, 2, "xhi", "xlo")
                        elif g == 1:
                            post_x(0, 4, 2, "xhi", "xlo")
                            # batch-1 pairs 1-2 use fresh slots (no WAR):
                            # early in the queue, they can never block it
                            post_x(1, 0, 2, "x1hi", "x1lo")
                        elif g == 2:
                            post_x(0, 6, 2, "xhi", "xlo")
                        elif g == 3:
                            emit_small_consts()
                            post_x(1, 2, 2, "x1hi", "x1lo")

            def prop_stage(b, src_hi, src_lo, bias_sb, nf, out_cb, wd_budget):
                # ---- z = P2 @ src + bias (single hi-bias DR matmul) ----
                # nf: number of 128-wide feature tiles in src (4 for stage A,
                # 2 for stage B). out_cb(dc, fj, pt) evicts the PSUM tile.
                for dc in range(4):
                    dsl = slice(dc * 512, (dc + 1) * 512)
                    for fj in range(nf):
                        fsl = slice(fj * 128, (fj + 1) * 128)
                        pt = ps.tile([128, 512], F32, tag="ps")
                        for sp in range(NT // 2):
                            ssl = slice(2 * sp, 2 * sp + 2)
                            nc.tensor.matmul(
                                pt[:], src_hi[:, ssl, fsl],
                                p2thi_sb[:, ssl, dsl],
                                start=(sp == 0), stop=False, perf_mode=DRM,
                            )
                        for sp in range(NT // 2):
                            ssl = slice(2 * sp, 2 * sp + 2)
                            nc.tensor.matmul(
                                pt[:], src_lo[:, ssl, fsl],
                                p2thi_sb[:, ssl, dsl],
                                start=False, stop=False, perf_mode=DRM,
                            )
                        for sp in range(NT // 2):
                            ssl = slice(2 * sp, 2 * sp + 2)
                            nc.tensor.matmul(
                                pt[:], src_hi[:, ssl, fsl],
                                p2tlo_sb[:, ssl, dsl],
                                start=False, stop=False, perf_mode=DRM,
                            )
                        # bias: all four hi/lo row-pairs packed into one
                        # K=4x2 DoubleRow matmul (exact same sum as the old
                        # two-instruction form)
                        nc.tensor.matmul(
                            pt[:], bias_sb[:, :, fsl], exd_sb[:, :, dsl],
                            start=False, stop=True, perf_mode=DRM,
                        )
                        out_cb(dc, fj, pt)
                        emit_wd(wd_budget)

            def stageA(b, t_hi, t_lo, h1hi, h1lo):
                def evict(dc, fj, pt):
                    dsl = slice(dc * 512, (dc + 1) * 512)
                    v = vpool.tile([128, 512], F16, tag="v")
                    nc.scalar.activation(v[:], pt[:], ACTF.Lrelu,
                                         scale=K1, alpha=ALPHA)
                    nc.scalar.activation(h1hi[:, fj, dsl], v[:], ACTF.Copy)
                    nc.vector.tensor_tensor(
                        h1lo[:, fj, dsl], v[:], h1hi[:, fj, dsl],
                        ALU.subtract)

                prop_stage(b, t_hi, t_lo, biasA_sb, 4, evict,
                           1 if b == 0 else 0)

            def w23_stage(h1hi, h1lo, t2hi, t2lo):
                # ---- W23: t2 = h1 @ W23 ----
                for nt in range(NT):
                    nsl = slice(nt * 128, (nt + 1) * 128)
                    pt = ps.tile([128, 512], F32, tag="ps")
                    for fp in range(2):
                        fsl = slice(2 * fp, 2 * fp + 2)
                        nc.tensor.matmul(
                            pt[:, 0:256], h1hi[:, fsl, nsl], w23hi_sb[:, fsl, :],
                            start=(fp == 0), stop=False, perf_mode=DRM,
                        )
                    for fp in range(2):
                        fsl = slice(2 * fp, 2 * fp + 2)
                        nc.tensor.matmul(
                            pt[:, 0:256], h1lo[:, fsl, nsl], w23hi_sb[:, fsl, :],
                            start=False, stop=False, perf_mode=DRM,
                        )
                    for fp in range(2):
                        fsl = slice(2 * fp, 2 * fp + 2)
                        nc.tensor.matmul(
                            pt[:, 0:256], h1hi[:, fsl, nsl], w23lo_sb[:, fsl, :],
                            start=False, stop=(fp == 1), perf_mode=DRM,
                        )
                    nc.scalar.activation(t2hi[:, nt, :], pt[:, 0:256], ACTF.Copy,
                                         scale=K2)
                    nc.vector.scalar_tensor_tensor(
                        t2lo[:, nt, :], pt[:, 0:256], K2, t2hi[:, nt, :],
                        ALU.mult, ALU.subtract,
                    )

            def stageB(b, t2hi, t2lo, h3):
                def evict(dc, fj, pt):
                    dsl = slice(dc * 512, (dc + 1) * 512)
                    nc.scalar.activation(h3[:, fj, dsl], pt[:], ACTF.Lrelu,
                                         alpha=ALPHA)

                prop_stage(b, t2hi, t2lo, biasB_sb, 2, evict, 0)

            def tail_stages(b, h3):
                # ---- W45: t3 = h3 @ W45 (fp16) ----
                # psd bank: no wait on stage-B eviction chains draining ps
                pt45 = psd.tile([128, 512], F32, tag="psd")
                for nt in range(NT):
                    nsl = slice(nt * 128, (nt + 1) * 128)
                    for cj in range(2):
                        nc.tensor.matmul(
                            pt45[:, nt * 3 : nt * 3 + 3],
                            h3[:, cj, nsl], w45_sb[:, cj, :],
                            start=(cj == 0), stop=(cj == 1),
                        )
                nc.scalar.activation(t3hi[:], pt45[:, 0 : NT * 3], ACTF.Copy,
                                     scale=K3)
                nc.vector.scalar_tensor_tensor(
                    t3lo[:], pt45[:, 0 : NT * 3], K3, t3hi[:],
                    ALU.mult, ALU.subtract,
                )

                # ---- stage C: z5 = P2 @ t3 + bias ; h5 = leaky(z5) ----
                psC = psd.tile([128, 512], F32, tag="psd")
                for di in range(NT):
                    dsl = slice(di * 128, (di + 1) * 128)
                    osl = slice(di * 3, di * 3 + 3)
                    for sp in range(NT // 2):
                        t3sl = t3hi[:, 6 * sp : 6 * sp + 6].rearrange(
                            "p (s c) -> p s c", s=2, c=3)
                        nc.tensor.matmul(
                            psC[:, osl], p2thi_sb[:, 2 * sp : 2 * sp + 2, dsl],
                            t3sl, start=(sp == 0), stop=False, perf_mode=DRM,
                        )
                    for sp in range(NT // 2):
                        t3sl = t3lo[:, 6 * sp : 6 * sp + 6].rearrange(
                            "p (s c) -> p s c", s=2, c=3)
                        nc.tensor.matmul(
                            psC[:, osl], p2thi_sb[:, 2 * sp : 2 * sp + 2, dsl],
                            t3sl, start=False, stop=False, perf_mode=DRM,
                        )
                    for sp in range(NT // 2):
                        t3sl = t3hi[:, 6 * sp : 6 * sp + 6].rearrange(
                            "p (s c) -> p s c", s=2, c=3)
                        nc.tensor.matmul(
                            psC[:, osl], p2tlo_sb[:, 2 * sp : 2 * sp + 2, dsl],
                            t3sl, start=False, stop=False, perf_mode=DRM,
                        )
                    nc.tensor.matmul(
                        psC[:, osl], exd_sb[:, :, dsl], biasC_sb[:],
                        start=False, stop=True, perf_mode=DRM,
                    )
                nc.scalar.activation(
                    h5_t[b][:].rearrange("p c d -> p d c"),
                    psC[:, 0 : NT * 3].rearrange("p (d c) -> p d c", d=NT, c=3),
                    ACTF.Lrelu, alpha=ALPHA,
                )
                nc.sync.dma_start(
                    cc_in[b][:].rearrange("c p n -> p c n"), h5_t[b][:])
                nc.gpsimd.collective_compute(
                    "AllGather",
                    ALU.bypass,
                    replica_groups=[list(range(N_CORES))],
                    ins=[cc_in[b][:]],
                    outs=[cc_out[b][:]],
                )

            def featT_dma(half):
                for c in range(3):
                    nc.sync.dma_start(
                        featT_sb[:, half : B : BL, c * NT : (c + 1) * NT],
                        cc_out[half][:, c].rearrange("core p n -> p core n"),
                    )

            def decoder_pass(half):
                pd = psd.tile([128, 512], F32, tag="psd")
                for kt in range(KDEC):
                    rhs = featT_sb[:, half : B : BL, kt : kt + 1]
                    for ct in range(6):
                        nc.tensor.matmul(
                            pd[:, ct * 8 : (ct + 1) * 8],
                            wd_tiles[kt][:, ct * 128 : (ct + 1) * 128],
                            rhs, start=(kt == 0), stop=(kt == KDEC - 1),
                        )
                for ct in range(6):
                    nc.scalar.activation(
                        y_sb[:, ct, half : B : BL], pd[:, ct * 8 : (ct + 1) * 8],
                        ACTF.Tanh, bias=bdp_sb[:, ct : ct + 1], scale=KD,
                    )

            # ---- emission schedule ----
            t_hi0 = tpool.tile([128, NT, 512], F8, tag="t_hi0")
            t_lo0 = tpool.tile([128, NT, 512], F8, tag="t_lo0")
            t_hi1 = tpool.tile([128, NT, 512], F8, tag="t_hi1")
            t_lo1 = tpool.tile([128, NT, 512], F8, tag="t_lo1")

            l0_stage(0, t_hi0, t_lo0)
            # p2t streams behind batch-0 x and batch-1's first two pairs;
            # batch-1's WAR-gated pairs go after p2t in queue order (their
            # slot-reuse waits fire mid-L0(b1) and would head-of-line-block
            # p2t otherwise; arriving bus-interleaved with p2t is in time).
            emit_p2(8)
            post_x(1, 4, 2, "x1hi", "x1lo")
            post_x(1, 6, 2, "x1hi", "x1lo")
            l0_stage(1, t_hi1, t_lo1)
            # x tiles are dead from here: 24 decoder k-tiles into their slots
            for tag in ("xhi", "xlo", "x1hi"):
                borrow_wd(xpool, tag, 4)
                borrow_wd(xpool, tag, 4)
            borrow_wd(xpool, "x1lo", 2)

            h1hi = h1pool.tile([128, 4, N], F8, tag="h1hi")
            h1lo = h1pool.tile([128, 4, N], F8, tag="h1lo")
            stageA(0, t_hi0, t_lo0, h1hi, h1lo)
            emit_wd(N_WD_DIRECT_GROUPS)  # any remainder
            borrow_wd(tpool, "t_hi0", 5)
            borrow_wd(tpool, "t_lo0", 5)

            t2hi = t2pool.tile([128, NT, 256], F8, tag="t2hi")
            t2lo = t2pool.tile([128, NT, 256], F8, tag="t2lo")
            w23_stage(h1hi, h1lo, t2hi, t2lo)
            h3 = h3pool.tile([128, 2, N], F16, tag="h3")
            stageB(0, t2hi, t2lo, h3)
            tail_stages(0, h3)
            featT_dma(0)

            h1hi = h1pool.tile([128, 4, N], F8, tag="h1hi")
            h1lo = h1pool.tile([128, 4, N], F8, tag="h1lo")
            stageA(1, t_hi1, t_lo1, h1hi, h1lo)
            borrow_wd(tpool, "t_hi1", 5)
            borrow_wd(tpool, "t_lo1", 5)
            assert wd_pos[0] == KDEC, wd_pos[0]

            t2hi = t2pool.tile([128, NT, 256], F8, tag="t2hi")
            t2lo = t2pool.tile([128, NT, 256], F8, tag="t2lo")
            w23_stage(h1hi, h1lo, t2hi, t2lo)
            h3 = h3pool.tile([128, 2, N], F16, tag="h3")
            stageB(1, t2hi, t2lo, h3)
            tail_stages(1, h3)
            decoder_pass(0)  # fills part of the cc1 wait
            featT_dma(1)
            decoder_pass(1)
            nc.sync.dma_start(y_out[:], y_sb[:])

    _split_multi_waits(nc)
    return nc


# ---------------------------------------------------------------------------
def _split8(a):
    hi = np.asarray(a, np.float32).astype(NPF8)
    lo = (np.asarray(a, np.float64) - hi.astype(np.float64)).astype(
        np.float32).astype(NPF8)
    return hi, lo


def _host_precompute(x, edges, Ws, bs, Wd, bd_np):
    edges = np.asarray(edges)
    src = edges[0].astype(np.int64)
    dst = edges[1].astype(np.int64)

    deg = np.bincount(dst, minlength=N).astype(np.float64) + 1.0
    isd = 1.0 / np.sqrt(deg)
    idg = 1.0 / deg

    P = np.zeros((N, N), np.float64)
    np.add.at(P, (dst, src), isd[src] * isd[dst])
    P[np.arange(N), np.arange(N)] += idg
    P2 = P @ P
    r = P.sum(axis=1)

    W0, W1, W2, W3, W4, W5 = [np.asarray(w, np.float64) for w in Ws]
    b0, b1, b2, b3, b4, b5 = [np.asarray(b, np.float64) for b in bs]
    W01 = W0 @ W1
    W23 = W2 @ W3
    W45 = W4 @ W5
    a1 = b0 @ W1
    a3 = b2 @ W3
    a5 = b4 @ W5

    # p2t[si, p, d] = P2[d, si*128+p] * SP2
    p2t_s = np.ascontiguousarray((P2.T * SP2).reshape(NT, 128, N))
    p2thi, p2tlo = _split8(p2t_s)

    w01_pad = np.zeros((C_PAD, 512), np.float64)
    w01_pad[:C_IN] = W01
    w01hi, w01lo = _split8((w01_pad * SW01).reshape(CT, 128, 512))
    w23hi, w23lo = _split8((W23 * SW23).reshape(4, 128, 256))
    w45_np = W45.reshape(2, 128, 3).astype(NPF16)

    # Packed bias operands: one K=4x2 DoubleRow matmul computes
    #   (a_hi+a_lo)(r_hi+r_lo) + (b_hi+b_lo)*SEXD
    # exactly as the old two-matmul form.
    rhi, rlo = _split8(r * SEXD)
    one8 = np.full(N, SEXD, np.float32).astype(NPF8)
    exdp_np = np.zeros((4, 2, N), NPF8)
    exdp_np[0] = np.stack([rhi, rlo])
    exdp_np[1] = np.stack([rhi, rlo])
    exdp_np[2] = np.stack([one8, one8])
    exdp = np.ascontiguousarray(exdp_np)

    def bias4(a, bvec, s):
        ahi, alo = _split8(np.asarray(a) * s)
        bhi, blo = _split8(np.asarray(bvec) * s)
        out = np.zeros((4, 2, len(ahi)), NPF8)
        out[0] = np.stack([ahi, ahi])
        out[1] = np.stack([alo, alo])
        out[2] = np.stack([bhi, blo])
        return np.ascontiguousarray(out)

    biasA_np = bias4(a1, b1, SBA)
    biasB_np = bias4(a3, b3, SBB)
    biasC_np = bias4(a5, b5, SBC)

    # x: pad channels, scale, split; layout [BL,NT,p=chan,CT,j=node]
    x_np = np.asarray(x, np.float32)
    x_pad = np.zeros((B, N, C_PAD), np.float32)
    x_pad[:, :, :C_IN] = x_np * SX
    xt_all = np.ascontiguousarray(
        x_pad.reshape(B, NT, 128, CT, 128).transpose(0, 1, 4, 3, 2))
    xhi_all, xlo_all = _split8(xt_all)

    # decoder: permuted rows j' = c*2048 + node
    Wd_np = np.asarray(Wd, np.float64)
    node = np.arange(N)
    rows = np.concatenate([node * 3 + c for c in range(3)])  # j' -> orig j
    Wd_perm = Wd_np[rows]  # [6144, 6144] in j' order
    bd_full = np.asarray(bd_np, np.float64)

    shared = {
        "p2thi": p2thi, "p2tlo": p2tlo,
        "w01hi": w01hi, "w01lo": w01lo,
        "w23hi": w23hi, "w23lo": w23lo,
        "w45p": w45_np,
        "exdp": exdp,
        "biasA": biasA_np, "biasB": biasB_np, "biasC": biasC_np,
    }
    in_maps = []
    for c in range(N_CORES):
        wd_c = np.ascontiguousarray(
            Wd_perm[:, c * DEC_SH : (c + 1) * DEC_SH]
            .reshape(KDEC, 128, DEC_SH).astype(NPF16))
        bd_c = np.ascontiguousarray(
            bd_full[c * DEC_SH : (c + 1) * DEC_SH]
            .reshape(6, 128).T.astype(np.float32))  # [128, 6]
        m = dict(shared)
        m["xhi"] = xhi_all[c * BL : (c + 1) * BL]
        m["xlo"] = xlo_all[c * BL : (c + 1) * BL]
        m["wdp"] = wd_c
        m["bdp"] = bd_c
        in_maps.append(m)
    return in_maps


_NC_CACHE = {}


def kernel(**inputs) -> np.ndarray:
    x = inputs["x"]
    edges = inputs["edges"]
    Ws = [inputs[f"W{i}"] for i in range(6)]
    bs = [inputs[f"b{i}"] for i in range(6)]
    Wd = inputs["Wd"]
    bd_np = inputs["bd"]

    in_maps = _host_precompute(x, edges, Ws, bs, Wd, bd_np)

    if "nc" not in _NC_CACHE:
        _NC_CACHE["nc"] = _build_program()
    nc = _NC_CACHE["nc"]

    res = run_bass_kernel_spmd(nc, in_maps, list(range(N_CORES)))

    out = np.zeros((B, D_DEC), np.float32)
    for c in range(N_CORES):
        y = res.results[c]["y"]  # [128, 6, 16]
        out[:, c * DEC_SH : (c + 1) * DEC_SH] = (
            0.1 * y.transpose(2, 1, 0).reshape(B, DEC_SH))
    return out.reshape(B, N, 3)



# revision 48
# speedup vs baseline: 1.1043x; 1.0076x over previous
"""Trainium2 Bass kernel for nn_DeformGCN (6-layer GCN + dense decoder).

Strategy (v3, fp8 DoubleRow):
  - Host precompute from `edges`: dense propagation matrix P (N x N) with
    P[dst,src] += 1/sqrt(deg_s*deg_d) and P[n,n] += 1/deg_n, then P2 = P @ P.
    GCN layer pairs fuse into 3 propagation stages (A, B, C):
      z = P2 @ (h @ (Wa@Wb)) + r (x) (ba@Wb) + 1 (x) bb,  r = P @ 1
    followed by LeakyReLU(0.01).
  - All heavy matmuls run as fp8e4m3 DoubleRow (2 x 128-deep products per
    instruction at 0.5 cycles/row = 4x bf16 MAC rate). Accuracy is restored
    with a hi/lo residual split of both operands; the lo*lo term is dropped
    (3-term scheme, 0.75x the bf16 row count). End-to-end rel err ~2e-3.
  - Activations are split on the fly during PSUM eviction:
      hi = ACT copy(psum, scale=k) -> fp8 ; lo = DVE (psum*k - hi) -> fp8
    LeakyReLU is the ACT engine's native Lrelu (alpha=0.01) — one op.
  - Small stages (W45, decoder) run in fp16 (1.0 cycles/row, 10-bit mantissa).
  - Decoder is column-sharded (768 cols/core) and oriented [col_part x batch]
    so its cost is 48*6*16 rows. Features are AllGather'd per local batch (2
    collectives; the first fully overlaps with batch 1's GCN compute, and
    decoder pass 0 fills part of the second collective's wait).
  - Biases ride the propagation matmuls as ONE K=4x2 DoubleRow matmul per
    PSUM group: rows (a*r)hi/lo and (b*1)hi/lo packed on 4 partitions --
    exactly the old two-instruction sum at half the PE cost. Both hi and lo
    bias halves are required: layer-5's bias scale (1/sqrt(64)) makes an
    fp8-hi-only bias error ~5e-2 end-to-end.
  - v3 schedule: both batches' L0 run back-to-back up front (the front is
    DMA-bus-bound: w01 + x + p2t = 22.6 MB must land before stage A starts;
    batch 1's L0 fills the otherwise-idle PE). DMAs are batched into
    multi-tile transfers (DGE fixed costs ~0.6-1 us each dominate small
    ones); each queue's program order IS the bus priority since queues run
    far ahead of the PE. Batch 1's x stream uses its own SBUF tags so slot
    WARs never head-of-line-block the queue behind them. W45/stage-C PSUM
    comes from the decoder's pool so it never waits on stage-B evictions.
"""

import os
import numpy as np
import ml_dtypes

_STAGE_LIMIT = int(os.environ.get("KSTAGE", "99"))

import bass_rust
import concourse.bass as bass
import concourse.mybir as mybir
import concourse.tile as tile
from concourse.tile import ScopedClock
from concourse.bass_utils import run_bass_kernel_spmd

F8 = mybir.dt.float8e4
F16 = mybir.dt.float16
F32 = mybir.dt.float32
NPF8 = ml_dtypes.float8_e4m3
NPF16 = np.float16
DRM = mybir.MatmulPerfMode.DoubleRow
ALU = mybir.AluOpType
ACTF = mybir.ActivationFunctionType

N_CORES = 8
B = 16
N = 2048
C_IN = 1475
C_PAD = 1536           # 12 * 128
NT = N // 128          # 16 node tiles
CT = C_PAD // 128      # 12 channel tiles
BL = B // N_CORES      # 2 local batches
D_DEC = N * 3          # 6144
DEC_SH = D_DEC // N_CORES  # 768 decoder columns per core
KDEC = D_DEC // 128    # 48 decoder k tiles
ALPHA = 0.01

# scales (powers of two; products of the two operand scales give the PSUM
# scale, the ACT evict rescales to the next storage scale)
SX = 16.0
SW01 = 2048.0
SCT = 32.0
SP2 = 64.0
SCH1 = 128.0
SW23 = 512.0
SCT2 = 512.0
SCT3 = 2048.0
SEXD = 64.0
SBA = SCT * SP2 / SEXD     # 32
SBB = SCT2 * SP2 / SEXD    # 512
SBC = SCT3 * SP2 / SEXD    # 2048
KT = SCT / (SX * SW01)     # 2^-10
K1 = SCH1 / (SCT * SP2)    # 2^-4
K2 = SCT2 / (SCH1 * SW23)  # 2^-7
K3 = SCT3 / (SCT2 * SP2)   # 2^-4
KD = 1.0 / (SCT3 * SP2)    # 2^-17


# ---------------------------------------------------------------------------
# Workaround: this walrus build caps sync-waits per control instruction very
# low, so TileContext's tail drain (which waits on every proc's semaphore)
# fails codegen. Split the global-clock waits into one single-wait
# EventSemaphore each, then emit a bare Drain.
def _patched_drain_and_barrier(self, tick_clock, wait_clock):
    nc = self.nc
    num_to_handle = {h.num: h for h in self.sems.allocated().values()}
    probe = nc.sync.nop(nofuse=True)
    wait_clock.add_sem_waits(probe.ins, ScopedClock({None: tick_clock.global_clock}))
    waits = list(probe.ins.sync_info.on_wait)
    probe.ins.sync_info = bass_rust.SyncInfo(on_wait=[], on_update=[])
    engines = [nc.sync, nc.scalar, nc.vector, nc.tensor, nc.gpsimd]
    for i, w in enumerate(waits):
        h = num_to_handle.get(w.id)
        if h is None:
            raise RuntimeError(f"no sem handle for {w.id} ({w.ant_name})")
        engines[i % len(engines)].wait_ge(h, w.wait_value)
    nc.all_engine_barrier()
    nc.sync.drain()
    assert self.sems is not None
    popped = nc._tile_sem_poison_stack.pop()
    assert popped is self._sem_poison
    nc.clear_and_free_semaphores(list(self.sems.allocated().values()))
    nc.all_engine_barrier()


tile.TileContext._drain_and_barrier = _patched_drain_and_barrier


def _split_multi_waits(nc, max_waits=1):
    """This walrus build rejects instructions carrying more than one sync
    wait. Hoist extra waits into standalone EventSemaphore instructions
    placed immediately before the instruction on the same engine queue."""
    ctr = 0
    for fn in nc.m.functions:
        for bb in fn.blocks:
            insts = bb.instructions
            new = []
            changed = False
            for inst in insts:
                si = inst.sync_info
                waits = list(si.on_wait) if si is not None else []
                if len(waits) > max_waits:
                    changed = True
                    for w in waits[:-max_waits]:
                        ev = bass_rust.InstEventSemaphore(
                            name=f"splitw_{ctr}", ins=[], outs=[]
                        )
                        ctr += 1
                        ev.engine = inst.engine
                        ev.sync_info = bass_rust.SyncInfo(
                            on_wait=[w], on_update=[]
                        )
                        new.append(ev)
                    inst.sync_info = bass_rust.SyncInfo(
                        on_wait=waits[-max_waits:], on_update=list(si.on_update)
                    )
                new.append(inst)
            if changed:
                bb.instructions = new


# ---------------------------------------------------------------------------
# v3 schedule: L0(b0), L0(b1) run back-to-back up front (the front window is
# DMA-bandwidth-bound: w01 + x + p2t = 22.6 MB must land before stage A can
# start; batch 1's L0 gives the PE ~31 us of work that was otherwise idle
# wait). Then chain(b0) stages A..C + collective 0 overlap chain(b1); the
# decoder passes slot in where their inputs are ready (pass 0 fills part of
# the collective-1 wait). DMAs are batched into multi-tile transfers (DGE
# fixed costs ~0.6-1 us/DMA dominate small ones) and ordered by need within
# each queue — queues drain in program order far ahead of the PE, so program
# order per queue IS the bus priority. Decoder weights stream into one
# dedicated 2-tile slot plus group-tiles borrowed from x/t tiles as they die.
# NOTE: both bias DR matmuls (hi and lo) are required — layer-5's bias scale
# (1/sqrt(64)) makes the fp8-hi-only bias error ~5e-2 end-to-end.
def _build_program() -> bass.Bass:
    nc = bass.Bass()

    xhi = nc.declare_dram_parameter("xhi", [BL, NT, 128, CT, 128], F8, isOutput=False)
    xlo = nc.declare_dram_parameter("xlo", [BL, NT, 128, CT, 128], F8, isOutput=False)
    p2thi = nc.declare_dram_parameter("p2thi", [NT, 128, N], F8, isOutput=False)
    p2tlo = nc.declare_dram_parameter("p2tlo", [NT, 128, N], F8, isOutput=False)
    w01hi = nc.declare_dram_parameter("w01hi", [CT, 128, 512], F8, isOutput=False)
    w01lo = nc.declare_dram_parameter("w01lo", [CT, 128, 512], F8, isOutput=False)
    w23hi = nc.declare_dram_parameter("w23hi", [4, 128, 256], F8, isOutput=False)
    w23lo = nc.declare_dram_parameter("w23lo", [4, 128, 256], F8, isOutput=False)
    w45p = nc.declare_dram_parameter("w45p", [2, 128, 3], F16, isOutput=False)
    exdp = nc.declare_dram_parameter("exdp", [4, 2, N], F8, isOutput=False)
    biasA = nc.declare_dram_parameter("biasA", [4, 2, 512], F8, isOutput=False)
    biasB = nc.declare_dram_parameter("biasB", [4, 2, 256], F8, isOutput=False)
    biasC = nc.declare_dram_parameter("biasC", [4, 2, 3], F8, isOutput=False)
    wdp = nc.declare_dram_parameter("wdp", [KDEC, 128, DEC_SH], F16, isOutput=False)
    bdp = nc.declare_dram_parameter("bdp", [6, 1, 128], F16, isOutput=False)
    y_out = nc.declare_dram_parameter("y", [128, 6, B], F32, isOutput=True)

    cc_in = [nc.dram_tensor(f"cc_in{b}", [3, 128, NT], F16) for b in range(BL)]
    cc_out = [
        nc.dram_tensor(f"cc_out{b}", [N_CORES, 3, 128, NT], F16, addr_space="Shared")
        for b in range(BL)
    ]

    with tile.TileContext(nc) as tc:
        with (
            tc.tile_pool(name="const", bufs=1) as constp,
            tc.tile_pool(name="xp", bufs=2) as xpool,
            tc.tile_pool(name="tp", bufs=1) as tpool,
            tc.tile_pool(name="h1p", bufs=1) as h1pool,
            tc.tile_pool(name="t2p", bufs=1) as t2pool,
            tc.tile_pool(name="h3p", bufs=1) as h3pool,
            tc.tile_pool(name="vp", bufs=2) as vpool,
            tc.tile_pool(name="wdpool", bufs=1) as wdpool,
            tc.tile_pool(name="ps", bufs=5, space="PSUM") as ps,
            tc.tile_pool(name="psd", bufs=2, space="PSUM") as psd,
        ):
            # ---- const tiles ----
            w01hi_sb = constp.tile([128, CT, 512], F8, tag="w01hi")
            w01lo_sb = constp.tile([128, CT, 512], F8, tag="w01lo")
            p2thi_sb = constp.tile([128, NT, N], F8, tag="p2thi")
            p2tlo_sb = constp.tile([128, NT, N], F8, tag="p2tlo")
            w23hi_sb = constp.tile([128, 4, 256], F8, tag="w23hi")
            w23lo_sb = constp.tile([128, 4, 256], F8, tag="w23lo")
            w45_sb = constp.tile([128, 2, 3], F16, tag="w45")
            exd_sb = constp.tile([4, 2, N], F8, tag="exd")
            biasA_sb = constp.tile([4, 2, 512], F8, tag="biasA")
            biasB_sb = constp.tile([4, 2, 256], F8, tag="biasB")
            biasC_sb = constp.tile([4, 2, 3], F8, tag="biasC")
            bdp_sb = constp.tile([1, 6, 128], F16, tag="bdp")
            ones8 = constp.tile([1, 8], F16, tag="ones8")
            featT_sb = constp.tile([128, B, KDEC], F16, tag="featT")
            y_sb = constp.tile([128, 6, B], F32, tag="ysb")
            t3hi = constp.tile([128, NT * 3], F8, tag="t3hi")
            t3lo = constp.tile([128, NT * 3], F8, tag="t3lo")
            rl5 = constp.tile([128, NT * 3], F16, tag="rl5")
            h5_0 = constp.tile([128, 3, NT], F16, tag="h5_0")
            h5_1 = constp.tile([128, 3, NT], F16, tag="h5_1")
            h5_t = [h5_0, h5_1]

            wd_tiles: list = [None] * KDEC

            prefetched = {}

            def post_x(b, g0, ng, tag_hi, tag_lo):
                """DMA x node-tile groups [2*g0, 2*(g0+ng)) as ONE transfer
                per hi/lo half (DGE fixed costs dominate small DMAs). Batch 1
                rides the scalar/vector HWDGE queues with its own tags, so a
                WAR slot-reuse wait in one stream never head-of-line-blocks
                another stream's DMAs."""
                ghi = xpool.tile([128, 2 * ng, CT, 128], F8, tag=tag_hi)
                glo = xpool.tile([128, 2 * ng, CT, 128], F8, tag=tag_lo)
                nc.sync.dma_start(
                    ghi[:], xhi[b, 2 * g0 : 2 * g0 + 2 * ng].rearrange(
                        "n p c j -> p n c j"))
                nc.gpsimd.dma_start(
                    glo[:], xlo[b, 2 * g0 : 2 * g0 + 2 * ng].rearrange(
                        "n p c j -> p n c j"))
                for i in range(ng):
                    prefetched[(b, g0 + i)] = ((ghi, glo), i)

            # Startup: a 2-tile w01hi starter chunk and a single first x
            # group gate the PE's first matmul (~3.5us of transfers); the
            # rest of w01hi follows immediately. Batch 1 gets its own tags
            # (x1*) so its DMAs carry no WAR on batch-0 slots — a WAR
            # head-of-line-blocks the queue and lets p2t cut ahead on the
            # shared bus.
            nc.sync.dma_start(
                w01hi_sb[:, 0:2, :], w01hi[0:2].rearrange("c p f -> p c f"))
            post_x(0, 0, 1, "xhi", "xlo")
            nc.sync.dma_start(
                w01hi_sb[:, 2:CT, :], w01hi[2:CT].rearrange("c p f -> p c f"))
            nc.gpsimd.dma_start(
                w01lo_sb[:], w01lo[:].rearrange("c p f -> p c f"))
            post_x(0, 1, 1, "xhi", "xlo")

            def emit_small_consts():
                nc.sync.dma_start(w23hi_sb[:], w23hi[:].rearrange("c p f -> p c f"))
                nc.sync.dma_start(w23lo_sb[:], w23lo[:].rearrange("c p f -> p c f"))
                nc.sync.dma_start(w45_sb[:], w45p[:].rearrange("c p f -> p c f"))
                nc.sync.dma_start(exd_sb[:], exdp[:])
                nc.sync.dma_start(biasA_sb[:], biasA[:])
                nc.sync.dma_start(biasB_sb[:], biasB[:])
                nc.sync.dma_start(biasC_sb[:], biasC[:])
                nc.sync.dma_start(bdp_sb[:], bdp[:].rearrange("c o p -> o c p"))
                nc.vector.memset(ones8[:], 1.0)

            # p2t quad-row DMAs: hi on the HWDGE (sync) path, lo via the
            # otherwise-idle Pool engine's SWDGE; 4 src tiles per transfer to
            # amortize the ~1us per-DMA DGE cost.
            p2_rows = [(4 * i, 0) for i in range(4)] + [(4 * i, 1) for i in range(4)]
            p2_pos = [0]

            def emit_p2(n):
                for _ in range(n):
                    if p2_pos[0] >= len(p2_rows):
                        return
                    si, hl = p2_rows[p2_pos[0]]
                    p2_pos[0] += 1
                    if hl == 0:
                        nc.sync.dma_start(
                            p2thi_sb[:, si : si + 4, :],
                            p2thi[si : si + 4].rearrange("s p n -> p s n"))
                    else:
                        nc.gpsimd.dma_start(
                            p2tlo_sb[:, si : si + 4, :],
                            p2tlo[si : si + 4].rearrange("s p n -> p s n"))

            wd_pos = [0]
            wd_direct = [0]
            N_WD_DIRECT_GROUPS = 1

            def borrow_wd(pool, tag, n):
                """Land n decoder k-tiles in one group DMA into a dead tile's
                slot (tag must never be allocated again afterwards)."""
                k0 = wd_pos[0]
                assert k0 + n <= KDEC
                wd_pos[0] += n
                gt = pool.tile([128, n, DEC_SH], F16, tag=tag)
                nc.gpsimd.dma_start(
                    gt[:], wdp[k0 : k0 + n].rearrange("k p f -> p k f"))
                for i in range(n):
                    wd_tiles[k0 + i] = gt[:, i, :]

            def emit_wd(n):
                """Stream decoder-weight k-tiles into the dedicated pool,
                2 per transfer."""
                for _ in range(n):
                    if wd_direct[0] >= N_WD_DIRECT_GROUPS or wd_pos[0] + 2 > KDEC:
                        return
                    wd_direct[0] += 1
                    borrow_wd(wdpool, "wd", 2)

            def l0_stage(b, t_hi, t_lo):
                # ---- L0: t = x @ W01 (3-term fp8 DR) ----
                # x streams in node-tile groups: hi via HWDGE, lo via SWDGE.
                # Groups 0/1 land as singles (startup latency), the rest as
                # pair transfers; batch 1's stream is fully posted during
                # batch 0's loop.
                for g in range(NT // 2):
                    (ghi, glo), gi = prefetched.pop((b, g))
                    for k in range(2):
                        nt = 2 * g + k
                        pt = ps.tile([128, 512], F32, tag="ps")
                        first = True
                        for p in range(CT // 2):
                            nc.tensor.matmul(
                                pt[:], ghi[:, 2 * gi + k, 2 * p : 2 * p + 2, :],
                                w01hi_sb[:, 2 * p : 2 * p + 2, :],
                                start=first, stop=False, perf_mode=DRM,
                            )
                            first = False
                        for p in range(CT // 2):
                            nc.tensor.matmul(
                                pt[:], glo[:, 2 * gi + k, 2 * p : 2 * p + 2, :],
                                w01hi_sb[:, 2 * p : 2 * p + 2, :],
                                start=False, stop=False, perf_mode=DRM,
                            )
                        for p in range(CT // 2):
                            nc.tensor.matmul(
                                pt[:], ghi[:, 2 * gi + k, 2 * p : 2 * p + 2, :],
                                w01lo_sb[:, 2 * p : 2 * p + 2, :],
                                start=False, stop=(p == CT // 2 - 1),
                                perf_mode=DRM,
                            )
                        nc.scalar.activation(t_hi[:, nt, :], pt[:], ACTF.Copy,
                                             scale=KT)
                        nc.vector.scalar_tensor_tensor(
                            t_lo[:, nt, :], pt[:], KT, t_hi[:, nt, :],
                            ALU.mult, ALU.subtract,
                        )
                    if b == 0:
                        if g == 0:
                            post_x(0, 2, 2, "xhi", "xlo")
                        elif g == 1:
                            post_x(0, 4, 2, "xhi", "xlo")
                            # batch-1 pairs 1-2 use fresh slots (no WAR):
                            # early in the queue, they can never block it
                            post_x(1, 0, 2, "x1hi", "x1lo")
                        elif g == 2:
                            post_x(0, 6, 2, "xhi", "xlo")
                        elif g == 3:
                            emit_small_consts()
                            post_x(1, 2, 2, "x1hi", "x1lo")

            def prop_stage(b, src_hi, src_lo, bias_sb, nf, out_cb, wd_budget):
                # ---- z = P2 @ src + bias (single hi-bias DR matmul) ----
                # nf: number of 128-wide feature tiles in src (4 for stage A,
                # 2 for stage B). out_cb(dc, fj, pt) evicts the PSUM tile.
                for dc in range(4):
                    dsl = slice(dc * 512, (dc + 1) * 512)
                    for fj in range(nf):
                        fsl = slice(fj * 128, (fj + 1) * 128)
                        pt = ps.tile([128, 512], F32, tag="ps")
                        for sp in range(NT // 2):
                            ssl = slice(2 * sp, 2 * sp + 2)
                            nc.tensor.matmul(
                                pt[:], src_hi[:, ssl, fsl],
                                p2thi_sb[:, ssl, dsl],
                                start=(sp == 0), stop=False, perf_mode=DRM,
                            )
                        for sp in range(NT // 2):
                            ssl = slice(2 * sp, 2 * sp + 2)
                            nc.tensor.matmul(
                                pt[:], src_lo[:, ssl, fsl],
                                p2thi_sb[:, ssl, dsl],
                                start=False, stop=False, perf_mode=DRM,
                            )
                        for sp in range(NT // 2):
                            ssl = slice(2 * sp, 2 * sp + 2)
                            nc.tensor.matmul(
                                pt[:], src_hi[:, ssl, fsl],
                                p2tlo_sb[:, ssl, dsl],
                                start=False, stop=False, perf_mode=DRM,
                            )
                        # bias: all four hi/lo row-pairs packed into one
                        # K=4x2 DoubleRow matmul (exact same sum as the old
                        # two-instruction form)
                        nc.tensor.matmul(
                            pt[:], bias_sb[:, :, fsl], exd_sb[:, :, dsl],
                            start=False, stop=True, perf_mode=DRM,
                        )
                        out_cb(dc, fj, pt)
                        emit_wd(wd_budget)

            def stageA(b, t_hi, t_lo, h1hi, h1lo):
                def evict(dc, fj, pt):
                    dsl = slice(dc * 512, (dc + 1) * 512)
                    v = vpool.tile([128, 512], F16, tag="v")
                    nc.scalar.activation(v[:], pt[:], ACTF.Lrelu,
                                         scale=K1, alpha=ALPHA)
                    nc.scalar.activation(h1hi[:, fj, dsl], v[:], ACTF.Copy)
                    nc.vector.tensor_tensor(
                        h1lo[:, fj, dsl], v[:], h1hi[:, fj, dsl],
                        ALU.subtract)

                prop_stage(b, t_hi, t_lo, biasA_sb, 4, evict,
                           1 if b == 0 else 0)

            def w23_stage(h1hi, h1lo, t2hi, t2lo):
                # ---- W23: t2 = h1 @ W23 ----
                for nt in range(NT):
                    nsl = slice(nt * 128, (nt + 1) * 128)
                    pt = ps.tile([128, 512], F32, tag="ps")
                    for fp in range(2):
                        fsl = slice(2 * fp, 2 * fp + 2)
                        nc.tensor.matmul(
                            pt[:, 0:256], h1hi[:, fsl, nsl], w23hi_sb[:, fsl, :],
                            start=(fp == 0), stop=False, perf_mode=DRM,
                        )
                    for fp in range(2):
                        fsl = slice(2 * fp, 2 * fp + 2)
                        nc.tensor.matmul(
                            pt[:, 0:256], h1lo[:, fsl, nsl], w23hi_sb[:, fsl, :],
                            start=False, stop=False, perf_mode=DRM,
                        )
                    for fp in range(2):
                        fsl = slice(2 * fp, 2 * fp + 2)
                        nc.tensor.matmul(
                            pt[:, 0:256], h1hi[:, fsl, nsl], w23lo_sb[:, fsl, :],
                            start=False, stop=(fp == 1), perf_mode=DRM,
                        )
                    nc.scalar.activation(t2hi[:, nt, :], pt[:, 0:256], ACTF.Copy,
                                         scale=K2)
                    nc.vector.scalar_tensor_tensor(
                        t2lo[:, nt, :], pt[:, 0:256], K2, t2hi[:, nt, :],
                        ALU.mult, ALU.subtract,
                    )

            def stageB(b, t2hi, t2lo, h3):
                def evict(dc, fj, pt):
                    dsl = slice(dc * 512, (dc + 1) * 512)
                    nc.scalar.activation(h3[:, fj, dsl], pt[:], ACTF.Lrelu,
                                         alpha=ALPHA)

                prop_stage(b, t2hi, t2lo, biasB_sb, 2, evict, 0)

            def tail_stages(b, h3):
                # ---- W45: t3 = h3 @ W45 (fp16) ----
                # psd bank: no wait on stage-B eviction chains draining ps
                pt45 = psd.tile([128, 512], F32, tag="psd")
                for nt in range(NT):
                    nsl = slice(nt * 128, (nt + 1) * 128)
                    for cj in range(2):
                        nc.tensor.matmul(
                            pt45[:, nt * 3 : nt * 3 + 3],
                            h3[:, cj, nsl], w45_sb[:, cj, :],
                            start=(cj == 0), stop=(cj == 1),
                        )
                nc.scalar.activation(t3hi[:], pt45[:, 0 : NT * 3], ACTF.Copy,
                                     scale=K3)
                nc.vector.scalar_tensor_tensor(
                    t3lo[:], pt45[:, 0 : NT * 3], K3, t3hi[:],
                    ALU.mult, ALU.subtract,
                )

                # ---- stage C: z5 = P2 @ t3 + bias ; h5 = leaky(z5) ----
                psC = psd.tile([128, 512], F32, tag="psd")
                for di in range(NT):
                    dsl = slice(di * 128, (di + 1) * 128)
                    osl = slice(di * 3, di * 3 + 3)
                    for sp in range(NT // 2):
                        t3sl = t3hi[:, 6 * sp : 6 * sp + 6].rearrange(
                            "p (s c) -> p s c", s=2, c=3)
                        nc.tensor.matmul(
                            psC[:, osl], p2thi_sb[:, 2 * sp : 2 * sp + 2, dsl],
                            t3sl, start=(sp == 0), stop=False, perf_mode=DRM,
                        )
                    for sp in range(NT // 2):
                        t3sl = t3lo[:, 6 * sp : 6 * sp + 6].rearrange(
                            "p (s c) -> p s c", s=2, c=3)
                        nc.tensor.matmul(
                            psC[:, osl], p2thi_sb[:, 2 * sp : 2 * sp + 2, dsl],
                            t3sl, start=False, stop=False, perf_mode=DRM,
                        )
                    for sp in range(NT // 2):
                        t3sl = t3hi[:, 6 * sp : 6 * sp + 6].rearrange(
                            "p (s c) -> p s c", s=2, c=3)
                        nc.tensor.matmul(
                            psC[:, osl], p2tlo_sb[:, 2 * sp : 2 * sp + 2, dsl],
                            t3sl, start=False, stop=False, perf_mode=DRM,
                        )
                    nc.tensor.matmul(
                        psC[:, osl], exd_sb[:, :, dsl], biasC_sb[:],
                        start=False, stop=True, perf_mode=DRM,
                    )
                nc.scalar.activation(
                    h5_t[b][:].rearrange("p c d -> p d c"),
                    psC[:, 0 : NT * 3].rearrange("p (d c) -> p d c", d=NT, c=3),
                    ACTF.Lrelu, alpha=ALPHA,
                )
                nc.sync.dma_start(
                    cc_in[b][:].rearrange("c p n -> p c n"), h5_t[b][:])
                nc.gpsimd.collective_compute(
                    "AllGather",
                    ALU.bypass,
                    replica_groups=[list(range(N_CORES))],
                    ins=[cc_in[b][:]],
                    outs=[cc_out[b][:]],
                )

            def featT_dma(half):
                for c in range(3):
                    nc.sync.dma_start(
                        featT_sb[:, half : B : BL, c * NT : (c + 1) * NT],
                        cc_out[half][:, c].rearrange("core p n -> p core n"),
                    )

            def decoder_pass(half):
                # bias rides the accumulation as a K=1 matmul row per ct
                # chunk so the eviction is ONE ACT (six tiny serial ACTs
                # previously cost ~2.4us on the kernel tail)
                pd = psd.tile([128, 512], F32, tag="psd")
                for kt in range(KDEC):
                    rhs = featT_sb[:, half : B : BL, kt : kt + 1]
                    for ct in range(6):
                        nc.tensor.matmul(
                            pd[:, ct * 8 : (ct + 1) * 8],
                            wd_tiles[kt][:, ct * 128 : (ct + 1) * 128],
                            rhs, start=(kt == 0), stop=False,
                        )
                for ct in range(6):
                    nc.tensor.matmul(
                        pd[:, ct * 8 : (ct + 1) * 8],
                        bdp_sb[:, ct, :], ones8[:],
                        start=False, stop=(ct == 5),
                    )
                nc.scalar.activation(
                    y_sb[:, :, half : B : BL].rearrange("p c b -> p (c b)"),
                    pd[:, 0:48], ACTF.Tanh, scale=KD,
                )

            # ---- emission schedule ----
            t_hi0 = tpool.tile([128, NT, 512], F8, tag="t_hi0")
            t_lo0 = tpool.tile([128, NT, 512], F8, tag="t_lo0")
            t_hi1 = tpool.tile([128, NT, 512], F8, tag="t_hi1")
            t_lo1 = tpool.tile([128, NT, 512], F8, tag="t_lo1")

            l0_stage(0, t_hi0, t_lo0)
            # p2t streams behind batch-0 x and batch-1's first two pairs;
            # batch-1's WAR-gated pairs go after p2t in queue order (their
            # slot-reuse waits fire mid-L0(b1) and would head-of-line-block
            # p2t otherwise; arriving bus-interleaved with p2t is in time).
            emit_p2(8)
            post_x(1, 4, 2, "x1hi", "x1lo")
            post_x(1, 6, 2, "x1hi", "x1lo")
            l0_stage(1, t_hi1, t_lo1)
            # x tiles are dead from here: 24 decoder k-tiles into their slots
            for tag in ("xhi", "xlo", "x1hi"):
                borrow_wd(xpool, tag, 4)
                borrow_wd(xpool, tag, 4)
            borrow_wd(xpool, "x1lo", 2)

            h1hi = h1pool.tile([128, 4, N], F8, tag="h1hi")
            h1lo = h1pool.tile([128, 4, N], F8, tag="h1lo")
            stageA(0, t_hi0, t_lo0, h1hi, h1lo)
            emit_wd(N_WD_DIRECT_GROUPS)  # any remainder
            borrow_wd(tpool, "t_hi0", 5)
            borrow_wd(tpool, "t_lo0", 5)

            t2hi = t2pool.tile([128, NT, 256], F8, tag="t2hi")
            t2lo = t2pool.tile([128, NT, 256], F8, tag="t2lo")
            w23_stage(h1hi, h1lo, t2hi, t2lo)
            h3 = h3pool.tile([128, 2, N], F16, tag="h3")
            stageB(0, t2hi, t2lo, h3)
            tail_stages(0, h3)
            featT_dma(0)

            h1hi = h1pool.tile([128, 4, N], F8, tag="h1hi")
            h1lo = h1pool.tile([128, 4, N], F8, tag="h1lo")
            stageA(1, t_hi1, t_lo1, h1hi, h1lo)
            borrow_wd(tpool, "t_hi1", 5)
            borrow_wd(tpool, "t_lo1", 5)
            assert wd_pos[0] == KDEC, wd_pos[0]

            t2hi = t2pool.tile([128, NT, 256], F8, tag="t2hi")
            t2lo = t2pool.tile([128, NT, 256], F8, tag="t2lo")
            w23_stage(h1hi, h1lo, t2hi, t2lo)
            h3 = h3pool.tile([128, 2, N], F16, tag="h3")
            stageB(1, t2hi, t2lo, h3)
            tail_stages(1, h3)
            decoder_pass(0)  # fills part of the cc1 wait
            featT_dma(1)
            decoder_pass(1)
            nc.sync.dma_start(y_out[:], y_sb[:])

    _split_multi_waits(nc)
    return nc


# ---------------------------------------------------------------------------
def _split8(a):
    hi = np.asarray(a, np.float32).astype(NPF8)
    lo = (np.asarray(a, np.float64) - hi.astype(np.float64)).astype(
        np.float32).astype(NPF8)
    return hi, lo


def _host_precompute(x, edges, Ws, bs, Wd, bd_np):
    edges = np.asarray(edges)
    src = edges[0].astype(np.int64)
    dst = edges[1].astype(np.int64)

    deg = np.bincount(dst, minlength=N).astype(np.float64) + 1.0
    isd = 1.0 / np.sqrt(deg)
    idg = 1.0 / deg

    P = np.zeros((N, N), np.float64)
    np.add.at(P, (dst, src), isd[src] * isd[dst])
    P[np.arange(N), np.arange(N)] += idg
    P2 = P @ P
    r = P.sum(axis=1)

    W0, W1, W2, W3, W4, W5 = [np.asarray(w, np.float64) for w in Ws]
    b0, b1, b2, b3, b4, b5 = [np.asarray(b, np.float64) for b in bs]
    W01 = W0 @ W1
    W23 = W2 @ W3
    W45 = W4 @ W5
    a1 = b0 @ W1
    a3 = b2 @ W3
    a5 = b4 @ W5

    # p2t[si, p, d] = P2[d, si*128+p] * SP2
    p2t_s = np.ascontiguousarray((P2.T * SP2).reshape(NT, 128, N))
    p2thi, p2tlo = _split8(p2t_s)

    w01_pad = np.zeros((C_PAD, 512), np.float64)
    w01_pad[:C_IN] = W01
    w01hi, w01lo = _split8((w01_pad * SW01).reshape(CT, 128, 512))
    w23hi, w23lo = _split8((W23 * SW23).reshape(4, 128, 256))
    w45_np = W45.reshape(2, 128, 3).astype(NPF16)

    # Packed bias operands: one K=4x2 DoubleRow matmul computes
    #   (a_hi+a_lo)(r_hi+r_lo) + (b_hi+b_lo)*SEXD
    # exactly as the old two-matmul form.
    rhi, rlo = _split8(r * SEXD)
    one8 = np.full(N, SEXD, np.float32).astype(NPF8)
    exdp_np = np.zeros((4, 2, N), NPF8)
    exdp_np[0] = np.stack([rhi, rlo])
    exdp_np[1] = np.stack([rhi, rlo])
    exdp_np[2] = np.stack([one8, one8])
    exdp = np.ascontiguousarray(exdp_np)

    def bias4(a, bvec, s):
        ahi, alo = _split8(np.asarray(a) * s)
        bhi, blo = _split8(np.asarray(bvec) * s)
        out = np.zeros((4, 2, len(ahi)), NPF8)
        out[0] = np.stack([ahi, ahi])
        out[1] = np.stack([alo, alo])
        out[2] = np.stack([bhi, blo])
        return np.ascontiguousarray(out)

    biasA_np = bias4(a1, b1, SBA)
    biasB_np = bias4(a3, b3, SBB)
    biasC_np = bias4(a5, b5, SBC)

    # x: pad channels, scale, split; layout [BL,NT,p=chan,CT,j=node]
    x_np = np.asarray(x, np.float32)
    x_pad = np.zeros((B, N, C_PAD), np.float32)
    x_pad[:, :, :C_IN] = x_np * SX
    xt_all = np.ascontiguousarray(
        x_pad.reshape(B, NT, 128, CT, 128).transpose(0, 1, 4, 3, 2))
    xhi_all, xlo_all = _split8(xt_all)

    # decoder: permuted rows j' = c*2048 + node
    Wd_np = np.asarray(Wd, np.float64)
    node = np.arange(N)
    rows = np.concatenate([node * 3 + c for c in range(3)])  # j' -> orig j
    Wd_perm = Wd_np[rows]  # [6144, 6144] in j' order
    bd_full = np.asarray(bd_np, np.float64)

    shared = {
        "p2thi": p2thi, "p2tlo": p2tlo,
        "w01hi": w01hi, "w01lo": w01lo,
        "w23hi": w23hi, "w23lo": w23lo,
        "w45p": w45_np,
        "exdp": exdp,
        "biasA": biasA_np, "biasB": biasB_np, "biasC": biasC_np,
    }
    in_maps = []
    for c in range(N_CORES):
        wd_c = np.ascontiguousarray(
            Wd_perm[:, c * DEC_SH : (c + 1) * DEC_SH]
            .reshape(KDEC, 128, DEC_SH).astype(NPF16))
        bd_c = np.ascontiguousarray(
            (bd_full[c * DEC_SH : (c + 1) * DEC_SH] * (SCT3 * SP2))
            .reshape(6, 1, 128).astype(NPF16))  # [6, 1, 128], pre-divided by KD
        m = dict(shared)
        m["xhi"] = xhi_all[c * BL : (c + 1) * BL]
        m["xlo"] = xlo_all[c * BL : (c + 1) * BL]
        m["wdp"] = wd_c
        m["bdp"] = bd_c
        in_maps.append(m)
    return in_maps


_NC_CACHE = {}


def kernel(**inputs) -> np.ndarray:
    x = inputs["x"]
    edges = inputs["edges"]
    Ws = [inputs[f"W{i}"] for i in range(6)]
    bs = [inputs[f"b{i}"] for i in range(6)]
    Wd = inputs["Wd"]
    bd_np = inputs["bd"]

    in_maps = _host_precompute(x, edges, Ws, bs, Wd, bd_np)

    if "nc" not in _NC_CACHE:
        _NC_CACHE["nc"] = _build_program()
    nc = _NC_CACHE["nc"]

    res = run_bass_kernel_spmd(nc, in_maps, list(range(N_CORES)))

    out = np.zeros((B, D_DEC), np.float32)
    for c in range(N_CORES):
        y = res.results[c]["y"]  # [128, 6, 16]
        out[:, c * DEC_SH : (c + 1) * DEC_SH] = (
            0.1 * y.transpose(2, 1, 0).reshape(B, DEC_SH))
    return out.reshape(B, N, 3)



# revision 57
# speedup vs baseline: 1.1100x; 1.0051x over previous
"""Trainium2 Bass kernel for nn_DeformGCN (6-layer GCN + dense decoder).

Strategy (v3, fp8 DoubleRow):
  - Host precompute from `edges`: dense propagation matrix P (N x N) with
    P[dst,src] += 1/sqrt(deg_s*deg_d) and P[n,n] += 1/deg_n, then P2 = P @ P.
    GCN layer pairs fuse into 3 propagation stages (A, B, C):
      z = P2 @ (h @ (Wa@Wb)) + r (x) (ba@Wb) + 1 (x) bb,  r = P @ 1
    followed by LeakyReLU(0.01).
  - All heavy matmuls run as fp8e4m3 DoubleRow (2 x 128-deep products per
    instruction at 0.5 cycles/row = 4x bf16 MAC rate). Accuracy is restored
    with a hi/lo residual split of both operands; the lo*lo term is dropped
    (3-term scheme, 0.75x the bf16 row count). End-to-end rel err ~2e-3.
  - Activations are split on the fly during PSUM eviction:
      hi = ACT copy(psum, scale=k) -> fp8 ; lo = DVE (psum*k - hi) -> fp8
    LeakyReLU is the ACT engine's native Lrelu (alpha=0.01) — one op.
  - Small stages (W45, decoder) run in fp16 (1.0 cycles/row, 10-bit mantissa).
  - Decoder is column-sharded (768 cols/core) and oriented [col_part x batch]
    so its cost is 48*6*16 rows. Features are AllGather'd per local batch (2
    collectives; the first fully overlaps with batch 1's GCN compute, and
    decoder pass 0 fills part of the second collective's wait).
  - Biases ride the propagation matmuls as ONE K=4x2 DoubleRow matmul per
    PSUM group: rows (a*r)hi/lo and (b*1)hi/lo packed on 4 partitions --
    exactly the old two-instruction sum at half the PE cost. Both hi and lo
    bias halves are required: layer-5's bias scale (1/sqrt(64)) makes an
    fp8-hi-only bias error ~5e-2 end-to-end.
  - v3 schedule: both batches' L0 run back-to-back up front (the front is
    DMA-bus-bound: w01 + x + p2t = 22.6 MB must land before stage A starts;
    batch 1's L0 fills the otherwise-idle PE). DMAs are batched into
    multi-tile transfers (DGE fixed costs ~0.6-1 us each dominate small
    ones); each queue's program order IS the bus priority since queues run
    far ahead of the PE. Batch 1's x stream uses its own SBUF tags so slot
    WARs never head-of-line-block the queue behind them. W45/stage-C PSUM
    comes from the decoder's pool so it never waits on stage-B evictions.
"""

import os
import numpy as np
import ml_dtypes

_STAGE_LIMIT = int(os.environ.get("KSTAGE", "99"))

import bass_rust
import concourse.bass as bass
import concourse.mybir as mybir
import concourse.tile as tile
from concourse.tile import ScopedClock
from concourse.bass_utils import run_bass_kernel_spmd

F8 = mybir.dt.float8e4
F16 = mybir.dt.float16
F32 = mybir.dt.float32
NPF8 = ml_dtypes.float8_e4m3
NPF16 = np.float16
DRM = mybir.MatmulPerfMode.DoubleRow
ALU = mybir.AluOpType
ACTF = mybir.ActivationFunctionType

N_CORES = 8
B = 16
N = 2048
C_IN = 1475
CP = 123               # channel-tile partition count (12*123 = 1476 >= 1475)
C_PAD = 12 * CP        # 1476: only one padded channel shipped, not 61
NT = N // 128          # 16 node tiles
CT = 12                # channel tiles
BL = B // N_CORES      # 2 local batches
D_DEC = N * 3          # 6144
DEC_SH = D_DEC // N_CORES  # 768 decoder columns per core
KDEC = D_DEC // 128    # 48 decoder k tiles
ALPHA = 0.01

# scales (powers of two; products of the two operand scales give the PSUM
# scale, the ACT evict rescales to the next storage scale)
SX = 16.0
SW01 = 2048.0
SCT = 32.0
SP2 = 64.0
SCH1 = 128.0
SW23 = 512.0
SCT2 = 512.0
SCT3 = 2048.0
SEXD = 64.0
SBA = SCT * SP2 / SEXD     # 32
SBB = SCT2 * SP2 / SEXD    # 512
SBC = SCT3 * SP2 / SEXD    # 2048
KT = SCT / (SX * SW01)     # 2^-10
K1 = SCH1 / (SCT * SP2)    # 2^-4
K2 = SCT2 / (SCH1 * SW23)  # 2^-7
K3 = SCT3 / (SCT2 * SP2)   # 2^-4
KD = 1.0 / (SCT3 * SP2)    # 2^-17


# ---------------------------------------------------------------------------
# Workaround: this walrus build caps sync-waits per control instruction very
# low, so TileContext's tail drain (which waits on every proc's semaphore)
# fails codegen. Split the global-clock waits into one single-wait
# EventSemaphore each, then emit a bare Drain.
def _patched_drain_and_barrier(self, tick_clock, wait_clock):
    nc = self.nc
    num_to_handle = {h.num: h for h in self.sems.allocated().values()}
    probe = nc.sync.nop(nofuse=True)
    wait_clock.add_sem_waits(probe.ins, ScopedClock({None: tick_clock.global_clock}))
    waits = list(probe.ins.sync_info.on_wait)
    probe.ins.sync_info = bass_rust.SyncInfo(on_wait=[], on_update=[])
    engines = [nc.sync, nc.scalar, nc.vector, nc.tensor, nc.gpsimd]
    for i, w in enumerate(waits):
        h = num_to_handle.get(w.id)
        if h is None:
            raise RuntimeError(f"no sem handle for {w.id} ({w.ant_name})")
        engines[i % len(engines)].wait_ge(h, w.wait_value)
    nc.all_engine_barrier()
    nc.sync.drain()
    assert self.sems is not None
    popped = nc._tile_sem_poison_stack.pop()
    assert popped is self._sem_poison
    nc.clear_and_free_semaphores(list(self.sems.allocated().values()))
    nc.all_engine_barrier()


tile.TileContext._drain_and_barrier = _patched_drain_and_barrier


def _split_multi_waits(nc, max_waits=1):
    """This walrus build rejects instructions carrying more than one sync
    wait. Hoist extra waits into standalone EventSemaphore instructions
    placed immediately before the instruction on the same engine queue."""
    ctr = 0
    for fn in nc.m.functions:
        for bb in fn.blocks:
            insts = bb.instructions
            new = []
            changed = False
            for inst in insts:
                si = inst.sync_info
                waits = list(si.on_wait) if si is not None else []
                if len(waits) > max_waits:
                    changed = True
                    for w in waits[:-max_waits]:
                        ev = bass_rust.InstEventSemaphore(
                            name=f"splitw_{ctr}", ins=[], outs=[]
                        )
                        ctr += 1
                        ev.engine = inst.engine
                        ev.sync_info = bass_rust.SyncInfo(
                            on_wait=[w], on_update=[]
                        )
                        new.append(ev)
                    inst.sync_info = bass_rust.SyncInfo(
                        on_wait=waits[-max_waits:], on_update=list(si.on_update)
                    )
                new.append(inst)
            if changed:
                bb.instructions = new


# ---------------------------------------------------------------------------
# v3 schedule: L0(b0), L0(b1) run back-to-back up front (the front window is
# DMA-bandwidth-bound: w01 + x + p2t = 22.6 MB must land before stage A can
# start; batch 1's L0 gives the PE ~31 us of work that was otherwise idle
# wait). Then chain(b0) stages A..C + collective 0 overlap chain(b1); the
# decoder passes slot in where their inputs are ready (pass 0 fills part of
# the collective-1 wait). DMAs are batched into multi-tile transfers (DGE
# fixed costs ~0.6-1 us/DMA dominate small ones) and ordered by need within
# each queue — queues drain in program order far ahead of the PE, so program
# order per queue IS the bus priority. Decoder weights stream into one
# dedicated 2-tile slot plus group-tiles borrowed from x/t tiles as they die.
# NOTE: both bias DR matmuls (hi and lo) are required — layer-5's bias scale
# (1/sqrt(64)) makes the fp8-hi-only bias error ~5e-2 end-to-end.
def _build_program() -> bass.Bass:
    nc = bass.Bass()

    xhi = nc.declare_dram_parameter("xhi", [BL, NT, CP, CT, 128], F8, isOutput=False)
    xlo = nc.declare_dram_parameter("xlo", [BL, NT, CP, CT, 128], F8, isOutput=False)
    p2thi = nc.declare_dram_parameter("p2thi", [NT, 128, N], F8, isOutput=False)
    p2tlo = nc.declare_dram_parameter("p2tlo", [NT, 128, N], F8, isOutput=False)
    w01hi = nc.declare_dram_parameter("w01hi", [CT, CP, 512], F8, isOutput=False)
    w01lo = nc.declare_dram_parameter("w01lo", [CT, CP, 512], F8, isOutput=False)
    w23hi = nc.declare_dram_parameter("w23hi", [4, 128, 256], F8, isOutput=False)
    w23lo = nc.declare_dram_parameter("w23lo", [4, 128, 256], F8, isOutput=False)
    w45p = nc.declare_dram_parameter("w45p", [2, 128, 3], F16, isOutput=False)
    exdp = nc.declare_dram_parameter("exdp", [4, 2, N], F8, isOutput=False)
    biasA = nc.declare_dram_parameter("biasA", [4, 2, 512], F8, isOutput=False)
    biasB = nc.declare_dram_parameter("biasB", [4, 2, 256], F8, isOutput=False)
    biasC = nc.declare_dram_parameter("biasC", [4, 2, 3], F8, isOutput=False)
    wdp = nc.declare_dram_parameter("wdp", [KDEC, 128, DEC_SH], F16, isOutput=False)
    bdp = nc.declare_dram_parameter("bdp", [6, 1, 128], F16, isOutput=False)
    y_out = nc.declare_dram_parameter("y", [128, 6, B], F32, isOutput=True)

    cc_in = [nc.dram_tensor(f"cc_in{b}", [3, 128, NT], F16) for b in range(BL)]
    cc_out = [
        nc.dram_tensor(f"cc_out{b}", [N_CORES, 3, 128, NT], F16, addr_space="Shared")
        for b in range(BL)
    ]

    with tile.TileContext(nc) as tc:
        with (
            tc.tile_pool(name="const", bufs=1) as constp,
            tc.tile_pool(name="xp", bufs=2) as xpool,
            tc.tile_pool(name="tp", bufs=1) as tpool,
            tc.tile_pool(name="h1p", bufs=1) as h1pool,
            tc.tile_pool(name="t2p", bufs=1) as t2pool,
            tc.tile_pool(name="h3p", bufs=1) as h3pool,
            tc.tile_pool(name="vp", bufs=2) as vpool,
            tc.tile_pool(name="wdpool", bufs=1) as wdpool,
            tc.tile_pool(name="ps", bufs=5, space="PSUM") as ps,
            tc.tile_pool(name="psd", bufs=2, space="PSUM") as psd,
        ):
            # ---- const tiles ----
            w01hi_sb = constp.tile([CP, CT, 512], F8, tag="w01hi")
            w01lo_sb = constp.tile([CP, CT, 512], F8, tag="w01lo")
            p2thi_sb = constp.tile([128, NT, N], F8, tag="p2thi")
            p2tlo_sb = constp.tile([128, NT, N], F8, tag="p2tlo")
            w23hi_sb = constp.tile([128, 4, 256], F8, tag="w23hi")
            w23lo_sb = constp.tile([128, 4, 256], F8, tag="w23lo")
            w45_sb = constp.tile([128, 2, 3], F16, tag="w45")
            exd_sb = constp.tile([4, 2, N], F8, tag="exd")
            biasA_sb = constp.tile([4, 2, 512], F8, tag="biasA")
            biasB_sb = constp.tile([4, 2, 256], F8, tag="biasB")
            biasC_sb = constp.tile([4, 2, 3], F8, tag="biasC")
            bdp_sb = constp.tile([1, 6, 128], F16, tag="bdp")
            ones8 = constp.tile([1, 8], F16, tag="ones8")
            featT_sb = constp.tile([128, B, KDEC], F16, tag="featT")
            y_sb = constp.tile([128, 6, B], F32, tag="ysb")
            t3hi = constp.tile([128, NT * 3], F8, tag="t3hi")
            t3lo = constp.tile([128, NT * 3], F8, tag="t3lo")
            rl5 = constp.tile([128, NT * 3], F16, tag="rl5")
            h5_0 = constp.tile([128, 3, NT], F16, tag="h5_0")
            h5_1 = constp.tile([128, 3, NT], F16, tag="h5_1")
            h5_t = [h5_0, h5_1]

            wd_tiles: list = [None] * KDEC

            prefetched = {}

            def post_x(b, g0, ng, tag_hi, tag_lo):
                """DMA x node-tile groups [2*g0, 2*(g0+ng)) as ONE transfer
                per hi/lo half (DGE fixed costs dominate small DMAs). Batch 1
                rides the scalar/vector HWDGE queues with its own tags, so a
                WAR slot-reuse wait in one stream never head-of-line-blocks
                another stream's DMAs."""
                ghi = xpool.tile([CP, 2 * ng, CT, 128], F8, tag=tag_hi)
                glo = xpool.tile([CP, 2 * ng, CT, 128], F8, tag=tag_lo)
                nc.sync.dma_start(
                    ghi[:], xhi[b, 2 * g0 : 2 * g0 + 2 * ng].rearrange(
                        "n p c j -> p n c j"))
                nc.gpsimd.dma_start(
                    glo[:], xlo[b, 2 * g0 : 2 * g0 + 2 * ng].rearrange(
                        "n p c j -> p n c j"))
                for i in range(ng):
                    prefetched[(b, g0 + i)] = ((ghi, glo), i)

            # Startup: a 2-tile w01hi starter chunk and a single first x
            # group gate the PE's first matmul (~3.5us of transfers); the
            # rest of w01hi follows immediately. Batch 1 gets its own tags
            # (x1*) so its DMAs carry no WAR on batch-0 slots — a WAR
            # head-of-line-blocks the queue and lets p2t cut ahead on the
            # shared bus.
            nc.sync.dma_start(
                w01hi_sb[:, 0:2, :], w01hi[0:2].rearrange("c p f -> p c f"))
            post_x(0, 0, 1, "xhi", "xlo")
            nc.sync.dma_start(
                w01hi_sb[:, 2:CT, :], w01hi[2:CT].rearrange("c p f -> p c f"))
            nc.gpsimd.dma_start(
                w01lo_sb[:], w01lo[:].rearrange("c p f -> p c f"))
            post_x(0, 1, 1, "xhi", "xlo")

            def emit_small_consts():
                nc.sync.dma_start(w23hi_sb[:], w23hi[:].rearrange("c p f -> p c f"))
                nc.sync.dma_start(w23lo_sb[:], w23lo[:].rearrange("c p f -> p c f"))
                nc.sync.dma_start(w45_sb[:], w45p[:].rearrange("c p f -> p c f"))
                nc.sync.dma_start(exd_sb[:], exdp[:])
                nc.sync.dma_start(biasA_sb[:], biasA[:])
                nc.sync.dma_start(biasB_sb[:], biasB[:])
                nc.sync.dma_start(biasC_sb[:], biasC[:])
                nc.sync.dma_start(bdp_sb[:], bdp[:].rearrange("c o p -> o c p"))
                nc.vector.memset(ones8[:], 1.0)

            # p2t quad-row DMAs: hi on the HWDGE (sync) path, lo via the
            # otherwise-idle Pool engine's SWDGE; 4 src tiles per transfer to
            # amortize the ~1us per-DMA DGE cost.
            p2_rows = [(4 * i, 0) for i in range(4)] + [(4 * i, 1) for i in range(4)]
            p2_pos = [0]

            def emit_p2(n):
                for _ in range(n):
                    if p2_pos[0] >= len(p2_rows):
                        return
                    si, hl = p2_rows[p2_pos[0]]
                    p2_pos[0] += 1
                    if hl == 0:
                        nc.sync.dma_start(
                            p2thi_sb[:, si : si + 4, :],
                            p2thi[si : si + 4].rearrange("s p n -> p s n"))
                    else:
                        nc.gpsimd.dma_start(
                            p2tlo_sb[:, si : si + 4, :],
                            p2tlo[si : si + 4].rearrange("s p n -> p s n"))

            wd_pos = [0]
            wd_direct = [0]
            N_WD_DIRECT_GROUPS = 1

            def borrow_wd(pool, tag, n):
                """Land n decoder k-tiles in one group DMA into a dead tile's
                slot (tag must never be allocated again afterwards)."""
                k0 = wd_pos[0]
                assert k0 + n <= KDEC
                wd_pos[0] += n
                gt = pool.tile([128, n, DEC_SH], F16, tag=tag)
                nc.gpsimd.dma_start(
                    gt[:], wdp[k0 : k0 + n].rearrange("k p f -> p k f"))
                for i in range(n):
                    wd_tiles[k0 + i] = gt[:, i, :]

            def emit_wd(n):
                """Stream decoder-weight k-tiles into the dedicated pool,
                2 per transfer."""
                for _ in range(n):
                    if wd_direct[0] >= N_WD_DIRECT_GROUPS or wd_pos[0] + 2 > KDEC:
                        return
                    wd_direct[0] += 1
                    borrow_wd(wdpool, "wd", 2)

            def l0_stage(b, t_hi, t_lo):
                # ---- L0: t = x @ W01 (3-term fp8 DR) ----
                # x streams in node-tile groups: hi via HWDGE, lo via SWDGE.
                # Groups 0/1 land as singles (startup latency), the rest as
                # pair transfers; batch 1's stream is fully posted during
                # batch 0's loop.
                for g in range(NT // 2):
                    (ghi, glo), gi = prefetched.pop((b, g))
                    for k in range(2):
                        nt = 2 * g + k
                        pt = ps.tile([128, 512], F32, tag="ps")
                        first = True
                        for p in range(CT // 2):
                            nc.tensor.matmul(
                                pt[:], ghi[:, 2 * gi + k, 2 * p : 2 * p + 2, :],
                                w01hi_sb[:, 2 * p : 2 * p + 2, :],
                                start=first, stop=False, perf_mode=DRM,
                            )
                            first = False
                        for p in range(CT // 2):
                            nc.tensor.matmul(
                                pt[:], glo[:, 2 * gi + k, 2 * p : 2 * p + 2, :],
                                w01hi_sb[:, 2 * p : 2 * p + 2, :],
                                start=False, stop=False, perf_mode=DRM,
                            )
                        for p in range(CT // 2):
                            nc.tensor.matmul(
                                pt[:], ghi[:, 2 * gi + k, 2 * p : 2 * p + 2, :],
                                w01lo_sb[:, 2 * p : 2 * p + 2, :],
                                start=False, stop=(p == CT // 2 - 1),
                                perf_mode=DRM,
                            )
                        nc.scalar.activation(t_hi[:, nt, :], pt[:], ACTF.Copy,
                                             scale=KT)
                        nc.vector.scalar_tensor_tensor(
                            t_lo[:, nt, :], pt[:], KT, t_hi[:, nt, :],
                            ALU.mult, ALU.subtract,
                        )
                    if b == 0:
                        if g == 0:
                            post_x(0, 2, 2, "xhi", "xlo")
                        elif g == 1:
                            post_x(0, 4, 2, "xhi", "xlo")
                            # batch-1 pairs 1-2 use fresh slots (no WAR):
                            # early in the queue, they can never block it
                            post_x(1, 0, 2, "x1hi", "x1lo")
                        elif g == 2:
                            post_x(0, 6, 2, "xhi", "xlo")
                        elif g == 3:
                            emit_small_consts()
                            post_x(1, 2, 2, "x1hi", "x1lo")

            def prop_stage(b, src_hi, src_lo, bias_sb, nf, out_cb, wd_budget):
                # ---- z = P2 @ src + bias (single hi-bias DR matmul) ----
                # nf: number of 128-wide feature tiles in src (4 for stage A,
                # 2 for stage B). out_cb(dc, fj, pt) evicts the PSUM tile.
                for dc in range(4):
                    dsl = slice(dc * 512, (dc + 1) * 512)
                    for fj in range(nf):
                        fsl = slice(fj * 128, (fj + 1) * 128)
                        pt = ps.tile([128, 512], F32, tag="ps")
                        for sp in range(NT // 2):
                            ssl = slice(2 * sp, 2 * sp + 2)
                            nc.tensor.matmul(
                                pt[:], src_hi[:, ssl, fsl],
                                p2thi_sb[:, ssl, dsl],
                                start=(sp == 0), stop=False, perf_mode=DRM,
                            )
                        for sp in range(NT // 2):
                            ssl = slice(2 * sp, 2 * sp + 2)
                            nc.tensor.matmul(
                                pt[:], src_lo[:, ssl, fsl],
                                p2thi_sb[:, ssl, dsl],
                                start=False, stop=False, perf_mode=DRM,
                            )
                        for sp in range(NT // 2):
                            ssl = slice(2 * sp, 2 * sp + 2)
                            nc.tensor.matmul(
                                pt[:], src_hi[:, ssl, fsl],
                                p2tlo_sb[:, ssl, dsl],
                                start=False, stop=False, perf_mode=DRM,
                            )
                        # bias: all four hi/lo row-pairs packed into one
                        # K=4x2 DoubleRow matmul (exact same sum as the old
                        # two-instruction form)
                        nc.tensor.matmul(
                            pt[:], bias_sb[:, :, fsl], exd_sb[:, :, dsl],
                            start=False, stop=True, perf_mode=DRM,
                        )
                        out_cb(dc, fj, pt)
                        emit_wd(wd_budget)

            def stageA(b, t_hi, t_lo, h1hi, h1lo):
                def evict(dc, fj, pt):
                    dsl = slice(dc * 512, (dc + 1) * 512)
                    v = vpool.tile([128, 512], F16, tag="v")
                    nc.scalar.activation(v[:], pt[:], ACTF.Lrelu,
                                         scale=K1, alpha=ALPHA)
                    nc.scalar.activation(h1hi[:, fj, dsl], v[:], ACTF.Copy)
                    nc.vector.tensor_tensor(
                        h1lo[:, fj, dsl], v[:], h1hi[:, fj, dsl],
                        ALU.subtract)

                prop_stage(b, t_hi, t_lo, biasA_sb, 4, evict,
                           1 if b == 0 else 0)

            def w23_stage(h1hi, h1lo, t2hi, t2lo):
                # ---- W23: t2 = h1 @ W23 ----
                for nt in range(NT):
                    nsl = slice(nt * 128, (nt + 1) * 128)
                    pt = ps.tile([128, 512], F32, tag="ps")
                    for fp in range(2):
                        fsl = slice(2 * fp, 2 * fp + 2)
                        nc.tensor.matmul(
                            pt[:, 0:256], h1hi[:, fsl, nsl], w23hi_sb[:, fsl, :],
                            start=(fp == 0), stop=False, perf_mode=DRM,
                        )
                    for fp in range(2):
                        fsl = slice(2 * fp, 2 * fp + 2)
                        nc.tensor.matmul(
                            pt[:, 0:256], h1lo[:, fsl, nsl], w23hi_sb[:, fsl, :],
                            start=False, stop=False, perf_mode=DRM,
                        )
                    for fp in range(2):
                        fsl = slice(2 * fp, 2 * fp + 2)
                        nc.tensor.matmul(
                            pt[:, 0:256], h1hi[:, fsl, nsl], w23lo_sb[:, fsl, :],
                            start=False, stop=(fp == 1), perf_mode=DRM,
                        )
                    nc.scalar.activation(t2hi[:, nt, :], pt[:, 0:256], ACTF.Copy,
                                         scale=K2)
                    nc.vector.scalar_tensor_tensor(
                        t2lo[:, nt, :], pt[:, 0:256], K2, t2hi[:, nt, :],
                        ALU.mult, ALU.subtract,
                    )

            def stageB(b, t2hi, t2lo, h3):
                def evict(dc, fj, pt):
                    dsl = slice(dc * 512, (dc + 1) * 512)
                    nc.scalar.activation(h3[:, fj, dsl], pt[:], ACTF.Lrelu,
                                         alpha=ALPHA)

                prop_stage(b, t2hi, t2lo, biasB_sb, 2, evict, 0)

            def tail_stages(b, h3):
                # ---- W45: t3 = h3 @ W45 (fp16) ----
                # psd bank: no wait on stage-B eviction chains draining ps
                pt45 = psd.tile([128, 512], F32, tag="psd")
                for nt in range(NT):
                    nsl = slice(nt * 128, (nt + 1) * 128)
                    for cj in range(2):
                        nc.tensor.matmul(
                            pt45[:, nt * 3 : nt * 3 + 3],
                            h3[:, cj, nsl], w45_sb[:, cj, :],
                            start=(cj == 0), stop=(cj == 1),
                        )
                nc.scalar.activation(t3hi[:], pt45[:, 0 : NT * 3], ACTF.Copy,
                                     scale=K3)
                nc.vector.scalar_tensor_tensor(
                    t3lo[:], pt45[:, 0 : NT * 3], K3, t3hi[:],
                    ALU.mult, ALU.subtract,
                )

                # ---- stage C: z5 = P2 @ t3 + bias ; h5 = leaky(z5) ----
                psC = psd.tile([128, 512], F32, tag="psd")
                for di in range(NT):
                    dsl = slice(di * 128, (di + 1) * 128)
                    osl = slice(di * 3, di * 3 + 3)
                    for sp in range(NT // 2):
                        t3sl = t3hi[:, 6 * sp : 6 * sp + 6].rearrange(
                            "p (s c) -> p s c", s=2, c=3)
                        nc.tensor.matmul(
                            psC[:, osl], p2thi_sb[:, 2 * sp : 2 * sp + 2, dsl],
                            t3sl, start=(sp == 0), stop=False, perf_mode=DRM,
                        )
                    for sp in range(NT // 2):
                        t3sl = t3lo[:, 6 * sp : 6 * sp + 6].rearrange(
                            "p (s c) -> p s c", s=2, c=3)
                        nc.tensor.matmul(
                            psC[:, osl], p2thi_sb[:, 2 * sp : 2 * sp + 2, dsl],
                            t3sl, start=False, stop=False, perf_mode=DRM,
                        )
                    for sp in range(NT // 2):
                        t3sl = t3hi[:, 6 * sp : 6 * sp + 6].rearrange(
                            "p (s c) -> p s c", s=2, c=3)
                        nc.tensor.matmul(
                            psC[:, osl], p2tlo_sb[:, 2 * sp : 2 * sp + 2, dsl],
                            t3sl, start=False, stop=False, perf_mode=DRM,
                        )
                    nc.tensor.matmul(
                        psC[:, osl], exd_sb[:, :, dsl], biasC_sb[:],
                        start=False, stop=True, perf_mode=DRM,
                    )
                nc.scalar.activation(
                    h5_t[b][:].rearrange("p c d -> p d c"),
                    psC[:, 0 : NT * 3].rearrange("p (d c) -> p d c", d=NT, c=3),
                    ACTF.Lrelu, alpha=ALPHA,
                )
                nc.sync.dma_start(
                    cc_in[b][:].rearrange("c p n -> p c n"), h5_t[b][:])
                nc.gpsimd.collective_compute(
                    "AllGather",
                    ALU.bypass,
                    replica_groups=[list(range(N_CORES))],
                    ins=[cc_in[b][:]],
                    outs=[cc_out[b][:]],
                )

            def featT_dma(half):
                for c in range(3):
                    nc.sync.dma_start(
                        featT_sb[:, half : B : BL, c * NT : (c + 1) * NT],
                        cc_out[half][:, c].rearrange("core p n -> p core n"),
                    )

            def decoder_pass(half):
                # bias rides the accumulation as a K=1 matmul row per ct
                # chunk so the eviction is ONE ACT (six tiny serial ACTs
                # previously cost ~2.4us on the kernel tail)
                pd = psd.tile([128, 512], F32, tag="psd")
                for kt in range(KDEC):
                    rhs = featT_sb[:, half : B : BL, kt : kt + 1]
                    for ct in range(6):
                        nc.tensor.matmul(
                            pd[:, ct * 8 : (ct + 1) * 8],
                            wd_tiles[kt][:, ct * 128 : (ct + 1) * 128],
                            rhs, start=(kt == 0), stop=False,
                        )
                for ct in range(6):
                    nc.tensor.matmul(
                        pd[:, ct * 8 : (ct + 1) * 8],
                        bdp_sb[:, ct, :], ones8[:],
                        start=False, stop=(ct == 5),
                    )
                nc.scalar.activation(
                    y_sb[:, :, half : B : BL].rearrange("p c b -> p (c b)"),
                    pd[:, 0:48], ACTF.Tanh, scale=KD,
                )

            # ---- emission schedule ----
            t_hi0 = tpool.tile([128, NT, 512], F8, tag="t_hi0")
            t_lo0 = tpool.tile([128, NT, 512], F8, tag="t_lo0")
            t_hi1 = tpool.tile([128, NT, 512], F8, tag="t_hi1")
            t_lo1 = tpool.tile([128, NT, 512], F8, tag="t_lo1")

            l0_stage(0, t_hi0, t_lo0)
            # p2t streams behind batch-0 x and batch-1's first two pairs;
            # batch-1's WAR-gated pairs go after p2t in queue order (their
            # slot-reuse waits fire mid-L0(b1) and would head-of-line-block
            # p2t otherwise; arriving bus-interleaved with p2t is in time).
            emit_p2(8)
            post_x(1, 4, 2, "x1hi", "x1lo")
            post_x(1, 6, 2, "x1hi", "x1lo")
            l0_stage(1, t_hi1, t_lo1)
            # x tiles are dead from here: 24 decoder k-tiles into their slots
            for tag in ("xhi", "xlo", "x1hi"):
                borrow_wd(xpool, tag, 4)
                borrow_wd(xpool, tag, 4)
            borrow_wd(xpool, "x1lo", 2)

            h1hi = h1pool.tile([128, 4, N], F8, tag="h1hi")
            h1lo = h1pool.tile([128, 4, N], F8, tag="h1lo")
            stageA(0, t_hi0, t_lo0, h1hi, h1lo)
            emit_wd(N_WD_DIRECT_GROUPS)  # any remainder
            borrow_wd(tpool, "t_hi0", 5)
            borrow_wd(tpool, "t_lo0", 5)

            t2hi = t2pool.tile([128, NT, 256], F8, tag="t2hi")
            t2lo = t2pool.tile([128, NT, 256], F8, tag="t2lo")
            w23_stage(h1hi, h1lo, t2hi, t2lo)
            h3 = h3pool.tile([128, 2, N], F16, tag="h3")
            stageB(0, t2hi, t2lo, h3)
            tail_stages(0, h3)
            featT_dma(0)

            h1hi = h1pool.tile([128, 4, N], F8, tag="h1hi")
            h1lo = h1pool.tile([128, 4, N], F8, tag="h1lo")
            stageA(1, t_hi1, t_lo1, h1hi, h1lo)
            borrow_wd(tpool, "t_hi1", 5)
            borrow_wd(tpool, "t_lo1", 5)
            assert wd_pos[0] == KDEC, wd_pos[0]

            t2hi = t2pool.tile([128, NT, 256], F8, tag="t2hi")
            t2lo = t2pool.tile([128, NT, 256], F8, tag="t2lo")
            w23_stage(h1hi, h1lo, t2hi, t2lo)
            h3 = h3pool.tile([128, 2, N], F16, tag="h3")
            stageB(1, t2hi, t2lo, h3)
            tail_stages(1, h3)
            decoder_pass(0)  # fills part of the cc1 wait
            featT_dma(1)
            decoder_pass(1)
            nc.sync.dma_start(y_out[:], y_sb[:])

    _split_multi_waits(nc)
    return nc


# ---------------------------------------------------------------------------
def _split8(a):
    hi = np.asarray(a, np.float32).astype(NPF8)
    lo = (np.asarray(a, np.float64) - hi.astype(np.float64)).astype(
        np.float32).astype(NPF8)
    return hi, lo


def _host_precompute(x, edges, Ws, bs, Wd, bd_np):
    edges = np.asarray(edges)
    src = edges[0].astype(np.int64)
    dst = edges[1].astype(np.int64)

    deg = np.bincount(dst, minlength=N).astype(np.float64) + 1.0
    isd = 1.0 / np.sqrt(deg)
    idg = 1.0 / deg

    P = np.zeros((N, N), np.float64)
    np.add.at(P, (dst, src), isd[src] * isd[dst])
    P[np.arange(N), np.arange(N)] += idg
    P2 = P @ P
    r = P.sum(axis=1)

    W0, W1, W2, W3, W4, W5 = [np.asarray(w, np.float64) for w in Ws]
    b0, b1, b2, b3, b4, b5 = [np.asarray(b, np.float64) for b in bs]
    W01 = W0 @ W1
    W23 = W2 @ W3
    W45 = W4 @ W5
    a1 = b0 @ W1
    a3 = b2 @ W3
    a5 = b4 @ W5

    # p2t[si, p, d] = P2[d, si*128+p] * SP2
    p2t_s = np.ascontiguousarray((P2.T * SP2).reshape(NT, 128, N))
    p2thi, p2tlo = _split8(p2t_s)

    w01_pad = np.zeros((C_PAD, 512), np.float64)
    w01_pad[:C_IN] = W01
    w01hi, w01lo = _split8((w01_pad * SW01).reshape(CT, CP, 512))
    w23hi, w23lo = _split8((W23 * SW23).reshape(4, 128, 256))
    w45_np = W45.reshape(2, 128, 3).astype(NPF16)

    # Packed bias operands: one K=4x2 DoubleRow matmul computes
    #   (a_hi+a_lo)(r_hi+r_lo) + (b_hi+b_lo)*SEXD
    # exactly as the old two-matmul form.
    rhi, rlo = _split8(r * SEXD)
    one8 = np.full(N, SEXD, np.float32).astype(NPF8)
    exdp_np = np.zeros((4, 2, N), NPF8)
    exdp_np[0] = np.stack([rhi, rlo])
    exdp_np[1] = np.stack([rhi, rlo])
    exdp_np[2] = np.stack([one8, one8])
    exdp = np.ascontiguousarray(exdp_np)

    def bias4(a, bvec, s):
        ahi, alo = _split8(np.asarray(a) * s)
        bhi, blo = _split8(np.asarray(bvec) * s)
        out = np.zeros((4, 2, len(ahi)), NPF8)
        out[0] = np.stack([ahi, ahi])
        out[1] = np.stack([alo, alo])
        out[2] = np.stack([bhi, blo])
        return np.ascontiguousarray(out)

    biasA_np = bias4(a1, b1, SBA)
    biasB_np = bias4(a3, b3, SBB)
    biasC_np = bias4(a5, b5, SBC)

    # x: pad channels, scale, split; layout [BL,NT,p=chan,CT,j=node]
    x_np = np.asarray(x, np.float32)
    x_pad = np.zeros((B, N, C_PAD), np.float32)
    x_pad[:, :, :C_IN] = x_np * SX
    xt_all = np.ascontiguousarray(
        x_pad.reshape(B, NT, 128, CT, CP).transpose(0, 1, 4, 3, 2))
    xhi_all, xlo_all = _split8(xt_all)

    # decoder: permuted rows j' = c*2048 + node
    Wd_np = np.asarray(Wd, np.float64)
    node = np.arange(N)
    rows = np.concatenate([node * 3 + c for c in range(3)])  # j' -> orig j
    Wd_perm = Wd_np[rows]  # [6144, 6144] in j' order
    bd_full = np.asarray(bd_np, np.float64)

    shared = {
        "p2thi": p2thi, "p2tlo": p2tlo,
        "w01hi": w01hi, "w01lo": w01lo,
        "w23hi": w23hi, "w23lo": w23lo,
        "w45p": w45_np,
        "exdp": exdp,
        "biasA": biasA_np, "biasB": biasB_np, "biasC": biasC_np,
    }
    in_maps = []
    for c in range(N_CORES):
        wd_c = np.ascontiguousarray(
            Wd_perm[:, c * DEC_SH : (c + 1) * DEC_SH]
            .reshape(KDEC, 128, DEC_SH).astype(NPF16))
        bd_c = np.ascontiguousarray(
            (bd_full[c * DEC_SH : (c + 1) * DEC_SH] * (SCT3 * SP2))
            .reshape(6, 1, 128).astype(NPF16))  # [6, 1, 128], pre-divided by KD
        m = dict(shared)
        m["xhi"] = xhi_all[c * BL : (c + 1) * BL]
        m["xlo"] = xlo_all[c * BL : (c + 1) * BL]
        m["wdp"] = wd_c
        m["bdp"] = bd_c
        in_maps.append(m)
    return in_maps


_NC_CACHE = {}


def kernel(**inputs) -> np.ndarray:
    x = inputs["x"]
    edges = inputs["edges"]
    Ws = [inputs[f"W{i}"] for i in range(6)]
    bs = [inputs[f"b{i}"] for i in range(6)]
    Wd = inputs["Wd"]
    bd_np = inputs["bd"]

    in_maps = _host_precompute(x, edges, Ws, bs, Wd, bd_np)

    if "nc" not in _NC_CACHE:
        _NC_CACHE["nc"] = _build_program()
    nc = _NC_CACHE["nc"]

    res = run_bass_kernel_spmd(nc, in_maps, list(range(N_CORES)))

    out = np.zeros((B, D_DEC), np.float32)
    for c in range(N_CORES):
        y = res.results[c]["y"]  # [128, 6, 16]
        out[:, c * DEC_SH : (c + 1) * DEC_SH] = (
            0.1 * y.transpose(2, 1, 0).reshape(B, DEC_SH))
    return out.reshape(B, N, 3)

